# revision 24
# baseline (speedup 1.0000x reference)
"""AdaptiveTokenMerger (ToMe block + merger) TRN2 Bass kernel.

Data-parallel over batch: 8 samples -> 8 NeuronCores, one sample per core.
Per-core pipeline (sample x [1024, 1024]):
  A (f32, ranking-critical): LN1 -> qkv -> MHA (transposed-softmax with the
    denominator folded in as an appended ones-column of v) -> Wo -> x_attn
  B: metric scores -> node_max/argmax -> ranks via pairwise comparisons ->
    dst scatter-add expressed as a one-hot matmul
  C (f32r): MLP over rows [x1_even(512); dst_new(512)], fused W1/W2 per
    token-quarter, output accumulated in PSUM across all 32 W1 column tiles
  D: pooling as a rank-dependent one-hot matmul -> Wp -> combined = 3q
  E (f32r): multi-query attention  F (f32r): FFN -> out [504, 1024]

Precision: everything upstream of the rank/argmax decisions is true fp32
(4 cyc/row on PE); post-merge matmuls use float32r (TF32-ish, 1 cyc/row).

PSUM budget (8 banks): BIGA/BIGB [128,1024] (2+2), MM [128,512] x2 (2),
AV [128,4,128] x2 (2).
"""
import numpy as np

import concourse.bass as bass
import concourse.tile as tile
from concourse import bacc, mybir
from concourse.bass import ts
from concourse.bass_utils import run_bass_kernel_spmd
from concourse.masks import make_identity

F32 = mybir.dt.float32
F32R = mybir.dt.float32r
U32 = mybir.dt.uint32

N, C, H = 1024, 1024, 16
R = 16
DH = C // H          # 64
NE = N // 2          # 512
NP = (N - R) // 2    # 504
PP = 126             # pooled tokens per partition tile
KT = C // 128        # 8
AF = mybir.ActivationFunctionType
OP = mybir.AluOpType

DEBUG = False


def build(debug=False):
    nc = bacc.Bacc("TRN2", target_bir_lowering=False, debug=False, num_devices=8)
    x_d = nc.dram_tensor("x", [N, C], F32, kind="ExternalInput").ap()
    wd = {}
    for name, shape in [
        ("g1", [C]), ("be1", [C]), ("Wqkv", [C, 3 * C]), ("bqkv", [3 * C]),
        ("Wo", [C, C]), ("bo", [C]), ("g2", [C]), ("be2", [C]),
        ("W1", [C, 4 * C]), ("bm1", [4 * C]), ("W2", [4 * C, C]), ("bm2", [C]),
        ("Wp", [C, C]), ("bp", [C]), ("Wq", [C, C]), ("bq", [C]),
        ("Wk", [C, DH]), ("bk", [DH]), ("Wv", [C, DH]), ("bv", [DH]),
        ("Wmo", [C, C]), ("bmo", [C]), ("Wf1", [C, 4 * C]), ("bf1", [4 * C]),
        ("Wf2", [4 * C, C]), ("bf2", [C]),
    ]:
        wd[name] = nc.dram_tensor(name, shape, F32, kind="ExternalInput").ap()
    out_d = nc.dram_tensor("out", [NP, C], F32, kind="ExternalOutput").ap()
    dbg = {}
    if debug:
        for name, shape in [
            ("dbg_xattn", [N, C]), ("dbg_nm", [NE]), ("dbg_rank", [NE]),
            ("dbg_nodeidx", [NE]), ("dbg_mlpin", [N, C]), ("dbg_mlpout", [N, C]),
            ("dbg_pooled", [NP, C]),
        ]:
            dbg[name] = nc.dram_tensor(name, shape, F32, kind="ExternalOutput").ap()
    with tile.TileContext(nc) as tc:
        _build_tile(nc, tc, x_d, wd, out_d, dbg)
    nc.compile()
    return nc


def _build_tile(nc, tc, x_d, wd, out_d, dbg):
    # DRAM spill buffers
    qkTd = nc.dram_tensor("qkTd", [2 * C, N], F32).ap()
    aoTd = nc.dram_tensor("aoTd", [C, N], F32).ap()
    h2d = nc.dram_tensor("h2d", [C, N], F32R).ap()
    x1d = nc.dram_tensor("x1d", [N, C], F32).ap()
    dstnd = nc.dram_tensor("dstnd", [NE, C], F32).ap()
    mod = nc.dram_tensor("mod", [N, C], F32R).ap()
    mqaTd = nc.dram_tensor("mqaTd", [C, NP], F32R).ap()

    pc = tc.alloc_tile_pool(name="const", bufs=1)
    psm = tc.alloc_tile_pool(name="small", bufs=1)
    pw = tc.alloc_tile_pool(name="wstream", bufs=2)
    pt = tc.alloc_tile_pool(name="tmp", bufs=2)
    pp = tc.alloc_tile_pool(name="psum", bufs=1, space="PSUM")

    _ct = {}

    def utile(pool, shape, dtype, tag, bufs=None):
        _ct[tag] = _ct.get(tag, 0) + 1
        kw = {"bufs": bufs} if bufs is not None else {}
        return pool.tile(shape, dtype, tag=tag, name=f"{tag}_{_ct[tag]}", **kw)

    def ps_bigA():
        return utile(pp, [128, 1024], F32, "BIGA")

    def ps_bigB():
        return utile(pp, [128, 1024], F32, "BIGB")

    def ps_mm():
        return utile(pp, [128, 512], F32, "MM", bufs=2)

    def ps_av():
        return utile(pp, [128, 4, 128], F32, "AV", bufs=2)

    # ---------- constants ----------
    ident = pc.tile([128, 128], F32)
    make_identity(nc, ident[:])
    ones_col = pc.tile([1, 128], F32)
    nc.gpsimd.memset(ones_col[:], 1.0)
    piota = pc.tile([128, 1], F32)
    nc.gpsimd.iota(piota[:], [[0, 1]], channel_multiplier=1,
                   allow_small_or_imprecise_dtypes=True)
    iota512_row = pc.tile([1, 512], F32)
    nc.gpsimd.iota(iota512_row[:], [[1, 512]], channel_multiplier=0,
                   allow_small_or_imprecise_dtypes=True)
    iota504_row = pc.tile([1, 504], F32)
    nc.gpsimd.iota(iota504_row[:], [[1, 504]], channel_multiplier=0,
                   allow_small_or_imprecise_dtypes=True)

    def bcast_row(row_ap, n, tag, pool, scale=1.0):
        t = utile(pool, [128, n], F32, tag)
        for c0 in range(0, n, 512):
            cw = min(512, n - c0)
            p = ps_mm()
            nc.tensor.matmul(p[:, :cw], ones_col[:], row_ap[:, c0:c0 + cw],
                             start=True, stop=True)
            if scale == 1.0:
                nc.vector.tensor_copy(t[:, c0:c0 + cw], p[:, :cw])
            else:
                nc.vector.tensor_scalar_mul(t[:, c0:c0 + cw], p[:, :cw], scale)
        return t

    def load_row(dram_ap, n, tag, pool):
        t = utile(pw, [1, n], F32, "rowstg", bufs=2)
        nc.sync.dma_start(t[:], dram_ap[None, :])
        return t

    def brow(name, pool, scale=1.0):
        n = wd[name].shape[0]
        return bcast_row(load_row(wd[name], n, name + "_r", pool), n,
                         name + "_b", pool, scale)

    def bcol(name, pool, scale=1.0):
        n = wd[name].shape[0]
        t = utile(pool, [128, n // 128], F32, name + "_c")
        nc.sync.dma_start(t[:], wd[name].rearrange("(t p) -> p t", p=128))
        if scale != 1.0:
            nc.vector.tensor_scalar_mul(t[:], t[:], scale)
        return t

    IOTA512B = bcast_row(iota512_row[:], 512, "iota512b", pc)
    IOTA504B = bcast_row(iota504_row[:], 504, "iota504b", pc)

    def transpose_blocks(src_tiles, dst, n_rows, n_cols):
        """dst[c, r] = src[r, c]; dst is tile-list or sink(bj, bi, pf, cw, rw)."""
        for bi in range((n_rows + 127) // 128):
            rw = min(128, n_rows - bi * 128)
            for bj in range((n_cols + 127) // 128):
                cw = min(128, n_cols - bj * 128)
                p = ps_av()
                pf = p.rearrange("p a b -> p (a b)")
                nc.tensor.transpose(pf[:cw, :rw],
                                    src_tiles[bi][:rw, bj * 128:bj * 128 + cw],
                                    ident[:rw, :rw])
                if callable(dst):
                    dst(bj, bi, pf, cw, rw)
                else:
                    nc.vector.tensor_copy(dst[bj][:cw, bi * 128:bi * 128 + rw],
                                          pf[:cw, :rw])

    def refined_rsqrt_recip(vv, tag):
        """returns 1/sqrt(vv) with one Newton step on sqrt (ACT sqrt is loose)."""
        s0 = utile(psm, [128, 1], F32, tag + "_s0")
        nc.scalar.sqrt(s0[:], vv[:])
        r0 = utile(psm, [128, 1], F32, tag + "_r0")
        nc.vector.reciprocal(r0[:], s0[:])
        t = utile(psm, [128, 1], F32, tag + "_t")
        nc.vector.tensor_tensor(t[:], vv[:], r0[:], OP.mult)
        nc.vector.tensor_tensor(t[:], t[:], s0[:], OP.add)
        nc.vector.tensor_scalar_mul(t[:], t[:], 0.5)
        rr = utile(psm, [128, 1], F32, tag + "_rr")
        nc.vector.reciprocal(rr[:], t[:])
        return rr

    def layer_norm(src, dst, gb, bb):
        m = utile(psm, [128, 1], F32, "ln_m")
        nc.vector.reduce_sum(m[:], src[:, :C], axis=mybir.AxisListType.X)
        nc.vector.tensor_scalar_mul(m[:], m[:], 1.0 / C)
        xc = utile(pt, [128, C], F32, "ln_xc")
        nc.vector.tensor_scalar(xc[:], src[:, :C], m[:], None, OP.subtract)
        ss = utile(psm, [128, 1], F32, "ln_ss")
        nc.scalar.activation(dst[:, :C], xc[:], AF.Square, accum_out=ss[:])
        v = utile(psm, [128, 1], F32, "ln_v")
        nc.vector.tensor_scalar(v[:], ss[:], 1.0 / C, 1e-5, OP.mult, OP.add)
        rstd = refined_rsqrt_recip(v, "ln")
        nc.vector.tensor_scalar(dst[:, :C], xc[:], rstd[:], None, OP.mult)
        nc.vector.tensor_tensor(dst[:, :C], dst[:, :C], gb[:], OP.mult)
        nc.vector.tensor_tensor(dst[:, :C], dst[:, :C], bb[:], OP.add)

    # ================= Stage A: LN1 -> hT =================
    pbA = tc.alloc_tile_pool(name="biasA", bufs=1)
    pHT = tc.alloc_tile_pool(name="pHT", bufs=1)
    pVP = tc.alloc_tile_pool(name="pVP", bufs=1)
    pAttn = tc.alloc_tile_pool(name="pAttn", bufs=1)

    g1b = brow("g1", pbA)
    be1b = brow("be1", pbA)
    hT = [utile(pHT, [128, N], F32, f"hT{k}") for k in range(8)]
    ht = []
    for i in range(8):
        xt = utile(pt, [128, C], F32, "xin")
        nc.sync.dma_start(xt[:], x_d[ts(i, 128), :])
        h = utile(pt, [128, C], F32, "ht", bufs=4)
        layer_norm(xt, h, g1b, be1b)
        ht.append(h)
    transpose_blocks(ht, hT, N, C)

    # ===== qk^T -> qkTd (DRAM) ; v_pad (SBUF) =====
    bqkT = bcol("bqkv", pbA)
    for mp in range(8):
        accq = ps_bigA()
        acck = ps_bigB()
        for k in range(KT):
            wq = utile(pw, [128, 128], F32, "wqkb", bufs=4)
            nc.sync.dma_start(wq[:], wd["Wqkv"][ts(k, 128), ts(mp, 128)])
            wk = utile(pw, [128, 128], F32, "wqkb", bufs=4)
            nc.sync.dma_start(wk[:],
                              wd["Wqkv"][ts(k, 128), C + mp * 128:C + (mp + 1) * 128])
            for n2 in range(2):
                nc.tensor.matmul(accq[:, ts(n2, 512)], wq[:], hT[k][:, ts(n2, 512)],
                                 start=(k == 0), stop=(k == KT - 1))
                nc.tensor.matmul(acck[:, ts(n2, 512)], wk[:], hT[k][:, ts(n2, 512)],
                                 start=(k == 0), stop=(k == KT - 1))
        stgq = utile(pAttn, [128, N], F32, "qkstg", bufs=2)
        nc.scalar.activation(stgq[:], accq[:], AF.Identity, bias=bqkT[:, mp:mp + 1])
        nc.sync.dma_start(qkTd[ts(mp, 128), :], stgq[:])
        stgk = utile(pAttn, [128, N], F32, "qkstg", bufs=2)
        nc.scalar.activation(stgk[:], acck[:], AF.Identity,
                             bias=bqkT[:, 8 + mp:9 + mp])
        nc.sync.dma_start(qkTd[C + mp * 128:C + (mp + 1) * 128, :], stgk[:])

    bvqkvb = bcast_row(load_row(wd["bqkv"][2 * C:], C, "bvq_r", pbA), C,
                       "bvq_b", pbA)
    v_pad = [utile(pVP, [128, H, DH + 1], F32, f"vp{j}") for j in range(8)]
    for j in range(8):
        nc.vector.memset(v_pad[j][:, :, DH:DH + 1], 1.0)
        acc = ps_bigA()
        for k in range(KT):
            wv = utile(pw, [128, C], F32, "ws4k")
            nc.sync.dma_start(wv[:], wd["Wqkv"][ts(k, 128), 2 * C:])
            for n2 in range(2):
                nc.tensor.matmul(acc[:, ts(n2, 512)], hT[k][:, ts(j, 128)],
                                 wv[:, ts(n2, 512)],
                                 start=(k == 0), stop=(k == KT - 1))
        for h in range(H):
            nc.vector.tensor_tensor(v_pad[j][:, h, :DH], acc[:, ts(h, DH)],
                                    bvqkvb[:, ts(h, DH)], OP.add)

    # ===== attention: stream kT/qT per head; out -> aoTd (already c-major) ==
    # out[dh|sum, i] = v_pad[j].T @ expT[j, i], accumulated over j-tiles.
    for h in range(H):
        kth = utile(pAttn, [64, N], F32, "kth", bufs=2)
        nc.sync.dma_start(kth[:], qkTd[C + h * 64:C + h * 64 + 64, :])
        qth = utile(pAttn, [64, N], F32, "qth", bufs=2)
        nc.sync.dma_start(qth[:], qkTd[h * 64:h * 64 + 64, :])
        av = [ps_av().rearrange("p a b -> p (a b)") for _ in range(2)]
        for j in range(8):
            for n2 in range(2):
                sp = ps_mm()
                nc.tensor.matmul(sp[:], kth[:, ts(j, 128)], qth[:, ts(n2, 512)],
                                 start=True, stop=True)
                et = utile(pAttn, [128, 512], F32, "exp", bufs=3)
                nc.scalar.activation(et[:], sp[:], AF.Exp, scale=float(DH ** -0.5))
                nc.tensor.matmul(av[n2][:DH + 1, :512], v_pad[j][:, h, :], et[:],
                                 start=(j == 0), stop=(j == 7))
        for n2 in range(2):
            rrow = utile(pAttn, [1, 512], F32, "rrow", bufs=2)
            nc.vector.reciprocal(rrow[:], av[n2][DH:DH + 1, :512])
            rb = ps_mm()
            nc.tensor.matmul(rb[:DH, :512], ones_col[:, :DH], rrow[:],
                             start=True, stop=True)
            rbs = utile(pAttn, [64, 512], F32, "rbs", bufs=2)
            nc.vector.tensor_copy(rbs[:], rb[:DH, :512])
            stg = utile(pAttn, [64, 512], F32, "aot_stg", bufs=2)
            nc.vector.tensor_tensor(stg[:], av[n2][:DH, :512], rbs[:],
                                    OP.mult)
            nc.sync.dma_start(aoTd[h * 64:h * 64 + 64, ts(n2, 512)], stg[:])
    pAttn.release()
    pVP.release()
    pHT.release()
    pbA.release()

    # ================= Wo -> x_attn, x1 (-> DRAM), metric =================
    pB2 = tc.alloc_tile_pool(name="pB2", bufs=1)
    bob = brow("bo", pB2)
    xa = [utile(pB2, [128, C], F32, f"xa{m}") for m in range(8)]
    rn = psm.tile([128, 8], F32)
    for m in range(8):
        acc = ps_bigA()
        for k in range(KT):
            ao = utile(pw, [128, 128], F32, "wqkb", bufs=4)
            nc.sync.dma_start(ao[:], aoTd[ts(k, 128), ts(m, 128)])
            wo = utile(pw, [128, C], F32, "ws4k")
            nc.sync.dma_start(wo[:], wd["Wo"][ts(k, 128), :])
            for n2 in range(2):
                nc.tensor.matmul(acc[:, ts(n2, 512)], ao[:], wo[:, ts(n2, 512)],
                                 start=(k == 0), stop=(k == KT - 1))
        nc.vector.tensor_tensor(xa[m][:], acc[:], bob[:], OP.add)
        xt = utile(pt, [128, C], F32, "xin")
        nc.sync.dma_start(xt[:], x_d[ts(m, 128), :])
        x1stg = utile(pw, [128, C], F32, "x1stg", bufs=2)
        nc.vector.tensor_tensor(x1stg[:], xa[m][:], xt[:], OP.add)
        nc.sync.dma_start(x1d[ts(m, 128), :], x1stg[:])
        ss = utile(psm, [128, 1], F32, "nrm_ss")
        sq = utile(pt, [128, C], F32, "ln_xc")
        nc.scalar.activation(sq[:], xa[m][:], AF.Square, accum_out=ss[:])
        rr = refined_rsqrt_recip(ss, "nrm")
        nc.vector.tensor_copy(rn[:, m:m + 1], rr[:])
        nc.vector.tensor_scalar(xa[m][:], xa[m][:], rn[:, m:m + 1], None, OP.mult)
        if dbg:
            nc.sync.dma_start(dbg["dbg_xattn"][ts(m, 128), :], xa[m][:])

    # ===== de-interleave metric -> maT/mbT; scores; node stats; ranks =====
    pB3 = tc.alloc_tile_pool(name="pB3", bufs=1)
    xae = [utile(pB3, [128, C], F32, f"xae{m}") for m in range(4)]
    xao = [utile(pB3, [128, C], F32, f"xao{m}") for m in range(4)]
    for m in range(4):
        nc.sync.dma_start(xae[m][:64, :], xa[2 * m][0:128:2, :])
        nc.sync.dma_start(xae[m][64:, :], xa[2 * m + 1][0:128:2, :])
        nc.sync.dma_start(xao[m][:64, :], xa[2 * m][1:128:2, :])
        nc.sync.dma_start(xao[m][64:, :], xa[2 * m + 1][1:128:2, :])
    pB4 = tc.alloc_tile_pool(name="pB4", bufs=1)
    maT = [utile(pB4, [128, NE], F32, f"maT{k}") for k in range(8)]
    mbT = [utile(pB4, [128, NE], F32, f"mbT{k}") for k in range(8)]
    transpose_blocks(xae, maT, NE, C)
    transpose_blocks(xao, mbT, NE, C)

    nm_t = psm.tile([128, 4], F32)
    ni_t = psm.tile([128, 4], F32)
    for m in range(4):
        acc = ps_bigA()
        for k in range(KT):
            nc.tensor.matmul(acc[:, :512], maT[k][:, ts(m, 128)], mbT[k][:],
                             start=(k == 0), stop=(k == KT - 1))
        mx8 = utile(psm, [128, 8], F32, "mx8")
        ix8 = utile(psm, [128, 8], U32, "ix8")
        nc.vector.max_with_indices(mx8[:], ix8[:], acc[:, :512])
        nc.vector.tensor_copy(nm_t[:, m:m + 1], mx8[:, 0:1])
        nc.vector.tensor_copy(ni_t[:, m:m + 1], ix8[:, 0:1])

    nm_row = utile(pB4, [1, 512], F32, "nm_row")
    for m in range(4):
        p = ps_av()
        pf = p.rearrange("p a b -> p (a b)")
        nc.tensor.transpose(pf[:1, :128], nm_t[:, m:m + 1], ident[:])
        nc.vector.tensor_copy(nm_row[:, ts(m, 128)], pf[:1, :128])
    NMB = bcast_row(nm_row[:], 512, "nmb", pB4)

    rank_t = psm.tile([128, 4], F32)
    for m in range(4):
        gt = utile(pB4, [128, 512], F32, "rk_gt")
        nc.vector.tensor_scalar(gt[:], NMB[:], nm_t[:, m:m + 1], None, OP.is_gt)
        eq = utile(pB4, [128, 512], F32, "rk_eq")
        nc.vector.tensor_scalar(eq[:], NMB[:], nm_t[:, m:m + 1], None, OP.is_equal)
        flt = utile(pB4, [128, 512], F32, "rk_flt")
        pio = utile(psm, [128, 1], F32, "rk_pio")
        nc.vector.tensor_scalar_add(pio[:], piota[:], float(128 * m))
        nc.vector.tensor_scalar(flt[:], IOTA512B[:], pio[:], None, OP.is_lt)
        nc.vector.tensor_tensor(eq[:], eq[:], flt[:], OP.mult)
        nc.vector.tensor_tensor(gt[:], gt[:], eq[:], OP.add)
        nc.vector.reduce_sum(rank_t[:, m:m + 1], gt[:], axis=mybir.AxisListType.X)
    if dbg:
        for (tt, nme) in [(nm_t, "dbg_nm"), (rank_t, "dbg_rank"),
                          (ni_t, "dbg_nodeidx")]:
            nc.sync.dma_start(dbg[nme].rearrange("(m p) -> p m", p=128), tt[:])
    pB4.release()
    pB3.release()
    pB2.release()

    # ================= dst merge (x1 from DRAM; dstn -> DRAM) =============
    pM = tc.alloc_tile_pool(name="pM", bufs=1)
    x1e = [utile(pM, [128, C + 8], F32, f"x1e{m}") for m in range(4)]
    x1o = [utile(pM, [128, C], F32, f"x1o{m}") for m in range(4)]
    for m in range(4):
        nc.vector.memset(x1e[m][:, C:C + 1], 1.0)
        nc.sync.dma_start(x1e[m][:, :C], x1d[256 * m:256 * m + 256:2, :])
        nc.sync.dma_start(x1o[m][:], x1d[256 * m + 1:256 * m + 256:2, :])
    st = [utile(pM, [128, 512], F32, f"st{m}") for m in range(4)]
    for m in range(4):
        msk = utile(psm, [128, 1], F32, "st_m")
        nc.vector.tensor_scalar(msk[:], rank_t[:, m:m + 1], float(R) - 0.5, None,
                                OP.is_lt)
        nc.vector.tensor_scalar(st[m][:], IOTA512B[:], ni_t[:, m:m + 1], None,
                                OP.is_equal)
        nc.vector.tensor_scalar(st[m][:], st[m][:], msk[:], None, OP.mult)
    for m in range(4):
        acc = ps_bigA()
        cacc = ps_av()
        for k in range(4):
            for n2 in range(2):
                nc.tensor.matmul(acc[:, ts(n2, 512)], st[k][:, ts(m, 128)],
                                 x1e[k][:, n2 * 512:n2 * 512 + 512],
                                 start=(k == 0), stop=(k == 3))
            nc.tensor.matmul(cacc[:, 0, :1], st[k][:, ts(m, 128)],
                             x1e[k][:, C:C + 1], start=(k == 0), stop=(k == 3))
        cnt = utile(psm, [128, 1], F32, "cnt")
        nc.vector.tensor_scalar_add(cnt[:], cacc[:, 0, 0:1], 1.0)
        rec = utile(psm, [128, 1], F32, "cntr")
        nc.vector.reciprocal(rec[:], cnt[:])
        dst_stg = utile(pM, [128, C], F32, "dst_stg", bufs=2)
        nc.vector.tensor_tensor(dst_stg[:], acc[:], x1o[m][:], OP.add)
        nc.vector.tensor_scalar(dst_stg[:], dst_stg[:], rec[:], None, OP.mult)
        nc.sync.dma_start(dstnd[ts(m, 128), :], dst_stg[:])

    # ========== MLP (f32r), rows streamed from DRAM ==========
    def row_src_ap(i):
        if i < 4:
            return x1d[256 * i:256 * i + 256:2, :]
        return dstnd[ts(i - 4, 128), :]

    pC3 = tc.alloc_tile_pool(name="pC3", bufs=1)
    g2b = brow("g2", pC3)
    be2b = brow("be2", pC3)
    h2 = []
    for i in range(8):
        rsrc = utile(pt, [128, C], F32, "xin")
        nc.sync.dma_start(rsrc[:], row_src_ap(i))
        h = utile(pt, [128, C], F32, "ht", bufs=4)
        layer_norm(rsrc, h, g2b, be2b)
        h2.append(h)
        if dbg:
            nc.sync.dma_start(dbg["dbg_mlpin"][ts(i, 128), :], rsrc[:])

    def h2_sink(bj, bi, pf, cw, rw):
        stg = utile(pC3, [128, 128], F32R, "h2stg", bufs=2)
        nc.vector.tensor_copy(stg[:cw, :rw], pf[:cw, :rw])
        nc.sync.dma_start(h2d[bj * 128:bj * 128 + cw, bi * 128:bi * 128 + rw],
                          stg[:cw, :rw])

    transpose_blocks(h2, h2_sink, N, C)
    pC3.release()
    pM.release()

    pC4 = tc.alloc_tile_pool(name="pC4", bufs=1)
    bm1T = bcol("bm1", pC4)
    bm2b = brow("bm2", pC4)
    h2q = [utile(pC4, [128, 256], F32R, f"h2q{k}") for k in range(8)]
    for q in range(4):
        for k in range(KT):
            nc.sync.dma_start(h2q[k][:], h2d[ts(k, 128), q * 256:q * 256 + 256])
        oacc = [ps_bigA(), ps_bigB()]     # out token tiles 2q, 2q+1
        for mt in range(32):
            yp = ps_mm()
            for k in range(KT):
                w1b = utile(pw, [128, 128], F32, "w1b", bufs=4)
                nc.sync.dma_start(w1b[:], wd["W1"][ts(k, 128), ts(mt, 128)])
                w1r = utile(pw, [128, 128], F32R, "w1r", bufs=4)
                nc.vector.tensor_copy(w1r[:], w1b[:])
                nc.tensor.matmul(yp[:, :256], w1r[:], h2q[k][:],
                                 start=(k == 0), stop=(k == KT - 1))
            g1t = utile(pC4, [128, 256], F32R, "g1t", bufs=3)
            nc.scalar.activation(g1t[:], yp[:, :256], AF.Gelu_apprx_tanh,
                                 bias=bm1T[:, mt:mt + 1])
            w2t = utile(pw, [128, C], F32, "ws4k")
            nc.sync.dma_start(w2t[:], wd["W2"][ts(mt, 128), :])
            w2r = utile(pC4, [128, C], F32R, "w2r", bufs=2)
            nc.vector.tensor_copy(w2r[:], w2t[:])
            for tl in range(2):
                for n2 in range(2):
                    nc.tensor.matmul(oacc[tl][:, ts(n2, 512)],
                                     g1t[:, tl * 128:tl * 128 + 128],
                                     w2r[:, ts(n2, 512)],
                                     start=(mt == 0), stop=(mt == 31))
        for tl in range(2):
            row = 2 * q + tl
            res = utile(pt, [128, C], F32, "xin")
            nc.sync.dma_start(res[:], row_src_ap(row))
            mf = utile(pC4, [128, C], F32, "mof", bufs=2)
            nc.vector.tensor_tensor(mf[:], oacc[tl][:], bm2b[:], OP.add)
            nc.vector.tensor_tensor(mf[:], mf[:], res[:], OP.add)
            mr = utile(pC4, [128, C], F32R, "mor", bufs=2)
            nc.vector.tensor_copy(mr[:], mf[:])
            nc.sync.dma_start(mod[ts(row, 128), :], mr[:])
            if dbg:
                nc.sync.dma_start(dbg["dbg_mlpout"][ts(row, 128), :], mf[:])
    pC4.release()

    # ================= Stage D: pooling + Wp -> combined^T =================
    pD = tc.alloc_tile_pool(name="pD", bufs=1)
    # ApT[p, f] = 0.5 iff source row p pools into output f:
    #   even block: base = rank[p]-16, match iff (2f - base) in {-1, 0}
    #   dst  block: base = d,          match iff (2(f-248) - base) in {-1, 0}
    iota2e = utile(pD, [128, 504], F32, "iota2e")
    nc.vector.tensor_scalar_mul(iota2e[:], IOTA504B[:], 2.0)
    apT = [utile(pD, [128, 504], F32R, f"apT{m}") for m in range(8)]
    for m in range(8):
        base = utile(psm, [128, 1], F32, "ap_r")
        if m < 4:
            nc.vector.tensor_scalar_add(base[:], rank_t[:, m:m + 1], -float(R))
        else:
            nc.vector.tensor_scalar_add(base[:], piota[:],
                                        float(128 * (m - 4) + NE - R))
        d1 = utile(pD, [128, 504], F32, "ap_d1")
        nc.vector.tensor_scalar(d1[:], iota2e[:], base[:], None, OP.subtract)
        a1 = utile(pD, [128, 504], F32, "ap_a1")
        nc.vector.tensor_scalar(a1[:], d1[:], -1.5, None, OP.is_ge)
        b1 = utile(pD, [128, 504], F32, "ap_b1")
        nc.vector.tensor_scalar(b1[:], d1[:], 0.5, None, OP.is_le)
        nc.vector.scalar_tensor_tensor(apT[m][:], a1[:], 0.5, b1[:],
                                       OP.mult, OP.mult)
    pooledT = [utile(pD, [128, NP], F32R, f"pooledT{k}") for k in range(8)]
    for m in range(4):
        acc = ps_bigA()
        for k in range(8):
            mob = utile(pw, [128, C], F32R, "mob", bufs=3)
            nc.sync.dma_start(mob[:], mod[ts(k, 128), :])
            for n2 in range(2):
                nc.tensor.matmul(acc[:PP, ts(n2, 512)],
                                 apT[k][:, m * PP:(m + 1) * PP],
                                 mob[:, ts(n2, 512)], start=(k == 0), stop=(k == 7))
        pst = utile(pD, [128, C], F32, "pstg", bufs=2)
        nc.vector.tensor_copy(pst[:PP, :], acc[:PP, :])
        if dbg:
            nc.sync.dma_start(dbg["dbg_pooled"][ts(m, PP), :], pst[:PP, :])
        for bj in range(8):
            p = ps_av()
            pf = p.rearrange("p a b -> p (a b)")
            nc.tensor.transpose(pf[:128, :PP], pst[:PP, ts(bj, 128)],
                                ident[:PP, :PP])
            nc.vector.tensor_copy(pooledT[bj][:, m * PP:(m + 1) * PP],
                                  pf[:128, :PP])

    pE = tc.alloc_tile_pool(name="pE", bufs=1)
    bp3T = bcol("bp", pE, scale=3.0)
    cmbTr = [utile(pE, [128, NP], F32R, f"cmbTr{m}") for m in range(8)]
    for m in range(8):
        acc = ps_mm()
        for k in range(KT):
            wb = utile(pw, [128, 128], F32, "w1b", bufs=4)
            nc.sync.dma_start(wb[:], wd["Wp"][ts(k, 128), ts(m, 128)])
            wr = utile(pw, [128, 128], F32R, "w1r", bufs=4)
            nc.vector.tensor_copy(wr[:], wb[:])
            nc.tensor.matmul(acc[:, :NP], wr[:], pooledT[k][:],
                             start=(k == 0), stop=(k == KT - 1))
        nc.scalar.activation(cmbTr[m][:], acc[:, :NP], AF.Identity,
                             bias=bp3T[:, m:m + 1], scale=3.0)

    # ================= Stage E: MQA =================
    bqT = bcol("bq", pE)
    mqT = [utile(pE, [128, NP], F32R, f"mqT{m}") for m in range(8)]
    for m in range(8):
        acc = ps_mm()
        for k in range(KT):
            wb = utile(pw, [128, 128], F32, "w1b", bufs=4)
            nc.sync.dma_start(wb[:], wd["Wq"][ts(k, 128), ts(m, 128)])
            wr = utile(pw, [128, 128], F32R, "w1r", bufs=4)
            nc.vector.tensor_copy(wr[:], wb[:])
            nc.tensor.matmul(acc[:, :NP], wr[:], cmbTr[k][:],
                             start=(k == 0), stop=(k == KT - 1))
        nc.scalar.activation(mqT[m][:], acc[:, :NP], AF.Identity,
                             bias=bqT[:, m:m + 1])

    wkv = utile(pE, [128, KT, 2 * DH], F32, "wkv")
    wkvr = utile(pE, [128, KT, 2 * DH], F32R, "wkvr")
    for k in range(KT):
        nc.sync.dma_start(wkv[:, k, :DH], wd["Wk"][ts(k, 128), :])
        nc.sync.dma_start(wkv[:, k, DH:], wd["Wv"][ts(k, 128), :])
    nc.vector.tensor_copy(wkvr[:], wkv[:])
    bkT = utile(pE, [64, 1], F32, "bkT")
    nc.sync.dma_start(bkT[:], wd["bk"][:, None])
    mkT = utile(pE, [128, NP], F32R, "mkT")
    macc = ps_mm()
    for k in range(KT):
        nc.tensor.matmul(macc[:64, :NP], wkvr[:, k, :DH], cmbTr[k][:],
                         start=(k == 0), stop=(k == KT - 1))
    mkf = utile(pE, [64, NP], F32, "mkf")
    nc.scalar.activation(mkf[:], macc[:64, :NP], AF.Identity, bias=bkT[:])
    nc.vector.tensor_copy(mkT[:64, :], mkf[:])
    nc.sync.dma_start(mkT[64:, :], mkT[:64, :])

    bvb = bcast_row(load_row(wd["bv"], DH, "bv_r", pE), DH, "bv_b", pE)
    mv_pad = [utile(pE, [128, DH + 1], F32, f"mvp{m}") for m in range(4)]
    for m in range(4):
        acc = ps_av()
        for k in range(KT):
            nc.tensor.matmul(acc[:PP, 0, :DH], cmbTr[k][:, m * PP:(m + 1) * PP],
                             wkvr[:, k, DH:], start=(k == 0), stop=(k == KT - 1))
        nc.vector.memset(mv_pad[m][:], 0.0)
        nc.vector.memset(mv_pad[m][:, DH:], 1.0)
        nc.vector.tensor_tensor(mv_pad[m][:PP, :DH], acc[:PP, 0, :DH], bvb[:PP, :],
                                OP.add)

    for h in range(H):
        po = (h % 2) * 64
        mqT_h = mqT[h // 2][po:po + 64, :]
        ep = []
        for mm in range(4):
            sp = ps_mm()
            nc.tensor.matmul(sp[:PP, :NP], mkT[po:po + 64, mm * PP:(mm + 1) * PP],
                             mqT_h[:], start=True, stop=True)
            et = utile(pE, [128, NP], F32, "e2", bufs=4)
            nc.scalar.activation(et[:PP, :], sp[:PP, :NP], AF.Exp,
                                 scale=float(DH ** -0.5))
            ep.append(et)
        av2 = ps_av().rearrange("p a b -> p (a b)")
        for mm in range(4):
            nc.tensor.matmul(av2[:DH + 1, :NP], mv_pad[mm][:PP, :],
                             ep[mm][:PP, :], start=(mm == 0), stop=(mm == 3))
        rrow = utile(pE, [1, NP], F32, "rrow2", bufs=2)
        nc.vector.reciprocal(rrow[:], av2[DH:DH + 1, :NP])
        rb = ps_mm()
        nc.tensor.matmul(rb[:DH, :NP], ones_col[:, :DH], rrow[:],
                         start=True, stop=True)
        rbs = utile(pE, [64, NP], F32, "rbs2", bufs=2)
        nc.vector.tensor_copy(rbs[:], rb[:DH, :NP])
        stg = utile(pE, [64, NP], F32R, "mqstg", bufs=2)
        nc.vector.tensor_tensor(stg[:], av2[:DH, :NP], rbs[:], OP.mult)
        nc.sync.dma_start(mqaTd[h * 64:h * 64 + 64, :], stg[:])
    pE.release()
    pD.release()

    # ================= Stage F: Wmo + FFN =================
    pF = tc.alloc_tile_pool(name="pF", bufs=1)
    mqaT = [utile(pF, [128, NP], F32R, f"mqaT{k}") for k in range(8)]
    for k in range(8):
        nc.sync.dma_start(mqaT[k][:, :NP], mqaTd[ts(k, 128), :])
    bmoT = bcol("bmo", pF)
    omoT = [utile(pF, [128, NP], F32R, f"omoT{m}") for m in range(8)]
    for m in range(8):
        acc = ps_mm()
        for k in range(KT):
            wb = utile(pw, [128, 128], F32, "w1b", bufs=4)
            nc.sync.dma_start(wb[:], wd["Wmo"][ts(k, 128), ts(m, 128)])
            wr = utile(pw, [128, 128], F32R, "w1r", bufs=4)
            nc.vector.tensor_copy(wr[:], wb[:])
            nc.tensor.matmul(acc[:, :NP], wr[:], mqaT[k][:],
                             start=(k == 0), stop=(k == KT - 1))
        nc.scalar.activation(omoT[m][:], acc[:, :NP], AF.Identity,
                             bias=bmoT[:, m:m + 1])

    bf1T = bcol("bf1", pF)
    bf2b = brow("bf2", pF)
    for half in range(2):
        t0 = half * 252
        oacc = [ps_bigA(), ps_bigB()]   # out token tiles 2*half, 2*half+1
        for kk in range(32):
            yp = ps_mm()
            for k in range(KT):
                wf1b = utile(pw, [128, 128], F32, "w1b", bufs=4)
                nc.sync.dma_start(wf1b[:], wd["Wf1"][ts(k, 128), ts(kk, 128)])
                wf1r = utile(pw, [128, 128], F32R, "w1r", bufs=4)
                nc.vector.tensor_copy(wf1r[:], wf1b[:])
                nc.tensor.matmul(yp[:, :252], wf1r[:], omoT[k][:, t0:t0 + 252],
                                 start=(k == 0), stop=(k == KT - 1))
            g2t = utile(pF, [128, 252], F32R, "g2t", bufs=3)
            nc.scalar.activation(g2t[:], yp[:, :252], AF.Silu,
                                 bias=bf1T[:, kk:kk + 1])
            wf2t = utile(pw, [128, C], F32, "ws4k")
            nc.sync.dma_start(wf2t[:], wd["Wf2"][ts(kk, 128), :])
            wf2r = utile(pF, [128, C], F32R, "wf2r", bufs=2)
            nc.vector.tensor_copy(wf2r[:], wf2t[:])
            for tl in range(2):
                for n2 in range(2):
                    nc.tensor.matmul(oacc[tl][:PP, ts(n2, 512)],
                                     g2t[:, tl * 126:tl * 126 + 126],
                                     wf2r[:, ts(n2, 512)],
                                     start=(kk == 0), stop=(kk == 31))
        for tl in range(2):
            row0 = (2 * half + tl) * PP
            of = utile(pF, [128, C], F32, "of", bufs=2)
            nc.vector.tensor_tensor(of[:PP, :], oacc[tl][:PP, :], bf2b[:PP, :],
                                    OP.add)
            nc.sync.dma_start(out_d[row0:row0 + PP, :], of[:PP, :])
    pF.release()
    for pool in (pt, pw, psm, pc, pp):
        pool.release()


_BUILT = None


def kernel(**inputs):
    global _BUILT
    if _BUILT is None:
        _BUILT = build(debug=DEBUG)
    nc = _BUILT
    x = np.ascontiguousarray(inputs["x"], dtype=np.float32)
    base = {k: np.ascontiguousarray(v, dtype=np.float32) for k, v in inputs.items()
            if k != "x"}
    in_maps = []
    for i in range(8):
        m = dict(base)
        m["x"] = x[i]
        in_maps.append(m)
    res = run_bass_kernel_spmd(nc, in_maps, core_ids=list(range(8)))
    out = np.stack([res.results[i]["out"] for i in range(8)], axis=0)
    return out.astype(np.float32)


# revision 38
# speedup vs baseline: 5816.0467x; 5816.0467x over previous
"""AdaptiveTokenMerger (ToMe block + merger) TRN2 Bass kernel.

Data-parallel over batch: 8 samples -> 8 NeuronCores, one sample per core.
Per-core pipeline (sample x [1024, 1024]):
  A (f32, ranking-critical): LN1 -> qkv -> MHA (transposed-softmax with the
    denominator folded in as an appended ones-column of v) -> Wo -> x_attn
  B: metric scores -> node_max/argmax -> ranks via pairwise comparisons ->
    dst scatter-add expressed as a one-hot matmul
  C (f32r): MLP over rows [x1_even(512); dst_new(512)], fused W1/W2 per
    token-quarter, output accumulated in PSUM across all 32 W1 column tiles
  D: pooling as a rank-dependent one-hot matmul -> Wp -> combined = 3q
  E (f32r): multi-query attention  F (f32r): FFN -> out [504, 1024]

Precision: everything upstream of the rank/argmax decisions is true fp32
(4 cyc/row on PE); post-merge matmuls use float32r (TF32-ish, 1 cyc/row).

PSUM budget (8 banks): BIGA/BIGB [128,1024] (2+2), MM [128,512] x2 (2),
AV [128,4,128] x2 (2).
"""
import numpy as np

import concourse.bass as bass
import concourse.tile as tile
from concourse import bacc, mybir
from concourse.bass import ts
from concourse.bass_utils import run_bass_kernel_spmd
from concourse.masks import make_identity

F32 = mybir.dt.float32
F32R = mybir.dt.float32r
U32 = mybir.dt.uint32

N, C, H = 1024, 1024, 16
R = 16
DH = C // H          # 64
NE = N // 2          # 512
NP = (N - R) // 2    # 504
PP = 126             # pooled tokens per partition tile
KT = C // 128        # 8
AF = mybir.ActivationFunctionType
OP = mybir.AluOpType

DEBUG = False


def build(debug=False):
    nc = bacc.Bacc("TRN2", target_bir_lowering=False, debug=False, num_devices=8)
    x_d = nc.dram_tensor("x", [N, C], F32, kind="ExternalInput").ap()
    wd = {}
    for name, shape in [
        ("g1", [C]), ("be1", [C]), ("Wqkv", [C, 3 * C]), ("bqkv", [3 * C]),
        ("Wo", [C, C]), ("bo", [C]), ("g2", [C]), ("be2", [C]),
        ("W1", [C, 4 * C]), ("bm1", [4 * C]), ("W2", [4 * C, C]), ("bm2", [C]),
        ("Wp", [C, C]), ("bp", [C]), ("Wq", [C, C]), ("bq", [C]),
        ("Wk", [C, DH]), ("bk", [DH]), ("Wv", [C, DH]), ("bv", [DH]),
        ("Wmo", [C, C]), ("bmo", [C]), ("Wf1", [C, 4 * C]), ("bf1", [4 * C]),
        ("Wf2", [4 * C, C]), ("bf2", [C]),
    ]:
        wd[name] = nc.dram_tensor(name, shape, F32, kind="ExternalInput").ap()
    out_d = nc.dram_tensor("out", [NP, C], F32, kind="ExternalOutput").ap()
    dbg = {}
    if debug:
        for name, shape in [
            ("dbg_xattn", [N, C]), ("dbg_nm", [NE]), ("dbg_rank", [NE]),
            ("dbg_nodeidx", [NE]), ("dbg_mlpin", [N, C]), ("dbg_mlpout", [N, C]),
            ("dbg_pooled", [NP, C]),
        ]:
            dbg[name] = nc.dram_tensor(name, shape, F32, kind="ExternalOutput").ap()
    with tile.TileContext(nc) as tc:
        _build_tile(nc, tc, x_d, wd, out_d, dbg)
    nc.compile()
    return nc


def _build_tile(nc, tc, x_d, wd, out_d, dbg):
    # DRAM spill buffers
    qkTd = nc.dram_tensor("qkTd", [2 * C, N], F32).ap()
    aoTd = nc.dram_tensor("aoTd", [C, N], F32).ap()
    h2d = nc.dram_tensor("h2d", [C, N], F32R).ap()
    x1d = nc.dram_tensor("x1d", [N, C], F32).ap()
    dstnd = nc.dram_tensor("dstnd", [NE, C], F32).ap()
    mod = nc.dram_tensor("mod", [N, C], F32R).ap()
    mqaTd = nc.dram_tensor("mqaTd", [C, NP], F32R).ap()

    pc = tc.alloc_tile_pool(name="const", bufs=1)
    psm = tc.alloc_tile_pool(name="small", bufs=1)
    pw = tc.alloc_tile_pool(name="wstream", bufs=2)
    pt = tc.alloc_tile_pool(name="tmp", bufs=2)
    pp = tc.alloc_tile_pool(name="psum", bufs=1, space="PSUM")

    _ct = {}

    def utile(pool, shape, dtype, tag, bufs=None):
        _ct[tag] = _ct.get(tag, 0) + 1
        kw = {"bufs": bufs} if bufs is not None else {}
        return pool.tile(shape, dtype, tag=tag, name=f"{tag}_{_ct[tag]}", **kw)

    def ps_bigA():
        return utile(pp, [128, 1024], F32, "BIGA")

    def ps_bigB():
        return utile(pp, [128, 1024], F32, "BIGB")

    def ps_mm():
        return utile(pp, [128, 512], F32, "MM", bufs=2)

    def ps_av():
        return utile(pp, [128, 4, 128], F32, "AV", bufs=2)

    # ---------- constants ----------
    ident = pc.tile([128, 128], F32)
    make_identity(nc, ident[:])
    ones_col = pc.tile([1, 128], F32)
    nc.gpsimd.memset(ones_col[:], 1.0)
    piota = pc.tile([128, 1], F32)
    nc.gpsimd.iota(piota[:], [[0, 1]], channel_multiplier=1,
                   allow_small_or_imprecise_dtypes=True)
    iota512_row = pc.tile([1, 512], F32)
    nc.gpsimd.iota(iota512_row[:], [[1, 512]], channel_multiplier=0,
                   allow_small_or_imprecise_dtypes=True)
    iota504_row = pc.tile([1, 504], F32)
    nc.gpsimd.iota(iota504_row[:], [[1, 504]], channel_multiplier=0,
                   allow_small_or_imprecise_dtypes=True)

    def bcast_row(row_ap, n, tag, pool, scale=1.0):
        t = utile(pool, [128, n], F32, tag)
        for c0 in range(0, n, 512):
            cw = min(512, n - c0)
            p = ps_mm()
            nc.tensor.matmul(p[:, :cw], ones_col[:], row_ap[:, c0:c0 + cw],
                             start=True, stop=True)
            if scale == 1.0:
                nc.vector.tensor_copy(t[:, c0:c0 + cw], p[:, :cw])
            else:
                nc.vector.tensor_scalar_mul(t[:, c0:c0 + cw], p[:, :cw], scale)
        return t

    def load_row(dram_ap, n, tag, pool):
        t = utile(pw, [1, n], F32, "rowstg", bufs=2)
        nc.sync.dma_start(t[:], dram_ap[None, :])
        return t

    def brow(name, pool, scale=1.0):
        n = wd[name].shape[0]
        return bcast_row(load_row(wd[name], n, name + "_r", pool), n,
                         name + "_b", pool, scale)

    def bcol(name, pool, scale=1.0):
        n = wd[name].shape[0]
        t = utile(pool, [128, n // 128], F32, name + "_c")
        nc.sync.dma_start(t[:], wd[name].rearrange("(t p) -> p t", p=128))
        if scale != 1.0:
            nc.vector.tensor_scalar_mul(t[:], t[:], scale)
        return t

    IOTA512B = bcast_row(iota512_row[:], 512, "iota512b", pc)
    IOTA504B = bcast_row(iota504_row[:], 504, "iota504b", pc)

    def transpose_blocks(src_tiles, dst, n_rows, n_cols):
        """dst[c, r] = src[r, c]; dst is tile-list or sink(bj, bi, pf, cw, rw)."""
        for bi in range((n_rows + 127) // 128):
            rw = min(128, n_rows - bi * 128)
            for bj in range((n_cols + 127) // 128):
                cw = min(128, n_cols - bj * 128)
                p = ps_av()
                pf = p.rearrange("p a b -> p (a b)")
                nc.tensor.transpose(pf[:cw, :rw],
                                    src_tiles[bi][:rw, bj * 128:bj * 128 + cw],
                                    ident[:rw, :rw])
                if callable(dst):
                    dst(bj, bi, pf, cw, rw)
                else:
                    nc.vector.tensor_copy(dst[bj][:cw, bi * 128:bi * 128 + rw],
                                          pf[:cw, :rw])

    def refined_rsqrt_recip(vv, tag):
        """returns 1/sqrt(vv) with one Newton step on sqrt (ACT sqrt is loose)."""
        s0 = utile(psm, [128, 1], F32, tag + "_s0")
        nc.scalar.sqrt(s0[:], vv[:])
        r0 = utile(psm, [128, 1], F32, tag + "_r0")
        nc.vector.reciprocal(r0[:], s0[:])
        t = utile(psm, [128, 1], F32, tag + "_t")
        nc.vector.tensor_tensor(t[:], vv[:], r0[:], OP.mult)
        nc.vector.tensor_tensor(t[:], t[:], s0[:], OP.add)
        nc.vector.tensor_scalar_mul(t[:], t[:], 0.5)
        rr = utile(psm, [128, 1], F32, tag + "_rr")
        nc.vector.reciprocal(rr[:], t[:])
        return rr

    def layer_norm(src, dst, gb, bb):
        m = utile(psm, [128, 1], F32, "ln_m")
        nc.vector.reduce_sum(m[:], src[:, :C], axis=mybir.AxisListType.X)
        nc.vector.tensor_scalar_mul(m[:], m[:], 1.0 / C)
        xc = utile(pt, [128, C], F32, "ln_xc")
        nc.vector.tensor_scalar(xc[:], src[:, :C], m[:], None, OP.subtract)
        ss = utile(psm, [128, 1], F32, "ln_ss")
        nc.scalar.activation(dst[:, :C], xc[:], AF.Square, accum_out=ss[:])
        v = utile(psm, [128, 1], F32, "ln_v")
        nc.vector.tensor_scalar(v[:], ss[:], 1.0 / C, 1e-5, OP.mult, OP.add)
        rstd = refined_rsqrt_recip(v, "ln")
        nc.vector.tensor_scalar(dst[:, :C], xc[:], rstd[:], None, OP.mult)
        nc.vector.tensor_tensor(dst[:, :C], dst[:, :C], gb[:], OP.mult)
        nc.vector.tensor_tensor(dst[:, :C], dst[:, :C], bb[:], OP.add)

    # ================= Stage A: LN1 -> hT =================
    pbA = tc.alloc_tile_pool(name="biasA", bufs=1)
    pHT = tc.alloc_tile_pool(name="pHT", bufs=1)
    pVP = tc.alloc_tile_pool(name="pVP", bufs=1)
    pAttn = tc.alloc_tile_pool(name="pAttn", bufs=1)

    g1b = brow("g1", pbA)
    be1b = brow("be1", pbA)
    hT = [utile(pHT, [128, N], F32, f"hT{k}") for k in range(8)]
    ht = []
    for i in range(8):
        xt = utile(pt, [128, C], F32, "xin")
        nc.sync.dma_start(xt[:], x_d[ts(i, 128), :])
        h = utile(pt, [128, C], F32, "ht", bufs=4)
        layer_norm(xt, h, g1b, be1b)
        ht.append(h)
    transpose_blocks(ht, hT, N, C)

    # ===== qk^T -> qkTd (DRAM) ; v_pad (SBUF) =====
    bqkT = bcol("bqkv", pbA)
    for mp in range(8):
        accq = ps_bigA()
        acck = ps_bigB()
        for k in range(KT):
            wq = utile(pw, [128, 128], F32, "wqkb", bufs=4)
            nc.sync.dma_start(wq[:], wd["Wqkv"][ts(k, 128), ts(mp, 128)])
            wk = utile(pw, [128, 128], F32, "wqkb", bufs=4)
            nc.sync.dma_start(wk[:],
                              wd["Wqkv"][ts(k, 128), C + mp * 128:C + (mp + 1) * 128])
            for n2 in range(2):
                nc.tensor.matmul(accq[:, ts(n2, 512)], wq[:], hT[k][:, ts(n2, 512)],
                                 start=(k == 0), stop=(k == KT - 1))
                nc.tensor.matmul(acck[:, ts(n2, 512)], wk[:], hT[k][:, ts(n2, 512)],
                                 start=(k == 0), stop=(k == KT - 1))
        stgq = utile(pAttn, [128, N], F32, "qkstg", bufs=2)
        nc.scalar.activation(stgq[:], accq[:], AF.Identity, bias=bqkT[:, mp:mp + 1])
        nc.sync.dma_start(qkTd[ts(mp, 128), :], stgq[:])
        stgk = utile(pAttn, [128, N], F32, "qkstg", bufs=2)
        nc.scalar.activation(stgk[:], acck[:], AF.Identity,
                             bias=bqkT[:, 8 + mp:9 + mp])
        nc.sync.dma_start(qkTd[C + mp * 128:C + (mp + 1) * 128, :], stgk[:])

    bvqkvb = bcast_row(load_row(wd["bqkv"][2 * C:], C, "bvq_r", pbA), C,
                       "bvq_b", pbA)
    v_pad = [utile(pVP, [128, H, DH + 1], F32, f"vp{j}") for j in range(8)]
    for j in range(8):
        nc.vector.memset(v_pad[j][:, :, DH:DH + 1], 1.0)
        acc = ps_bigA()
        for k in range(KT):
            wv = utile(pw, [128, C], F32, "ws4k")
            nc.sync.dma_start(wv[:], wd["Wqkv"][ts(k, 128), 2 * C:])
            for n2 in range(2):
                nc.tensor.matmul(acc[:, ts(n2, 512)], hT[k][:, ts(j, 128)],
                                 wv[:, ts(n2, 512)],
                                 start=(k == 0), stop=(k == KT - 1))
        for h in range(H):
            nc.vector.tensor_tensor(v_pad[j][:, h, :DH], acc[:, ts(h, DH)],
                                    bvqkvb[:, ts(h, DH)], OP.add)

    # ===== attention: stream kT/qT per head; out -> aoTd (already c-major) ==
    # out[dh|sum, i] = v_pad[j].T @ expT[j, i], accumulated over j-tiles.
    for h in range(H):
        kth = utile(pAttn, [64, N], F32, "kth", bufs=2)
        nc.sync.dma_start(kth[:], qkTd[C + h * 64:C + h * 64 + 64, :])
        qth = utile(pAttn, [64, N], F32, "qth", bufs=2)
        nc.sync.dma_start(qth[:], qkTd[h * 64:h * 64 + 64, :])
        av = [ps_av().rearrange("p a b -> p (a b)") for _ in range(2)]
        for j in range(8):
            for n2 in range(2):
                sp = ps_mm()
                nc.tensor.matmul(sp[:], kth[:, ts(j, 128)], qth[:, ts(n2, 512)],
                                 start=True, stop=True)
                et = utile(pAttn, [128, 512], F32, "exp", bufs=3)
                nc.scalar.activation(et[:], sp[:], AF.Exp, scale=float(DH ** -0.5))
                nc.tensor.matmul(av[n2][:DH + 1, :512], v_pad[j][:, h, :], et[:],
                                 start=(j == 0), stop=(j == 7))
        for n2 in range(2):
            rrow = utile(pAttn, [1, 512], F32, "rrow", bufs=2)
            nc.vector.reciprocal(rrow[:], av[n2][DH:DH + 1, :512])
            rb = ps_mm()
            nc.tensor.matmul(rb[:DH, :512], ones_col[:, :DH], rrow[:],
                             start=True, stop=True)
            rbs = utile(pAttn, [64, 512], F32, "rbs", bufs=2)
            nc.vector.tensor_copy(rbs[:], rb[:DH, :512])
            stg = utile(pAttn, [64, 512], F32, "aot_stg", bufs=2)
            nc.vector.tensor_tensor(stg[:], av[n2][:DH, :512], rbs[:],
                                    OP.mult)
            nc.sync.dma_start(aoTd[h * 64:h * 64 + 64, ts(n2, 512)], stg[:])
    pAttn.release()
    pVP.release()
    pHT.release()
    pbA.release()

    # ================= Wo -> x_attn, x1 (-> DRAM), metric =================
    pB2 = tc.alloc_tile_pool(name="pB2", bufs=1)
    bob = brow("bo", pB2)
    xa = [utile(pB2, [128, C], F32, f"xa{m}") for m in range(8)]
    rn = psm.tile([128, 8], F32)
    for m in range(8):
        acc = ps_bigA()
        for k in range(KT):
            ao = utile(pw, [128, 128], F32, "wqkb", bufs=4)
            nc.sync.dma_start(ao[:], aoTd[ts(k, 128), ts(m, 128)])
            wo = utile(pw, [128, C], F32, "ws4k")
            nc.sync.dma_start(wo[:], wd["Wo"][ts(k, 128), :])
            for n2 in range(2):
                nc.tensor.matmul(acc[:, ts(n2, 512)], ao[:], wo[:, ts(n2, 512)],
                                 start=(k == 0), stop=(k == KT - 1))
        nc.vector.tensor_tensor(xa[m][:], acc[:], bob[:], OP.add)
        xt = utile(pt, [128, C], F32, "xin")
        nc.sync.dma_start(xt[:], x_d[ts(m, 128), :])
        x1stg = utile(pw, [128, C], F32, "x1stg", bufs=2)
        nc.vector.tensor_tensor(x1stg[:], xa[m][:], xt[:], OP.add)
        nc.sync.dma_start(x1d[ts(m, 128), :], x1stg[:])
        ss = utile(psm, [128, 1], F32, "nrm_ss")
        sq = utile(pt, [128, C], F32, "ln_xc")
        nc.scalar.activation(sq[:], xa[m][:], AF.Square, accum_out=ss[:])
        rr = refined_rsqrt_recip(ss, "nrm")
        nc.vector.tensor_copy(rn[:, m:m + 1], rr[:])
        nc.vector.tensor_scalar(xa[m][:], xa[m][:], rn[:, m:m + 1], None, OP.mult)
        if dbg:
            nc.sync.dma_start(dbg["dbg_xattn"][ts(m, 128), :], xa[m][:])

    # ===== de-interleave metric -> maT/mbT; scores; node stats; ranks =====
    pB3 = tc.alloc_tile_pool(name="pB3", bufs=1)
    xae = [utile(pB3, [128, C], F32, f"xae{m}") for m in range(4)]
    xao = [utile(pB3, [128, C], F32, f"xao{m}") for m in range(4)]
    for m in range(4):
        nc.sync.dma_start(xae[m][:64, :], xa[2 * m][0:128:2, :])
        nc.sync.dma_start(xae[m][64:, :], xa[2 * m + 1][0:128:2, :])
        nc.sync.dma_start(xao[m][:64, :], xa[2 * m][1:128:2, :])
        nc.sync.dma_start(xao[m][64:, :], xa[2 * m + 1][1:128:2, :])
    pB4 = tc.alloc_tile_pool(name="pB4", bufs=1)
    maT = [utile(pB4, [128, NE], F32, f"maT{k}") for k in range(8)]
    mbT = [utile(pB4, [128, NE], F32, f"mbT{k}") for k in range(8)]
    transpose_blocks(xae, maT, NE, C)
    transpose_blocks(xao, mbT, NE, C)

    nm_t = psm.tile([128, 4], F32)
    ni_t = psm.tile([128, 4], F32)
    for m in range(4):
        acc = ps_bigA()
        for k in range(KT):
            nc.tensor.matmul(acc[:, :512], maT[k][:, ts(m, 128)], mbT[k][:],
                             start=(k == 0), stop=(k == KT - 1))
        mx8 = utile(psm, [128, 8], F32, "mx8")
        ix8 = utile(psm, [128, 8], U32, "ix8")
        nc.vector.max_with_indices(mx8[:], ix8[:], acc[:, :512])
        nc.vector.tensor_copy(nm_t[:, m:m + 1], mx8[:, 0:1])
        nc.vector.tensor_copy(ni_t[:, m:m + 1], ix8[:, 0:1])

    nm_row = utile(pB4, [1, 512], F32, "nm_row")
    for m in range(4):
        p = ps_av()
        pf = p.rearrange("p a b -> p (a b)")
        nc.tensor.transpose(pf[:1, :128], nm_t[:, m:m + 1], ident[:])
        nc.vector.tensor_copy(nm_row[:, ts(m, 128)], pf[:1, :128])
    NMB = bcast_row(nm_row[:], 512, "nmb", pB4)

    rank_t = psm.tile([128, 4], F32)
    for m in range(4):
        gt = utile(pB4, [128, 512], F32, "rk_gt")
        nc.vector.tensor_scalar(gt[:], NMB[:], nm_t[:, m:m + 1], None, OP.is_gt)
        eq = utile(pB4, [128, 512], F32, "rk_eq")
        nc.vector.tensor_scalar(eq[:], NMB[:], nm_t[:, m:m + 1], None, OP.is_equal)
        flt = utile(pB4, [128, 512], F32, "rk_flt")
        pio = utile(psm, [128, 1], F32, "rk_pio")
        nc.vector.tensor_scalar_add(pio[:], piota[:], float(128 * m))
        nc.vector.tensor_scalar(flt[:], IOTA512B[:], pio[:], None, OP.is_lt)
        nc.vector.tensor_tensor(eq[:], eq[:], flt[:], OP.mult)
        nc.vector.tensor_tensor(gt[:], gt[:], eq[:], OP.add)
        nc.vector.reduce_sum(rank_t[:, m:m + 1], gt[:], axis=mybir.AxisListType.X)
    if dbg:
        for (tt, nme) in [(nm_t, "dbg_nm"), (rank_t, "dbg_rank"),
                          (ni_t, "dbg_nodeidx")]:
            nc.sync.dma_start(dbg[nme].rearrange("(m p) -> p m", p=128), tt[:])
    pB4.release()
    pB3.release()
    pB2.release()

    # ================= dst merge (x1 from DRAM; dstn -> DRAM) =============
    pM = tc.alloc_tile_pool(name="pM", bufs=1)
    x1e = [utile(pM, [128, C + 8], F32, f"x1e{m}") for m in range(4)]
    x1o = [utile(pM, [128, C], F32, f"x1o{m}") for m in range(4)]
    for m in range(4):
        nc.vector.memset(x1e[m][:, C:C + 1], 1.0)
        nc.sync.dma_start(x1e[m][:, :C], x1d[256 * m:256 * m + 256:2, :])
        nc.sync.dma_start(x1o[m][:], x1d[256 * m + 1:256 * m + 256:2, :])
    st = [utile(pM, [128, 512], F32, f"st{m}") for m in range(4)]
    for m in range(4):
        msk = utile(psm, [128, 1], F32, "st_m")
        nc.vector.tensor_scalar(msk[:], rank_t[:, m:m + 1], float(R) - 0.5, None,
                                OP.is_lt)
        nc.vector.tensor_scalar(st[m][:], IOTA512B[:], ni_t[:, m:m + 1], None,
                                OP.is_equal)
        nc.vector.tensor_scalar(st[m][:], st[m][:], msk[:], None, OP.mult)
    for m in range(4):
        acc = ps_bigA()
        cacc = ps_av()
        for k in range(4):
            for n2 in range(2):
                nc.tensor.matmul(acc[:, ts(n2, 512)], st[k][:, ts(m, 128)],
                                 x1e[k][:, n2 * 512:n2 * 512 + 512],
                                 start=(k == 0), stop=(k == 3))
            nc.tensor.matmul(cacc[:, 0, :1], st[k][:, ts(m, 128)],
                             x1e[k][:, C:C + 1], start=(k == 0), stop=(k == 3))
        cnt = utile(psm, [128, 1], F32, "cnt")
        nc.vector.tensor_scalar_add(cnt[:], cacc[:, 0, 0:1], 1.0)
        rec = utile(psm, [128, 1], F32, "cntr")
        nc.vector.reciprocal(rec[:], cnt[:])
        dst_stg = utile(pM, [128, C], F32, "dst_stg", bufs=2)
        nc.vector.tensor_tensor(dst_stg[:], acc[:], x1o[m][:], OP.add)
        nc.vector.tensor_scalar(dst_stg[:], dst_stg[:], rec[:], None, OP.mult)
        nc.sync.dma_start(dstnd[ts(m, 128), :], dst_stg[:])

    # ========== MLP (f32r): W1/W2 streamed once; SBUF out accumulation ======
    def row_src_ap(i):
        if i < 4:
            return x1d[256 * i:256 * i + 256:2, :]
        return dstnd[ts(i - 4, 128), :]

    pM.release()
    pC4 = tc.alloc_tile_pool(name="pC4", bufs=1)
    g2b = brow("g2", pC4)
    be2b = brow("be2", pC4)
    h2 = []
    for i in range(8):
        rsrc = utile(pt, [128, C], F32, "xin")
        nc.sync.dma_start(rsrc[:], row_src_ap(i))
        h = utile(pt, [128, C], F32, "ht", bufs=4)
        layer_norm(rsrc, h, g2b, be2b)
        h2.append(h)
        if dbg:
            nc.sync.dma_start(dbg["dbg_mlpin"][ts(i, 128), :], rsrc[:])
    h2T = [utile(pC4, [128, N], F32R, f"h2T{k}") for k in range(8)]
    transpose_blocks(h2, h2T, N, C)

    bm1T = bcol("bm1", pC4)
    bm2b = brow("bm2", pC4)
    for q in range(4):
        oacc = [ps_bigA(), ps_bigB()]     # out token tiles 2q, 2q+1
        for mtg in range(8):
            w1cs = []
            for k in range(KT):
                w1c = utile(pC4, [128, 512], F32, "w1c", bufs=3)
                nc.sync.dma_start(w1c[:], wd["W1"][ts(k, 128), ts(mtg, 512)])
                w1cr = utile(pC4, [128, 512], F32R, f"w1cr{k}", bufs=2)
                nc.vector.tensor_copy(w1cr[:], w1c[:])
                w1cs.append(w1cr)
            for mi in range(4):
                mt = mtg * 4 + mi
                yp = ps_mm() if mi % 2 == 0 else                     ps_av().rearrange("p a b -> p (a b)")
                for k in range(KT):
                    nc.tensor.matmul(yp[:, :256], w1cs[k][:, ts(mi, 128)],
                                     h2T[k][:, q * 256:q * 256 + 256],
                                     start=(k == 0), stop=(k == KT - 1))
                g1t = utile(pC4, [128, 256], F32R, "g1t", bufs=3)
                nc.scalar.activation(g1t[:], yp[:, :256], AF.Gelu_apprx_tanh,
                                     bias=bm1T[:, mt:mt + 1])
                w2t = utile(pC4, [128, C], F32, "w2s", bufs=3)
                nc.sync.dma_start(w2t[:], wd["W2"][ts(mt, 128), :])
                w2r = utile(pC4, [128, C], F32R, "w2r", bufs=3)
                nc.vector.tensor_copy(w2r[:], w2t[:])
                for tl in range(2):
                    for n2 in range(2):
                        nc.tensor.matmul(oacc[tl][:, ts(n2, 512)],
                                         g1t[:, tl * 128:tl * 128 + 128],
                                         w2r[:, ts(n2, 512)],
                                         start=(mt == 0), stop=(mt == 31))
        for tl in range(2):
            row = 2 * q + tl
            res = utile(pt, [128, C], F32, "xin")
            nc.sync.dma_start(res[:], row_src_ap(row))
            mf = utile(pC4, [128, C], F32, "mof", bufs=2)
            nc.vector.tensor_tensor(mf[:], oacc[tl][:], bm2b[:], OP.add)
            nc.vector.tensor_tensor(mf[:], mf[:], res[:], OP.add)
            mr = utile(pC4, [128, C], F32R, "mor", bufs=2)
            nc.vector.tensor_copy(mr[:], mf[:])
            nc.sync.dma_start(mod[ts(row, 128), :], mr[:])
            if dbg:
                nc.sync.dma_start(dbg["dbg_mlpout"][ts(row, 128), :], mf[:])
    pC4.release()

    # ================= Stage D: pooling + Wp -> combined^T =================
    pD = tc.alloc_tile_pool(name="pD", bufs=1)
    # ApT[p, f] = 0.5 iff source row p pools into output f:
    #   even block: base = rank[p]-16, match iff (2f - base) in {-1, 0}
    #   dst  block: base = d,          match iff (2(f-248) - base) in {-1, 0}
    iota2e = utile(pD, [128, 504], F32, "iota2e")
    nc.vector.tensor_scalar_mul(iota2e[:], IOTA504B[:], 2.0)
    apT = [utile(pD, [128, 504], F32R, f"apT{m}") for m in range(8)]
    for m in range(8):
        base = utile(psm, [128, 1], F32, "ap_r")
        if m < 4:
            nc.vector.tensor_scalar_add(base[:], rank_t[:, m:m + 1], -float(R))
        else:
            nc.vector.tensor_scalar_add(base[:], piota[:],
                                        float(128 * (m - 4) + NE - R))
        d1 = utile(pD, [128, 504], F32, "ap_d1")
        nc.vector.tensor_scalar(d1[:], iota2e[:], base[:], None, OP.subtract)
        a1 = utile(pD, [128, 504], F32, "ap_a1")
        nc.vector.tensor_scalar(a1[:], d1[:], -1.5, None, OP.is_ge)
        b1 = utile(pD, [128, 504], F32, "ap_b1")
        nc.vector.tensor_scalar(b1[:], d1[:], 0.5, None, OP.is_le)
        nc.vector.scalar_tensor_tensor(apT[m][:], a1[:], 0.5, b1[:],
                                       OP.mult, OP.mult)
    pooledT = [utile(pD, [128, NP], F32R, f"pooledT{k}") for k in range(8)]
    for m in range(4):
        acc = ps_bigA()
        for k in range(8):
            mob = utile(pD, [128, C], F32R, "mob", bufs=3)
            nc.sync.dma_start(mob[:], mod[ts(k, 128), :])
            for n2 in range(2):
                nc.tensor.matmul(acc[:PP, ts(n2, 512)],
                                 apT[k][:, m * PP:(m + 1) * PP],
                                 mob[:, ts(n2, 512)], start=(k == 0), stop=(k == 7))
        pst = utile(pD, [128, C], F32, "pstg", bufs=2)
        nc.vector.tensor_copy(pst[:PP, :], acc[:PP, :])
        if dbg:
            nc.sync.dma_start(dbg["dbg_pooled"][ts(m, PP), :], pst[:PP, :])
        for bj in range(8):
            p = ps_av()
            pf = p.rearrange("p a b -> p (a b)")
            nc.tensor.transpose(pf[:128, :PP], pst[:PP, ts(bj, 128)],
                                ident[:PP, :PP])
            nc.vector.tensor_copy(pooledT[bj][:, m * PP:(m + 1) * PP],
                                  pf[:128, :PP])

    pE = tc.alloc_tile_pool(name="pE", bufs=1)
    bp3T = bcol("bp", pD, scale=3.0)
    cmbTr = [utile(pD, [128, NP], F32R, f"cmbTr{m}") for m in range(8)]
    for mg in range(2):
        wcs = []
        for k in range(KT):
            wc = utile(pw, [128, 512], F32, "ws4k")
            nc.sync.dma_start(wc[:], wd["Wp"][ts(k, 128), ts(mg, 512)])
            wcr = utile(pD, [128, 512], F32R, f"wpc{k}", bufs=1)
            nc.vector.tensor_copy(wcr[:], wc[:])
            wcs.append(wcr)
        for mi in range(4):
            m = mg * 4 + mi
            acc = ps_mm()
            for k in range(KT):
                nc.tensor.matmul(acc[:, :NP], wcs[k][:, ts(mi, 128)],
                                 pooledT[k][:], start=(k == 0), stop=(k == KT - 1))
            nc.scalar.activation(cmbTr[m][:], acc[:, :NP], AF.Identity,
                                 bias=bp3T[:, m:m + 1], scale=3.0)

    # ================= Stage E: MQA =================
    bqT = bcol("bq", pE)

    def make_mqT(m):
        acc = ps_mm()
        for k in range(KT):
            wb = utile(pw, [128, 128], F32, "w1b", bufs=4)
            nc.sync.dma_start(wb[:], wd["Wq"][ts(k, 128), ts(m, 128)])
            wr = utile(pw, [128, 128], F32R, "w1r", bufs=4)
            nc.vector.tensor_copy(wr[:], wb[:])
            nc.tensor.matmul(acc[:, :NP], wr[:], cmbTr[k][:],
                             start=(k == 0), stop=(k == KT - 1))
        t = utile(pE, [128, NP], F32R, "mqT", bufs=2)
        nc.scalar.activation(t[:], acc[:, :NP], AF.Identity, bias=bqT[:, m:m + 1])
        return t

    wkv = utile(pE, [128, KT, 2 * DH], F32, "wkv")
    wkvr = utile(pE, [128, KT, 2 * DH], F32R, "wkvr")
    for k in range(KT):
        nc.sync.dma_start(wkv[:, k, :DH], wd["Wk"][ts(k, 128), :])
        nc.sync.dma_start(wkv[:, k, DH:], wd["Wv"][ts(k, 128), :])
    nc.vector.tensor_copy(wkvr[:], wkv[:])
    bkT = utile(pE, [64, 1], F32, "bkT")
    nc.sync.dma_start(bkT[:], wd["bk"][:, None])
    mkT = utile(pE, [128, NP], F32R, "mkT")
    macc = ps_mm()
    for k in range(KT):
        nc.tensor.matmul(macc[:64, :NP], wkvr[:, k, :DH], cmbTr[k][:],
                         start=(k == 0), stop=(k == KT - 1))
    mkf = utile(pE, [64, NP], F32, "mkf")
    nc.scalar.activation(mkf[:], macc[:64, :NP], AF.Identity, bias=bkT[:])
    nc.vector.tensor_copy(mkT[:64, :], mkf[:])
    nc.sync.dma_start(mkT[64:, :], mkT[:64, :])

    bvb = bcast_row(load_row(wd["bv"], DH, "bv_r", pE), DH, "bv_b", pE)
    mv_pad = [utile(pE, [128, DH + 1], F32, f"mvp{m}") for m in range(4)]
    for m in range(4):
        acc = ps_av()
        for k in range(KT):
            nc.tensor.matmul(acc[:PP, 0, :DH], cmbTr[k][:, m * PP:(m + 1) * PP],
                             wkvr[:, k, DH:], start=(k == 0), stop=(k == KT - 1))
        nc.vector.memset(mv_pad[m][:], 0.0)
        nc.vector.memset(mv_pad[m][:, DH:], 1.0)
        nc.vector.tensor_tensor(mv_pad[m][:PP, :DH], acc[:PP, 0, :DH], bvb[:PP, :],
                                OP.add)

    mqT_cur = None
    for h in range(H):
        po = (h % 2) * 64
        if h % 2 == 0:
            mqT_cur = make_mqT(h // 2)
        mqT_h = mqT_cur[po:po + 64, :]
        ep = []
        for mm in range(4):
            sp = ps_mm()
            nc.tensor.matmul(sp[:PP, :NP], mkT[po:po + 64, mm * PP:(mm + 1) * PP],
                             mqT_h[:], start=True, stop=True)
            et = utile(pE, [128, NP], F32, "e2", bufs=4)
            nc.scalar.activation(et[:PP, :], sp[:PP, :NP], AF.Exp,
                                 scale=float(DH ** -0.5))
            ep.append(et)
        av2 = ps_av().rearrange("p a b -> p (a b)")
        for mm in range(4):
            nc.tensor.matmul(av2[:DH + 1, :NP], mv_pad[mm][:PP, :],
                             ep[mm][:PP, :], start=(mm == 0), stop=(mm == 3))
        rrow = utile(pE, [1, NP], F32, "rrow2", bufs=2)
        nc.vector.reciprocal(rrow[:], av2[DH:DH + 1, :NP])
        rb = ps_mm()
        nc.tensor.matmul(rb[:DH, :NP], ones_col[:, :DH], rrow[:],
                         start=True, stop=True)
        rbs = utile(pE, [64, NP], F32, "rbs2", bufs=2)
        nc.vector.tensor_copy(rbs[:], rb[:DH, :NP])
        stg = utile(pE, [64, NP], F32R, "mqstg", bufs=2)
        nc.vector.tensor_tensor(stg[:], av2[:DH, :NP], rbs[:], OP.mult)
        nc.sync.dma_start(mqaTd[h * 64:h * 64 + 64, :], stg[:])
    pE.release()
    pD.release()

    # ================= Stage F: Wmo + FFN =================
    pF = tc.alloc_tile_pool(name="pF", bufs=1)
    mqaT = [utile(pF, [128, NP], F32R, f"mqaT{k}") for k in range(8)]
    for k in range(8):
        nc.sync.dma_start(mqaT[k][:, :NP], mqaTd[ts(k, 128), :])
    bmoT = bcol("bmo", pF)
    omoT = [utile(pF, [128, NP], F32R, f"omoT{m}") for m in range(8)]
    for mg in range(2):
        wcs = []
        for k in range(KT):
            wc = utile(pw, [128, 512], F32, "ws4k")
            nc.sync.dma_start(wc[:], wd["Wmo"][ts(k, 128), ts(mg, 512)])
            wcr = utile(pF, [128, 512], F32R, f"wmc{k}", bufs=1)
            nc.vector.tensor_copy(wcr[:], wc[:])
            wcs.append(wcr)
        for mi in range(4):
            m = mg * 4 + mi
            acc = ps_mm()
            for k in range(KT):
                nc.tensor.matmul(acc[:, :NP], wcs[k][:, ts(mi, 128)],
                                 mqaT[k][:], start=(k == 0), stop=(k == KT - 1))
            nc.scalar.activation(omoT[m][:], acc[:, :NP], AF.Identity,
                                 bias=bmoT[:, m:m + 1])

    bf1T = bcol("bf1", pF)
    bf2b = brow("bf2", pF)
    for half in range(2):
        t0 = half * 252
        oacc = [ps_bigA(), ps_bigB()]
        for kkg in range(8):
            wf1cs = []
            for k in range(KT):
                wf1c = utile(pF, [128, 512], F32, "w1c", bufs=3)
                nc.sync.dma_start(wf1c[:], wd["Wf1"][ts(k, 128), ts(kkg, 512)])
                wf1cr = utile(pF, [128, 512], F32R, f"w1cr{k}", bufs=1)
                nc.vector.tensor_copy(wf1cr[:], wf1c[:])
                wf1cs.append(wf1cr)
            for ki in range(4):
                kk = kkg * 4 + ki
                yp = ps_mm() if ki % 2 == 0 else                     ps_av().rearrange("p a b -> p (a b)")
                for k in range(KT):
                    nc.tensor.matmul(yp[:, :252], wf1cs[k][:, ts(ki, 128)],
                                     omoT[k][:, t0:t0 + 252],
                                     start=(k == 0), stop=(k == KT - 1))
                g2t = utile(pF, [128, 252], F32R, "g2t", bufs=3)
                nc.scalar.activation(g2t[:], yp[:, :252], AF.Silu,
                                     bias=bf1T[:, kk:kk + 1])
                wf2t = utile(pF, [128, C], F32, "w2s", bufs=3)
                nc.sync.dma_start(wf2t[:], wd["Wf2"][ts(kk, 128), :])
                wf2r = utile(pF, [128, C], F32R, "wf2r", bufs=3)
                nc.vector.tensor_copy(wf2r[:], wf2t[:])
                for tl in range(2):
                    for n2 in range(2):
                        nc.tensor.matmul(oacc[tl][:PP, ts(n2, 512)],
                                         g2t[:, tl * 126:tl * 126 + 126],
                                         wf2r[:, ts(n2, 512)],
                                         start=(kk == 0), stop=(kk == 31))
        for tl in range(2):
            row0 = (2 * half + tl) * PP
            of = utile(pF, [128, C], F32, "of", bufs=2)
            nc.vector.tensor_tensor(of[:PP, :], oacc[tl][:PP, :], bf2b[:PP, :],
                                    OP.add)
            nc.sync.dma_start(out_d[row0:row0 + PP, :], of[:PP, :])
    pF.release()
    for pool in (pt, pw, psm, pc, pp):
        pool.release()


_BUILT = None


def kernel(**inputs):
    global _BUILT
    if _BUILT is None:
        _BUILT = build(debug=DEBUG)
    nc = _BUILT
    x = np.ascontiguousarray(inputs["x"], dtype=np.float32)
    base = {k: np.ascontiguousarray(v, dtype=np.float32) for k, v in inputs.items()
            if k != "x"}
    in_maps = []
    for i in range(8):
        m = dict(base)
        m["x"] = x[i]
        in_maps.append(m)
    res = run_bass_kernel_spmd(nc, in_maps, core_ids=list(range(8)))
    out = np.stack([res.results[i]["out"] for i in range(8)], axis=0)
    return out.astype(np.float32)


# revision 39
# speedup vs baseline: 5966.8569x; 1.0259x over previous
"""AdaptiveTokenMerger (ToMe block + merger) TRN2 Bass kernel.

Data-parallel over batch: 8 samples -> 8 NeuronCores, one sample per core.
Per-core pipeline (sample x [1024, 1024]):
  A (f32, ranking-critical): LN1 -> qkv -> MHA (transposed-softmax with the
    denominator folded in as an appended ones-column of v) -> Wo -> x_attn
  B: metric scores -> node_max/argmax -> ranks via pairwise comparisons ->
    dst scatter-add expressed as a one-hot matmul
  C (f32r): MLP over rows [x1_even(512); dst_new(512)], fused W1/W2 per
    token-quarter, output accumulated in PSUM across all 32 W1 column tiles
  D: pooling as a rank-dependent one-hot matmul -> Wp -> combined = 3q
  E (f32r): multi-query attention  F (f32r): FFN -> out [504, 1024]

Precision: everything upstream of the rank/argmax decisions is true fp32
(4 cyc/row on PE); post-merge matmuls use float32r (TF32-ish, 1 cyc/row).

PSUM budget (8 banks): BIGA/BIGB [128,1024] (2+2), MM [128,512] x2 (2),
AV [128,4,128] x2 (2).
"""
import numpy as np

import concourse.bass as bass
import concourse.tile as tile
from concourse import bacc, mybir
from concourse.bass import ts
from concourse.bass_utils import run_bass_kernel_spmd
from concourse.masks import make_identity

F32 = mybir.dt.float32
F32R = mybir.dt.float32r
U32 = mybir.dt.uint32

N, C, H = 1024, 1024, 16
R = 16
DH = C // H          # 64
NE = N // 2          # 512
NP = (N - R) // 2    # 504
PP = 126             # pooled tokens per partition tile
KT = C // 128        # 8
AF = mybir.ActivationFunctionType
OP = mybir.AluOpType

DEBUG = False


def build(debug=False):
    nc = bacc.Bacc("TRN2", target_bir_lowering=False, debug=False, num_devices=8)
    x_d = nc.dram_tensor("x", [N, C], F32, kind="ExternalInput").ap()
    wd = {}
    for name, shape in [
        ("g1", [C]), ("be1", [C]), ("Wqkv", [C, 3 * C]), ("bqkv", [3 * C]),
        ("Wo", [C, C]), ("bo", [C]), ("g2", [C]), ("be2", [C]),
        ("W1", [C, 4 * C]), ("bm1", [4 * C]), ("W2", [4 * C, C]), ("bm2", [C]),
        ("Wp", [C, C]), ("bp", [C]), ("Wq", [C, C]), ("bq", [C]),
        ("Wk", [C, DH]), ("bk", [DH]), ("Wv", [C, DH]), ("bv", [DH]),
        ("Wmo", [C, C]), ("bmo", [C]), ("Wf1", [C, 4 * C]), ("bf1", [4 * C]),
        ("Wf2", [4 * C, C]), ("bf2", [C]),
    ]:
        wd[name] = nc.dram_tensor(name, shape, F32, kind="ExternalInput").ap()
    out_d = nc.dram_tensor("out", [NP, C], F32, kind="ExternalOutput").ap()
    dbg = {}
    if debug:
        for name, shape in [
            ("dbg_xattn", [N, C]), ("dbg_nm", [NE]), ("dbg_rank", [NE]),
            ("dbg_nodeidx", [NE]), ("dbg_mlpin", [N, C]), ("dbg_mlpout", [N, C]),
            ("dbg_pooled", [NP, C]),
        ]:
            dbg[name] = nc.dram_tensor(name, shape, F32, kind="ExternalOutput").ap()
    with tile.TileContext(nc) as tc:
        _build_tile(nc, tc, x_d, wd, out_d, dbg)
    nc.compile()
    return nc


def _build_tile(nc, tc, x_d, wd, out_d, dbg):
    # DRAM spill buffers
    qkTd = nc.dram_tensor("qkTd", [2 * C, N], F32).ap()
    aoTd = nc.dram_tensor("aoTd", [C, N], F32).ap()
    h2d = nc.dram_tensor("h2d", [C, N], F32R).ap()
    x1d = nc.dram_tensor("x1d", [N, C], F32).ap()
    dstnd = nc.dram_tensor("dstnd", [NE, C], F32).ap()
    mod = nc.dram_tensor("mod", [N, C], F32R).ap()
    mqaTd = nc.dram_tensor("mqaTd", [C, NP], F32R).ap()

    pc = tc.alloc_tile_pool(name="const", bufs=1)
    psm = tc.alloc_tile_pool(name="small", bufs=1)
    pw = tc.alloc_tile_pool(name="wstream", bufs=2)
    pt = tc.alloc_tile_pool(name="tmp", bufs=2)
    pp = tc.alloc_tile_pool(name="psum", bufs=1, space="PSUM")

    _ct = {}

    def utile(pool, shape, dtype, tag, bufs=None):
        _ct[tag] = _ct.get(tag, 0) + 1
        kw = {"bufs": bufs} if bufs is not None else {}
        return pool.tile(shape, dtype, tag=tag, name=f"{tag}_{_ct[tag]}", **kw)

    def ps_bigA():
        return utile(pp, [128, 1024], F32, "BIGA")

    def ps_bigB():
        return utile(pp, [128, 1024], F32, "BIGB")

    def ps_mm():
        return utile(pp, [128, 512], F32, "MM", bufs=2)

    def ps_av():
        return utile(pp, [128, 4, 128], F32, "AV", bufs=2)

    # ---------- constants ----------
    ident = pc.tile([128, 128], F32)
    make_identity(nc, ident[:])
    ones_col = pc.tile([1, 128], F32)
    nc.gpsimd.memset(ones_col[:], 1.0)
    piota = pc.tile([128, 1], F32)
    nc.gpsimd.iota(piota[:], [[0, 1]], channel_multiplier=1,
                   allow_small_or_imprecise_dtypes=True)
    iota512_row = pc.tile([1, 512], F32)
    nc.gpsimd.iota(iota512_row[:], [[1, 512]], channel_multiplier=0,
                   allow_small_or_imprecise_dtypes=True)
    iota504_row = pc.tile([1, 504], F32)
    nc.gpsimd.iota(iota504_row[:], [[1, 504]], channel_multiplier=0,
                   allow_small_or_imprecise_dtypes=True)

    def bcast_row(row_ap, n, tag, pool, scale=1.0):
        t = utile(pool, [128, n], F32, tag)
        for c0 in range(0, n, 512):
            cw = min(512, n - c0)
            p = ps_mm()
            nc.tensor.matmul(p[:, :cw], ones_col[:], row_ap[:, c0:c0 + cw],
                             start=True, stop=True)
            if scale == 1.0:
                nc.vector.tensor_copy(t[:, c0:c0 + cw], p[:, :cw])
            else:
                nc.vector.tensor_scalar_mul(t[:, c0:c0 + cw], p[:, :cw], scale)
        return t

    def load_row(dram_ap, n, tag, pool):
        t = utile(pw, [1, n], F32, "rowstg", bufs=2)
        nc.sync.dma_start(t[:], dram_ap[None, :])
        return t

    def brow(name, pool, scale=1.0):
        n = wd[name].shape[0]
        return bcast_row(load_row(wd[name], n, name + "_r", pool), n,
                         name + "_b", pool, scale)

    def bcol(name, pool, scale=1.0):
        n = wd[name].shape[0]
        t = utile(pool, [128, n // 128], F32, name + "_c")
        nc.sync.dma_start(t[:], wd[name].rearrange("(t p) -> p t", p=128))
        if scale != 1.0:
            nc.vector.tensor_scalar_mul(t[:], t[:], scale)
        return t

    IOTA512B = bcast_row(iota512_row[:], 512, "iota512b", pc)
    IOTA504B = bcast_row(iota504_row[:], 504, "iota504b", pc)

    def transpose_blocks(src_tiles, dst, n_rows, n_cols):
        """dst[c, r] = src[r, c]; dst is tile-list or sink(bj, bi, pf, cw, rw)."""
        for bi in range((n_rows + 127) // 128):
            rw = min(128, n_rows - bi * 128)
            for bj in range((n_cols + 127) // 128):
                cw = min(128, n_cols - bj * 128)
                p = ps_av()
                pf = p.rearrange("p a b -> p (a b)")
                nc.tensor.transpose(pf[:cw, :rw],
                                    src_tiles[bi][:rw, bj * 128:bj * 128 + cw],
                                    ident[:rw, :rw])
                if callable(dst):
                    dst(bj, bi, pf, cw, rw)
                else:
                    nc.vector.tensor_copy(dst[bj][:cw, bi * 128:bi * 128 + rw],
                                          pf[:cw, :rw])

    def refined_rsqrt_recip(vv, tag):
        """returns 1/sqrt(vv) with one Newton step on sqrt (ACT sqrt is loose)."""
        s0 = utile(psm, [128, 1], F32, tag + "_s0")
        nc.scalar.sqrt(s0[:], vv[:])
        r0 = utile(psm, [128, 1], F32, tag + "_r0")
        nc.vector.reciprocal(r0[:], s0[:])
        t = utile(psm, [128, 1], F32, tag + "_t")
        nc.vector.tensor_tensor(t[:], vv[:], r0[:], OP.mult)
        nc.vector.tensor_tensor(t[:], t[:], s0[:], OP.add)
        nc.vector.tensor_scalar_mul(t[:], t[:], 0.5)
        rr = utile(psm, [128, 1], F32, tag + "_rr")
        nc.vector.reciprocal(rr[:], t[:])
        return rr

    def layer_norm(src, dst, gb, bb):
        m = utile(psm, [128, 1], F32, "ln_m")
        nc.vector.reduce_sum(m[:], src[:, :C], axis=mybir.AxisListType.X)
        nc.vector.tensor_scalar_mul(m[:], m[:], 1.0 / C)
        xc = utile(pt, [128, C], F32, "ln_xc")
        nc.vector.tensor_scalar(xc[:], src[:, :C], m[:], None, OP.subtract)
        ss = utile(psm, [128, 1], F32, "ln_ss")
        nc.scalar.activation(dst[:, :C], xc[:], AF.Square, accum_out=ss[:])
        v = utile(psm, [128, 1], F32, "ln_v")
        nc.vector.tensor_scalar(v[:], ss[:], 1.0 / C, 1e-5, OP.mult, OP.add)
        rstd = refined_rsqrt_recip(v, "ln")
        nc.vector.tensor_scalar(dst[:, :C], xc[:], rstd[:], None, OP.mult)
        nc.vector.tensor_tensor(dst[:, :C], dst[:, :C], gb[:], OP.mult)
        nc.vector.tensor_tensor(dst[:, :C], dst[:, :C], bb[:], OP.add)

    # ================= Stage A: LN1 -> hT =================
    pbA = tc.alloc_tile_pool(name="biasA", bufs=1)
    pHT = tc.alloc_tile_pool(name="pHT", bufs=1)
    pVP = tc.alloc_tile_pool(name="pVP", bufs=1)
    pAttn = tc.alloc_tile_pool(name="pAttn", bufs=1)

    g1b = brow("g1", pbA)
    be1b = brow("be1", pbA)
    hT = [utile(pHT, [128, N], F32, f"hT{k}") for k in range(8)]
    ht = []
    for i in range(8):
        xt = utile(pt, [128, C], F32, "xin")
        nc.sync.dma_start(xt[:], x_d[ts(i, 128), :])
        h = utile(pt, [128, C], F32, "ht", bufs=4)
        layer_norm(xt, h, g1b, be1b)
        ht.append(h)
    transpose_blocks(ht, hT, N, C)

    # ===== qk^T -> qkTd (DRAM) ; v_pad (SBUF) =====
    bqkT = bcol("bqkv", pbA)
    for mp in range(8):
        accq = ps_bigA()
        acck = ps_bigB()
        for k in range(KT):
            wq = utile(pw, [128, 128], F32, "wqkb", bufs=4)
            nc.sync.dma_start(wq[:], wd["Wqkv"][ts(k, 128), ts(mp, 128)])
            wk = utile(pw, [128, 128], F32, "wqkb", bufs=4)
            nc.sync.dma_start(wk[:],
                              wd["Wqkv"][ts(k, 128), C + mp * 128:C + (mp + 1) * 128])
            for n2 in range(2):
                nc.tensor.matmul(accq[:, ts(n2, 512)], wq[:], hT[k][:, ts(n2, 512)],
                                 start=(k == 0), stop=(k == KT - 1))
                nc.tensor.matmul(acck[:, ts(n2, 512)], wk[:], hT[k][:, ts(n2, 512)],
                                 start=(k == 0), stop=(k == KT - 1))
        stgq = utile(pAttn, [128, N], F32, "qkstg", bufs=2)
        nc.scalar.activation(stgq[:], accq[:], AF.Identity, bias=bqkT[:, mp:mp + 1])
        nc.sync.dma_start(qkTd[ts(mp, 128), :], stgq[:])
        stgk = utile(pAttn, [128, N], F32, "qkstg", bufs=2)
        nc.scalar.activation(stgk[:], acck[:], AF.Identity,
                             bias=bqkT[:, 8 + mp:9 + mp])
        nc.sync.dma_start(qkTd[C + mp * 128:C + (mp + 1) * 128, :], stgk[:])

    bvqkvb = bcast_row(load_row(wd["bqkv"][2 * C:], C, "bvq_r", pbA), C,
                       "bvq_b", pbA)
    v_pad = [utile(pVP, [128, H, DH + 1], F32, f"vp{j}") for j in range(8)]
    for j in range(8):
        nc.vector.memset(v_pad[j][:, :, DH:DH + 1], 1.0)
        acc = ps_bigA()
        for k in range(KT):
            wv = utile(pVP, [128, C], F32, "wv", bufs=3)
            nc.sync.dma_start(wv[:], wd["Wqkv"][ts(k, 128), 2 * C:])
            for n2 in range(2):
                nc.tensor.matmul(acc[:, ts(n2, 512)], hT[k][:, ts(j, 128)],
                                 wv[:, ts(n2, 512)],
                                 start=(k == 0), stop=(k == KT - 1))
        for h in range(H):
            nc.vector.tensor_tensor(v_pad[j][:, h, :DH], acc[:, ts(h, DH)],
                                    bvqkvb[:, ts(h, DH)], OP.add)

    # ===== attention: stream kT/qT per head; out -> aoTd (already c-major) ==
    # out[dh|sum, i] = v_pad[j].T @ expT[j, i], accumulated over j-tiles.
    for h in range(H):
        kth = utile(pAttn, [64, N], F32, "kth", bufs=2)
        nc.sync.dma_start(kth[:], qkTd[C + h * 64:C + h * 64 + 64, :])
        qth = utile(pAttn, [64, N], F32, "qth", bufs=2)
        nc.sync.dma_start(qth[:], qkTd[h * 64:h * 64 + 64, :])
        av = [ps_av().rearrange("p a b -> p (a b)") for _ in range(2)]
        for j in range(8):
            for n2 in range(2):
                sp = ps_mm()
                nc.tensor.matmul(sp[:], kth[:, ts(j, 128)], qth[:, ts(n2, 512)],
                                 start=True, stop=True)
                et = utile(pAttn, [128, 512], F32, "exp", bufs=3)
                nc.scalar.activation(et[:], sp[:], AF.Exp, scale=float(DH ** -0.5))
                nc.tensor.matmul(av[n2][:DH + 1, :512], v_pad[j][:, h, :], et[:],
                                 start=(j == 0), stop=(j == 7))
        for n2 in range(2):
            rrow = utile(pAttn, [1, 512], F32, "rrow", bufs=2)
            nc.vector.reciprocal(rrow[:], av[n2][DH:DH + 1, :512])
            rb = ps_mm()
            nc.tensor.matmul(rb[:DH, :512], ones_col[:, :DH], rrow[:],
                             start=True, stop=True)
            rbs = utile(pAttn, [64, 512], F32, "rbs", bufs=2)
            nc.vector.tensor_copy(rbs[:], rb[:DH, :512])
            stg = utile(pAttn, [64, 512], F32, "aot_stg", bufs=2)
            nc.vector.tensor_tensor(stg[:], av[n2][:DH, :512], rbs[:],
                                    OP.mult)
            nc.sync.dma_start(aoTd[h * 64:h * 64 + 64, ts(n2, 512)], stg[:])
    pAttn.release()
    pVP.release()
    pHT.release()
    pbA.release()

    # ================= Wo -> x_attn, x1 (-> DRAM), metric =================
    pB2 = tc.alloc_tile_pool(name="pB2", bufs=1)
    bob = brow("bo", pB2)
    xa = [utile(pB2, [128, C], F32, f"xa{m}") for m in range(8)]
    rn = psm.tile([128, 8], F32)
    for m in range(8):
        acc = ps_bigA()
        for k in range(KT):
            ao = utile(pw, [128, 128], F32, "wqkb", bufs=4)
            nc.sync.dma_start(ao[:], aoTd[ts(k, 128), ts(m, 128)])
            wo = utile(pB2, [128, C], F32, "wo", bufs=3)
            nc.sync.dma_start(wo[:], wd["Wo"][ts(k, 128), :])
            for n2 in range(2):
                nc.tensor.matmul(acc[:, ts(n2, 512)], ao[:], wo[:, ts(n2, 512)],
                                 start=(k == 0), stop=(k == KT - 1))
        nc.vector.tensor_tensor(xa[m][:], acc[:], bob[:], OP.add)
        xt = utile(pt, [128, C], F32, "xin")
        nc.sync.dma_start(xt[:], x_d[ts(m, 128), :])
        x1stg = utile(pw, [128, C], F32, "x1stg", bufs=2)
        nc.vector.tensor_tensor(x1stg[:], xa[m][:], xt[:], OP.add)
        nc.sync.dma_start(x1d[ts(m, 128), :], x1stg[:])
        ss = utile(psm, [128, 1], F32, "nrm_ss")
        sq = utile(pt, [128, C], F32, "ln_xc")
        nc.scalar.activation(sq[:], xa[m][:], AF.Square, accum_out=ss[:])
        rr = refined_rsqrt_recip(ss, "nrm")
        nc.vector.tensor_copy(rn[:, m:m + 1], rr[:])
        nc.vector.tensor_scalar(xa[m][:], xa[m][:], rn[:, m:m + 1], None, OP.mult)
        if dbg:
            nc.sync.dma_start(dbg["dbg_xattn"][ts(m, 128), :], xa[m][:])

    # ===== de-interleave metric -> maT/mbT; scores; node stats; ranks =====
    pB3 = tc.alloc_tile_pool(name="pB3", bufs=1)
    xae = [utile(pB3, [128, C], F32, f"xae{m}") for m in range(4)]
    xao = [utile(pB3, [128, C], F32, f"xao{m}") for m in range(4)]
    for m in range(4):
        nc.sync.dma_start(xae[m][:64, :], xa[2 * m][0:128:2, :])
        nc.sync.dma_start(xae[m][64:, :], xa[2 * m + 1][0:128:2, :])
        nc.sync.dma_start(xao[m][:64, :], xa[2 * m][1:128:2, :])
        nc.sync.dma_start(xao[m][64:, :], xa[2 * m + 1][1:128:2, :])
    pB4 = tc.alloc_tile_pool(name="pB4", bufs=1)
    maT = [utile(pB4, [128, NE], F32, f"maT{k}") for k in range(8)]
    mbT = [utile(pB4, [128, NE], F32, f"mbT{k}") for k in range(8)]
    transpose_blocks(xae, maT, NE, C)
    transpose_blocks(xao, mbT, NE, C)

    nm_t = psm.tile([128, 4], F32)
    ni_t = psm.tile([128, 4], F32)
    for m in range(4):
        acc = ps_bigA()
        for k in range(KT):
            nc.tensor.matmul(acc[:, :512], maT[k][:, ts(m, 128)], mbT[k][:],
                             start=(k == 0), stop=(k == KT - 1))
        mx8 = utile(psm, [128, 8], F32, "mx8")
        ix8 = utile(psm, [128, 8], U32, "ix8")
        nc.vector.max_with_indices(mx8[:], ix8[:], acc[:, :512])
        nc.vector.tensor_copy(nm_t[:, m:m + 1], mx8[:, 0:1])
        nc.vector.tensor_copy(ni_t[:, m:m + 1], ix8[:, 0:1])

    nm_row = utile(pB4, [1, 512], F32, "nm_row")
    for m in range(4):
        p = ps_av()
        pf = p.rearrange("p a b -> p (a b)")
        nc.tensor.transpose(pf[:1, :128], nm_t[:, m:m + 1], ident[:])
        nc.vector.tensor_copy(nm_row[:, ts(m, 128)], pf[:1, :128])
    NMB = bcast_row(nm_row[:], 512, "nmb", pB4)

    rank_t = psm.tile([128, 4], F32)
    for m in range(4):
        gt = utile(pB4, [128, 512], F32, "rk_gt")
        nc.vector.tensor_scalar(gt[:], NMB[:], nm_t[:, m:m + 1], None, OP.is_gt)
        eq = utile(pB4, [128, 512], F32, "rk_eq")
        nc.vector.tensor_scalar(eq[:], NMB[:], nm_t[:, m:m + 1], None, OP.is_equal)
        flt = utile(pB4, [128, 512], F32, "rk_flt")
        pio = utile(psm, [128, 1], F32, "rk_pio")
        nc.vector.tensor_scalar_add(pio[:], piota[:], float(128 * m))
        nc.vector.tensor_scalar(flt[:], IOTA512B[:], pio[:], None, OP.is_lt)
        nc.vector.tensor_tensor(eq[:], eq[:], flt[:], OP.mult)
        nc.vector.tensor_tensor(gt[:], gt[:], eq[:], OP.add)
        nc.vector.reduce_sum(rank_t[:, m:m + 1], gt[:], axis=mybir.AxisListType.X)
    if dbg:
        for (tt, nme) in [(nm_t, "dbg_nm"), (rank_t, "dbg_rank"),
                          (ni_t, "dbg_nodeidx")]:
            nc.sync.dma_start(dbg[nme].rearrange("(m p) -> p m", p=128), tt[:])
    pB4.release()
    pB3.release()
    pB2.release()

    # ================= dst merge (x1 from DRAM; dstn -> DRAM) =============
    pM = tc.alloc_tile_pool(name="pM", bufs=1)
    x1e = [utile(pM, [128, C + 8], F32, f"x1e{m}") for m in range(4)]
    x1o = [utile(pM, [128, C], F32, f"x1o{m}") for m in range(4)]
    for m in range(4):
        nc.vector.memset(x1e[m][:, C:C + 1], 1.0)
        nc.sync.dma_start(x1e[m][:, :C], x1d[256 * m:256 * m + 256:2, :])
        nc.sync.dma_start(x1o[m][:], x1d[256 * m + 1:256 * m + 256:2, :])
    st = [utile(pM, [128, 512], F32, f"st{m}") for m in range(4)]
    for m in range(4):
        msk = utile(psm, [128, 1], F32, "st_m")
        nc.vector.tensor_scalar(msk[:], rank_t[:, m:m + 1], float(R) - 0.5, None,
                                OP.is_lt)
        nc.vector.tensor_scalar(st[m][:], IOTA512B[:], ni_t[:, m:m + 1], None,
                                OP.is_equal)
        nc.vector.tensor_scalar(st[m][:], st[m][:], msk[:], None, OP.mult)
    for m in range(4):
        acc = ps_bigA()
        cacc = ps_av()
        for k in range(4):
            for n2 in range(2):
                nc.tensor.matmul(acc[:, ts(n2, 512)], st[k][:, ts(m, 128)],
                                 x1e[k][:, n2 * 512:n2 * 512 + 512],
                                 start=(k == 0), stop=(k == 3))
            nc.tensor.matmul(cacc[:, 0, :1], st[k][:, ts(m, 128)],
                             x1e[k][:, C:C + 1], start=(k == 0), stop=(k == 3))
        cnt = utile(psm, [128, 1], F32, "cnt")
        nc.vector.tensor_scalar_add(cnt[:], cacc[:, 0, 0:1], 1.0)
        rec = utile(psm, [128, 1], F32, "cntr")
        nc.vector.reciprocal(rec[:], cnt[:])
        dst_stg = utile(pM, [128, C], F32, "dst_stg", bufs=2)
        nc.vector.tensor_tensor(dst_stg[:], acc[:], x1o[m][:], OP.add)
        nc.vector.tensor_scalar(dst_stg[:], dst_stg[:], rec[:], None, OP.mult)
        nc.sync.dma_start(dstnd[ts(m, 128), :], dst_stg[:])

    # ========== MLP (f32r): W1/W2 streamed once; SBUF out accumulation ======
    def row_src_ap(i):
        if i < 4:
            return x1d[256 * i:256 * i + 256:2, :]
        return dstnd[ts(i - 4, 128), :]

    pM.release()
    pC4 = tc.alloc_tile_pool(name="pC4", bufs=1)
    g2b = brow("g2", pC4)
    be2b = brow("be2", pC4)
    h2 = []
    for i in range(8):
        rsrc = utile(pt, [128, C], F32, "xin")
        nc.sync.dma_start(rsrc[:], row_src_ap(i))
        h = utile(pt, [128, C], F32, "ht", bufs=4)
        layer_norm(rsrc, h, g2b, be2b)
        h2.append(h)
        if dbg:
            nc.sync.dma_start(dbg["dbg_mlpin"][ts(i, 128), :], rsrc[:])
    h2T = [utile(pC4, [128, N], F32R, f"h2T{k}") for k in range(8)]
    transpose_blocks(h2, h2T, N, C)

    bm1T = bcol("bm1", pC4)
    bm2b = brow("bm2", pC4)
    for q in range(4):
        oacc = [ps_bigA(), ps_bigB()]     # out token tiles 2q, 2q+1
        for mtg in range(8):
            w1cs = []
            for k in range(KT):
                w1c = utile(pC4, [128, 512], F32, "w1c", bufs=4)
                nc.sync.dma_start(w1c[:], wd["W1"][ts(k, 128), ts(mtg, 512)])
                w1cr = utile(pC4, [128, 512], F32R, f"w1cr{k}", bufs=2)
                nc.vector.tensor_copy(w1cr[:], w1c[:])
                w1cs.append(w1cr)
            for mi in range(4):
                mt = mtg * 4 + mi
                yp = ps_mm() if mi % 2 == 0 else                     ps_av().rearrange("p a b -> p (a b)")
                for k in range(KT):
                    nc.tensor.matmul(yp[:, :256], w1cs[k][:, ts(mi, 128)],
                                     h2T[k][:, q * 256:q * 256 + 256],
                                     start=(k == 0), stop=(k == KT - 1))
                g1t = utile(pC4, [128, 256], F32R, "g1t", bufs=3)
                nc.scalar.activation(g1t[:], yp[:, :256], AF.Gelu_apprx_tanh,
                                     bias=bm1T[:, mt:mt + 1])
                w2t = utile(pC4, [128, C], F32, "w2s", bufs=4)
                nc.sync.dma_start(w2t[:], wd["W2"][ts(mt, 128), :])
                w2r = utile(pC4, [128, C], F32R, "w2r", bufs=3)
                nc.vector.tensor_copy(w2r[:], w2t[:])
                for tl in range(2):
                    for n2 in range(2):
                        nc.tensor.matmul(oacc[tl][:, ts(n2, 512)],
                                         g1t[:, tl * 128:tl * 128 + 128],
                                         w2r[:, ts(n2, 512)],
                                         start=(mt == 0), stop=(mt == 31))
        for tl in range(2):
            row = 2 * q + tl
            res = utile(pt, [128, C], F32, "xin")
            nc.sync.dma_start(res[:], row_src_ap(row))
            mf = utile(pC4, [128, C], F32, "mof", bufs=2)
            nc.vector.tensor_tensor(mf[:], oacc[tl][:], bm2b[:], OP.add)
            nc.vector.tensor_tensor(mf[:], mf[:], res[:], OP.add)
            mr = utile(pC4, [128, C], F32R, "mor", bufs=2)
            nc.vector.tensor_copy(mr[:], mf[:])
            nc.sync.dma_start(mod[ts(row, 128), :], mr[:])
            if dbg:
                nc.sync.dma_start(dbg["dbg_mlpout"][ts(row, 128), :], mf[:])
    pC4.release()

    # ================= Stage D: pooling + Wp -> combined^T =================
    pD = tc.alloc_tile_pool(name="pD", bufs=1)
    # ApT[p, f] = 0.5 iff source row p pools into output f:
    #   even block: base = rank[p]-16, match iff (2f - base) in {-1, 0}
    #   dst  block: base = d,          match iff (2(f-248) - base) in {-1, 0}
    iota2e = utile(pD, [128, 504], F32, "iota2e")
    nc.vector.tensor_scalar_mul(iota2e[:], IOTA504B[:], 2.0)
    apT = [utile(pD, [128, 504], F32R, f"apT{m}") for m in range(8)]
    for m in range(8):
        base = utile(psm, [128, 1], F32, "ap_r")
        if m < 4:
            nc.vector.tensor_scalar_add(base[:], rank_t[:, m:m + 1], -float(R))
        else:
            nc.vector.tensor_scalar_add(base[:], piota[:],
                                        float(128 * (m - 4) + NE - R))
        d1 = utile(pD, [128, 504], F32, "ap_d1")
        nc.vector.tensor_scalar(d1[:], iota2e[:], base[:], None, OP.subtract)
        a1 = utile(pD, [128, 504], F32, "ap_a1")
        nc.vector.tensor_scalar(a1[:], d1[:], -1.5, None, OP.is_ge)
        b1 = utile(pD, [128, 504], F32, "ap_b1")
        nc.vector.tensor_scalar(b1[:], d1[:], 0.5, None, OP.is_le)
        nc.vector.scalar_tensor_tensor(apT[m][:], a1[:], 0.5, b1[:],
                                       OP.mult, OP.mult)
    pooledT = [utile(pD, [128, NP], F32R, f"pooledT{k}") for k in range(8)]
    for m in range(4):
        acc = ps_bigA()
        for k in range(8):
            mob = utile(pD, [128, C], F32R, "mob", bufs=3)
            nc.sync.dma_start(mob[:], mod[ts(k, 128), :])
            for n2 in range(2):
                nc.tensor.matmul(acc[:PP, ts(n2, 512)],
                                 apT[k][:, m * PP:(m + 1) * PP],
                                 mob[:, ts(n2, 512)], start=(k == 0), stop=(k == 7))
        pst = utile(pD, [128, C], F32, "pstg", bufs=2)
        nc.vector.tensor_copy(pst[:PP, :], acc[:PP, :])
        if dbg:
            nc.sync.dma_start(dbg["dbg_pooled"][ts(m, PP), :], pst[:PP, :])
        for bj in range(8):
            p = ps_av()
            pf = p.rearrange("p a b -> p (a b)")
            nc.tensor.transpose(pf[:128, :PP], pst[:PP, ts(bj, 128)],
                                ident[:PP, :PP])
            nc.vector.tensor_copy(pooledT[bj][:, m * PP:(m + 1) * PP],
                                  pf[:128, :PP])

    pE = tc.alloc_tile_pool(name="pE", bufs=1)
    bp3T = bcol("bp", pD, scale=3.0)
    cmbTr = [utile(pD, [128, NP], F32R, f"cmbTr{m}") for m in range(8)]
    for mg in range(2):
        wcs = []
        for k in range(KT):
            wc = utile(pw, [128, 512], F32, "ws4k")
            nc.sync.dma_start(wc[:], wd["Wp"][ts(k, 128), ts(mg, 512)])
            wcr = utile(pD, [128, 512], F32R, f"wpc{k}", bufs=1)
            nc.vector.tensor_copy(wcr[:], wc[:])
            wcs.append(wcr)
        for mi in range(4):
            m = mg * 4 + mi
            acc = ps_mm()
            for k in range(KT):
                nc.tensor.matmul(acc[:, :NP], wcs[k][:, ts(mi, 128)],
                                 pooledT[k][:], start=(k == 0), stop=(k == KT - 1))
            nc.scalar.activation(cmbTr[m][:], acc[:, :NP], AF.Identity,
                                 bias=bp3T[:, m:m + 1], scale=3.0)

    # ================= Stage E: MQA =================
    bqT = bcol("bq", pE)

    def make_mqT(m):
        acc = ps_mm()
        for k in range(KT):
            wb = utile(pw, [128, 128], F32, "w1b", bufs=4)
            nc.sync.dma_start(wb[:], wd["Wq"][ts(k, 128), ts(m, 128)])
            wr = utile(pw, [128, 128], F32R, "w1r", bufs=4)
            nc.vector.tensor_copy(wr[:], wb[:])
            nc.tensor.matmul(acc[:, :NP], wr[:], cmbTr[k][:],
                             start=(k == 0), stop=(k == KT - 1))
        t = utile(pE, [128, NP], F32R, "mqT", bufs=2)
        nc.scalar.activation(t[:], acc[:, :NP], AF.Identity, bias=bqT[:, m:m + 1])
        return t

    wkv = utile(pE, [128, KT, 2 * DH], F32, "wkv")
    wkvr = utile(pE, [128, KT, 2 * DH], F32R, "wkvr")
    for k in range(KT):
        nc.sync.dma_start(wkv[:, k, :DH], wd["Wk"][ts(k, 128), :])
        nc.sync.dma_start(wkv[:, k, DH:], wd["Wv"][ts(k, 128), :])
    nc.vector.tensor_copy(wkvr[:], wkv[:])
    bkT = utile(pE, [64, 1], F32, "bkT")
    nc.sync.dma_start(bkT[:], wd["bk"][:, None])
    mkT = utile(pE, [128, NP], F32R, "mkT")
    macc = ps_mm()
    for k in range(KT):
        nc.tensor.matmul(macc[:64, :NP], wkvr[:, k, :DH], cmbTr[k][:],
                         start=(k == 0), stop=(k == KT - 1))
    mkf = utile(pE, [64, NP], F32, "mkf")
    nc.scalar.activation(mkf[:], macc[:64, :NP], AF.Identity, bias=bkT[:])
    nc.vector.tensor_copy(mkT[:64, :], mkf[:])
    nc.sync.dma_start(mkT[64:, :], mkT[:64, :])

    bvb = bcast_row(load_row(wd["bv"], DH, "bv_r", pE), DH, "bv_b", pE)
    mv_pad = [utile(pE, [128, DH + 1], F32, f"mvp{m}") for m in range(4)]
    for m in range(4):
        acc = ps_av()
        for k in range(KT):
            nc.tensor.matmul(acc[:PP, 0, :DH], cmbTr[k][:, m * PP:(m + 1) * PP],
                             wkvr[:, k, DH:], start=(k == 0), stop=(k == KT - 1))
        nc.vector.memset(mv_pad[m][:], 0.0)
        nc.vector.memset(mv_pad[m][:, DH:], 1.0)
        nc.vector.tensor_tensor(mv_pad[m][:PP, :DH], acc[:PP, 0, :DH], bvb[:PP, :],
                                OP.add)

    mqT_cur = None
    for h in range(H):
        po = (h % 2) * 64
        if h % 2 == 0:
            mqT_cur = make_mqT(h // 2)
        mqT_h = mqT_cur[po:po + 64, :]
        ep = []
        for mm in range(4):
            sp = ps_mm()
            nc.tensor.matmul(sp[:PP, :NP], mkT[po:po + 64, mm * PP:(mm + 1) * PP],
                             mqT_h[:], start=True, stop=True)
            et = utile(pE, [128, NP], F32, "e2", bufs=4)
            nc.scalar.activation(et[:PP, :], sp[:PP, :NP], AF.Exp,
                                 scale=float(DH ** -0.5))
            ep.append(et)
        av2 = ps_av().rearrange("p a b -> p (a b)")
        for mm in range(4):
            nc.tensor.matmul(av2[:DH + 1, :NP], mv_pad[mm][:PP, :],
                             ep[mm][:PP, :], start=(mm == 0), stop=(mm == 3))
        rrow = utile(pE, [1, NP], F32, "rrow2", bufs=2)
        nc.vector.reciprocal(rrow[:], av2[DH:DH + 1, :NP])
        rb = ps_mm()
        nc.tensor.matmul(rb[:DH, :NP], ones_col[:, :DH], rrow[:],
                         start=True, stop=True)
        rbs = utile(pE, [64, NP], F32, "rbs2", bufs=2)
        nc.vector.tensor_copy(rbs[:], rb[:DH, :NP])
        stg = utile(pE, [64, NP], F32R, "mqstg", bufs=2)
        nc.vector.tensor_tensor(stg[:], av2[:DH, :NP], rbs[:], OP.mult)
        nc.sync.dma_start(mqaTd[h * 64:h * 64 + 64, :], stg[:])
    pE.release()
    pD.release()

    # ================= Stage F: Wmo + FFN =================
    pF = tc.alloc_tile_pool(name="pF", bufs=1)
    mqaT = [utile(pF, [128, NP], F32R, f"mqaT{k}") for k in range(8)]
    for k in range(8):
        nc.sync.dma_start(mqaT[k][:, :NP], mqaTd[ts(k, 128), :])
    bmoT = bcol("bmo", pF)
    omoT = [utile(pF, [128, NP], F32R, f"omoT{m}") for m in range(8)]
    for mg in range(2):
        wcs = []
        for k in range(KT):
            wc = utile(pw, [128, 512], F32, "ws4k")
            nc.sync.dma_start(wc[:], wd["Wmo"][ts(k, 128), ts(mg, 512)])
            wcr = utile(pF, [128, 512], F32R, f"wmc{k}", bufs=1)
            nc.vector.tensor_copy(wcr[:], wc[:])
            wcs.append(wcr)
        for mi in range(4):
            m = mg * 4 + mi
            acc = ps_mm()
            for k in range(KT):
                nc.tensor.matmul(acc[:, :NP], wcs[k][:, ts(mi, 128)],
                                 mqaT[k][:], start=(k == 0), stop=(k == KT - 1))
            nc.scalar.activation(omoT[m][:], acc[:, :NP], AF.Identity,
                                 bias=bmoT[:, m:m + 1])

    bf1T = bcol("bf1", pF)
    bf2b = brow("bf2", pF)
    for half in range(2):
        t0 = half * 252
        oacc = [ps_bigA(), ps_bigB()]
        for kkg in range(8):
            wf1cs = []
            for k in range(KT):
                wf1c = utile(pF, [128, 512], F32, "w1c", bufs=3)
                nc.sync.dma_start(wf1c[:], wd["Wf1"][ts(k, 128), ts(kkg, 512)])
                wf1cr = utile(pF, [128, 512], F32R, f"w1cr{k}", bufs=1)
                nc.vector.tensor_copy(wf1cr[:], wf1c[:])
                wf1cs.append(wf1cr)
            for ki in range(4):
                kk = kkg * 4 + ki
                yp = ps_mm() if ki % 2 == 0 else                     ps_av().rearrange("p a b -> p (a b)")
                for k in range(KT):
                    nc.tensor.matmul(yp[:, :252], wf1cs[k][:, ts(ki, 128)],
                                     omoT[k][:, t0:t0 + 252],
                                     start=(k == 0), stop=(k == KT - 1))
                g2t = utile(pF, [128, 252], F32R, "g2t", bufs=3)
                nc.scalar.activation(g2t[:], yp[:, :252], AF.Silu,
                                     bias=bf1T[:, kk:kk + 1])
                wf2t = utile(pF, [128, C], F32, "w2s", bufs=3)
                nc.sync.dma_start(wf2t[:], wd["Wf2"][ts(kk, 128), :])
                wf2r = utile(pF, [128, C], F32R, "wf2r", bufs=3)
                nc.vector.tensor_copy(wf2r[:], wf2t[:])
                for tl in range(2):
                    for n2 in range(2):
                        nc.tensor.matmul(oacc[tl][:PP, ts(n2, 512)],
                                         g2t[:, tl * 126:tl * 126 + 126],
                                         wf2r[:, ts(n2, 512)],
                                         start=(kk == 0), stop=(kk == 31))
        for tl in range(2):
            row0 = (2 * half + tl) * PP
            of = utile(pF, [128, C], F32, "of", bufs=2)
            nc.vector.tensor_tensor(of[:PP, :], oacc[tl][:PP, :], bf2b[:PP, :],
                                    OP.add)
            nc.sync.dma_start(out_d[row0:row0 + PP, :], of[:PP, :])
    pF.release()
    for pool in (pt, pw, psm, pc, pp):
        pool.release()


_BUILT = None


def kernel(**inputs):
    global _BUILT
    if _BUILT is None:
        _BUILT = build(debug=DEBUG)
    nc = _BUILT
    x = np.ascontiguousarray(inputs["x"], dtype=np.float32)
    base = {k: np.ascontiguousarray(v, dtype=np.float32) for k, v in inputs.items()
            if k != "x"}
    in_maps = []
    for i in range(8):
        m = dict(base)
        m["x"] = x[i]
        in_maps.append(m)
    res = run_bass_kernel_spmd(nc, in_maps, core_ids=list(range(8)))
    out = np.stack([res.results[i]["out"] for i in range(8)], axis=0)
    return out.astype(np.float32)


# revision 45
# speedup vs baseline: 6010.8611x; 1.0074x over previous
"""AdaptiveTokenMerger (ToMe block + merger) TRN2 Bass kernel.

Data-parallel over batch: 8 samples -> 8 NeuronCores, one sample per core.
Per-core pipeline (sample x [1024, 1024]):
  A (f32, ranking-critical): LN1 -> qkv -> MHA (transposed-softmax with the
    denominator folded in as an appended ones-column of v) -> Wo -> x_attn
  B: metric scores -> node_max/argmax -> ranks via pairwise comparisons ->
    dst scatter-add expressed as a one-hot matmul
  C (f32r): MLP over rows [x1_even(512); dst_new(512)], fused W1/W2 per
    token-quarter, output accumulated in PSUM across all 32 W1 column tiles
  D: pooling as a rank-dependent one-hot matmul -> Wp -> combined = 3q
  E (f32r): multi-query attention  F (f32r): FFN -> out [504, 1024]

Precision: everything upstream of the rank/argmax decisions is true fp32
(4 cyc/row on PE); post-merge matmuls use float32r (TF32-ish, 1 cyc/row).

PSUM budget (8 banks): BIGA/BIGB [128,1024] (2+2), MM [128,512] x2 (2),
AV [128,4,128] x2 (2).
"""
import numpy as np

import concourse.bass as bass
import concourse.tile as tile
from concourse import bacc, mybir
from concourse.bass import ts
from concourse.bass_utils import run_bass_kernel_spmd
from concourse.masks import make_identity

F32 = mybir.dt.float32
F32R = mybir.dt.float32r
U32 = mybir.dt.uint32

N, C, H = 1024, 1024, 16
R = 16
DH = C // H          # 64
NE = N // 2          # 512
NP = (N - R) // 2    # 504
PP = 126             # pooled tokens per partition tile
KT = C // 128        # 8
AF = mybir.ActivationFunctionType
OP = mybir.AluOpType

DEBUG = False


def build(debug=False):
    nc = bacc.Bacc("TRN2", target_bir_lowering=False, debug=False, num_devices=8)
    x_d = nc.dram_tensor("x", [N, C], F32, kind="ExternalInput").ap()
    wd = {}
    for name, shape in [
        ("g1", [C]), ("be1", [C]), ("Wqkv", [C, 3 * C]), ("bqkv", [3 * C]),
        ("Wo", [C, C]), ("bo", [C]), ("g2", [C]), ("be2", [C]),
        ("W1", [C, 4 * C]), ("bm1", [4 * C]), ("W2", [4 * C, C]), ("bm2", [C]),
        ("Wp", [C, C]), ("bp", [C]), ("Wq", [C, C]), ("bq", [C]),
        ("Wk", [C, DH]), ("bk", [DH]), ("Wv", [C, DH]), ("bv", [DH]),
        ("Wmo", [C, C]), ("bmo", [C]), ("Wf1", [C, 4 * C]), ("bf1", [4 * C]),
        ("Wf2", [4 * C, C]), ("bf2", [C]),
    ]:
        wd[name] = nc.dram_tensor(name, shape, F32, kind="ExternalInput").ap()
    out_d = nc.dram_tensor("out", [NP, C], F32, kind="ExternalOutput").ap()
    dbg = {}
    if debug:
        for name, shape in [
            ("dbg_xattn", [N, C]), ("dbg_nm", [NE]), ("dbg_rank", [NE]),
            ("dbg_nodeidx", [NE]), ("dbg_mlpin", [N, C]), ("dbg_mlpout", [N, C]),
            ("dbg_pooled", [NP, C]),
        ]:
            dbg[name] = nc.dram_tensor(name, shape, F32, kind="ExternalOutput").ap()
    with tile.TileContext(nc) as tc:
        _build_tile(nc, tc, x_d, wd, out_d, dbg)
    nc.compile()
    return nc


def _build_tile(nc, tc, x_d, wd, out_d, dbg):
    # DRAM spill buffers
    qkTd = nc.dram_tensor("qkTd", [2 * C, N], F32).ap()
    aoTd = nc.dram_tensor("aoTd", [C, N], F32).ap()
    h2d = nc.dram_tensor("h2d", [C, N], F32R).ap()
    x1d = nc.dram_tensor("x1d", [N, C], F32).ap()
    dstnd = nc.dram_tensor("dstnd", [NE, C], F32).ap()
    mod = nc.dram_tensor("mod", [N, C], F32R).ap()
    mqaTd = nc.dram_tensor("mqaTd", [C, NP], F32R).ap()

    pc = tc.alloc_tile_pool(name="const", bufs=1)
    psm = tc.alloc_tile_pool(name="small", bufs=1)
    pw = tc.alloc_tile_pool(name="wstream", bufs=2)
    pt = tc.alloc_tile_pool(name="tmp", bufs=2)
    pp = tc.alloc_tile_pool(name="psum", bufs=1, space="PSUM")

    _ct = {}

    def utile(pool, shape, dtype, tag, bufs=None):
        _ct[tag] = _ct.get(tag, 0) + 1
        kw = {"bufs": bufs} if bufs is not None else {}
        return pool.tile(shape, dtype, tag=tag, name=f"{tag}_{_ct[tag]}", **kw)

    def ps_bigA():
        return utile(pp, [128, 1024], F32, "BIGA")

    def ps_bigB():
        return utile(pp, [128, 1024], F32, "BIGB")

    def ps_mm():
        return utile(pp, [128, 512], F32, "MM", bufs=2)

    def ps_av():
        return utile(pp, [128, 4, 128], F32, "AV", bufs=2)

    # ---------- constants ----------
    ident = pc.tile([128, 128], F32)
    make_identity(nc, ident[:])
    ones_col = pc.tile([1, 128], F32)
    nc.gpsimd.memset(ones_col[:], 1.0)
    piota = pc.tile([128, 1], F32)
    nc.gpsimd.iota(piota[:], [[0, 1]], channel_multiplier=1,
                   allow_small_or_imprecise_dtypes=True)
    iota512_row = pc.tile([1, 512], F32)
    nc.gpsimd.iota(iota512_row[:], [[1, 512]], channel_multiplier=0,
                   allow_small_or_imprecise_dtypes=True)
    iota504_row = pc.tile([1, 504], F32)
    nc.gpsimd.iota(iota504_row[:], [[1, 504]], channel_multiplier=0,
                   allow_small_or_imprecise_dtypes=True)

    def bcast_row(row_ap, n, tag, pool, scale=1.0):
        t = utile(pool, [128, n], F32, tag)
        for c0 in range(0, n, 512):
            cw = min(512, n - c0)
            p = ps_mm()
            nc.tensor.matmul(p[:, :cw], ones_col[:], row_ap[:, c0:c0 + cw],
                             start=True, stop=True)
            if scale == 1.0:
                nc.vector.tensor_copy(t[:, c0:c0 + cw], p[:, :cw])
            else:
                nc.vector.tensor_scalar_mul(t[:, c0:c0 + cw], p[:, :cw], scale)
        return t

    def load_row(dram_ap, n, tag, pool):
        t = utile(pw, [1, n], F32, "rowstg", bufs=1)
        nc.sync.dma_start(t[:], dram_ap[None, :])
        return t

    def brow(name, pool, scale=1.0):
        n = wd[name].shape[0]
        return bcast_row(load_row(wd[name], n, name + "_r", pool), n,
                         name + "_b", pool, scale)

    def bcol(name, pool, scale=1.0):
        n = wd[name].shape[0]
        t = utile(pool, [128, n // 128], F32, name + "_c")
        nc.sync.dma_start(t[:], wd[name].rearrange("(t p) -> p t", p=128))
        if scale != 1.0:
            nc.vector.tensor_scalar_mul(t[:], t[:], scale)
        return t

    IOTA512B = bcast_row(iota512_row[:], 512, "iota512b", pc)
    IOTA504B = bcast_row(iota504_row[:], 504, "iota504b", pc)

    def transpose_blocks(src_tiles, dst, n_rows, n_cols):
        """dst[c, r] = src[r, c]; dst is tile-list or sink(bj, bi, pf, cw, rw)."""
        for bi in range((n_rows + 127) // 128):
            rw = min(128, n_rows - bi * 128)
            for bj in range((n_cols + 127) // 128):
                cw = min(128, n_cols - bj * 128)
                p = ps_av()
                pf = p.rearrange("p a b -> p (a b)")
                nc.tensor.transpose(pf[:cw, :rw],
                                    src_tiles[bi][:rw, bj * 128:bj * 128 + cw],
                                    ident[:rw, :rw])
                if callable(dst):
                    dst(bj, bi, pf, cw, rw)
                else:
                    nc.vector.tensor_copy(dst[bj][:cw, bi * 128:bi * 128 + rw],
                                          pf[:cw, :rw])

    def refined_rsqrt_recip(vv, tag):
        """returns 1/sqrt(vv) with one Newton step on sqrt (ACT sqrt is loose)."""
        s0 = utile(psm, [128, 1], F32, tag + "_s0")
        nc.scalar.sqrt(s0[:], vv[:])
        r0 = utile(psm, [128, 1], F32, tag + "_r0")
        nc.vector.reciprocal(r0[:], s0[:])
        t = utile(psm, [128, 1], F32, tag + "_t")
        nc.vector.tensor_tensor(t[:], vv[:], r0[:], OP.mult)
        nc.vector.tensor_tensor(t[:], t[:], s0[:], OP.add)
        nc.vector.tensor_scalar_mul(t[:], t[:], 0.5)
        rr = utile(psm, [128, 1], F32, tag + "_rr")
        nc.vector.reciprocal(rr[:], t[:])
        return rr

    def layer_norm(src, dst, gb, bb):
        m = utile(psm, [128, 1], F32, "ln_m")
        nc.vector.reduce_sum(m[:], src[:, :C], axis=mybir.AxisListType.X)
        nc.vector.tensor_scalar_mul(m[:], m[:], 1.0 / C)
        xc = utile(pt, [128, C], F32, "ln_xc")
        nc.vector.tensor_scalar(xc[:], src[:, :C], m[:], None, OP.subtract)
        ss = utile(psm, [128, 1], F32, "ln_ss")
        nc.scalar.activation(dst[:, :C], xc[:], AF.Square, accum_out=ss[:])
        v = utile(psm, [128, 1], F32, "ln_v")
        nc.vector.tensor_scalar(v[:], ss[:], 1.0 / C, 1e-5, OP.mult, OP.add)
        rstd = refined_rsqrt_recip(v, "ln")
        nc.vector.tensor_scalar(dst[:, :C], xc[:], rstd[:], None, OP.mult)
        nc.vector.tensor_tensor(dst[:, :C], dst[:, :C], gb[:], OP.mult)
        nc.vector.tensor_tensor(dst[:, :C], dst[:, :C], bb[:], OP.add)

    # ================= Stage A: LN1 -> hT =================
    pbA = tc.alloc_tile_pool(name="biasA", bufs=1)
    pHT = tc.alloc_tile_pool(name="pHT", bufs=1)
    pVP = tc.alloc_tile_pool(name="pVP", bufs=1)
    pAttn = tc.alloc_tile_pool(name="pAttn", bufs=1)

    g1b = brow("g1", pbA)
    be1b = brow("be1", pbA)
    hT = [utile(pHT, [128, N], F32, f"hT{k}") for k in range(8)]
    ht = []
    for i in range(8):
        xt = utile(pt, [128, C], F32, "xin")
        nc.sync.dma_start(xt[:], x_d[ts(i, 128), :])
        h = utile(pt, [128, C], F32, "ht", bufs=4)
        layer_norm(xt, h, g1b, be1b)
        ht.append(h)
    transpose_blocks(ht, hT, N, C)

    # ===== qk^T -> qkTd (DRAM) ; v_pad (SBUF) =====
    bqkT = bcol("bqkv", pbA)
    for mp in range(8):
        accq = ps_bigA()
        acck = ps_bigB()
        for k in range(KT):
            wq = utile(pw, [128, 128], F32, "wqkb", bufs=4)
            nc.sync.dma_start(wq[:], wd["Wqkv"][ts(k, 128), ts(mp, 128)])
            wk = utile(pw, [128, 128], F32, "wqkb", bufs=4)
            nc.sync.dma_start(wk[:],
                              wd["Wqkv"][ts(k, 128), C + mp * 128:C + (mp + 1) * 128])
            for n2 in range(2):
                nc.tensor.matmul(accq[:, ts(n2, 512)], wq[:], hT[k][:, ts(n2, 512)],
                                 start=(k == 0), stop=(k == KT - 1))
                nc.tensor.matmul(acck[:, ts(n2, 512)], wk[:], hT[k][:, ts(n2, 512)],
                                 start=(k == 0), stop=(k == KT - 1))
        stgq = utile(pAttn, [128, N], F32, "qkstg", bufs=2)
        nc.scalar.activation(stgq[:], accq[:], AF.Identity, bias=bqkT[:, mp:mp + 1])
        nc.sync.dma_start(qkTd[ts(mp, 128), :], stgq[:])
        stgk = utile(pAttn, [128, N], F32, "qkstg", bufs=2)
        nc.scalar.activation(stgk[:], acck[:], AF.Identity,
                             bias=bqkT[:, 8 + mp:9 + mp])
        nc.sync.dma_start(qkTd[C + mp * 128:C + (mp + 1) * 128, :], stgk[:])

    bvqkvb = bcast_row(load_row(wd["bqkv"][2 * C:], C, "bvq_r", pbA), C,
                       "bvq_b", pbA)
    v_pad = [utile(pVP, [128, H, DH + 1], F32, f"vp{j}") for j in range(8)]
    for j in range(8):
        nc.vector.memset(v_pad[j][:, :, DH:DH + 1], 1.0)
        acc = ps_bigA()
        for k in range(KT):
            wv = utile(pVP, [128, C], F32, "wv", bufs=3)
            nc.sync.dma_start(wv[:], wd["Wqkv"][ts(k, 128), 2 * C:])
            for n2 in range(2):
                nc.tensor.matmul(acc[:, ts(n2, 512)], hT[k][:, ts(j, 128)],
                                 wv[:, ts(n2, 512)],
                                 start=(k == 0), stop=(k == KT - 1))
        for h in range(H):
            nc.vector.tensor_tensor(v_pad[j][:, h, :DH], acc[:, ts(h, DH)],
                                    bvqkvb[:, ts(h, DH)], OP.add)

    # ===== attention: stream kT/qT per head; out -> aoTd (already c-major) ==
    # out[dh|sum, i] = v_pad[j].T @ expT[j, i], accumulated over j-tiles.
    for h in range(H):
        kth = utile(pAttn, [64, N], F32, "kth", bufs=2)
        nc.sync.dma_start(kth[:], qkTd[C + h * 64:C + h * 64 + 64, :])
        qth = utile(pAttn, [64, N], F32, "qth", bufs=2)
        nc.sync.dma_start(qth[:], qkTd[h * 64:h * 64 + 64, :])
        av = [ps_av().rearrange("p a b -> p (a b)") for _ in range(2)]
        for j in range(8):
            for n2 in range(2):
                sp = ps_mm()
                nc.tensor.matmul(sp[:], kth[:, ts(j, 128)], qth[:, ts(n2, 512)],
                                 start=True, stop=True)
                et = utile(pAttn, [128, 512], F32, "exp", bufs=3)
                nc.scalar.activation(et[:], sp[:], AF.Exp, scale=float(DH ** -0.5))
                nc.tensor.matmul(av[n2][:DH + 1, :512], v_pad[j][:, h, :], et[:],
                                 start=(j == 0), stop=(j == 7))
        for n2 in range(2):
            rrow = utile(pAttn, [1, 512], F32, "rrow", bufs=2)
            nc.vector.reciprocal(rrow[:], av[n2][DH:DH + 1, :512])
            rb = ps_mm()
            nc.tensor.matmul(rb[:DH, :512], ones_col[:, :DH], rrow[:],
                             start=True, stop=True)
            rbs = utile(pAttn, [64, 512], F32, "rbs", bufs=2)
            nc.vector.tensor_copy(rbs[:], rb[:DH, :512])
            stg = utile(pAttn, [64, 512], F32, "aot_stg", bufs=2)
            nc.vector.tensor_tensor(stg[:], av[n2][:DH, :512], rbs[:],
                                    OP.mult)
            nc.sync.dma_start(aoTd[h * 64:h * 64 + 64, ts(n2, 512)], stg[:])
    pAttn.release()
    pVP.release()
    pHT.release()
    pbA.release()

    # ================= Wo -> x_attn, x1 (-> DRAM), metric =================
    pB2 = tc.alloc_tile_pool(name="pB2", bufs=1)
    bob = brow("bo", pB2)
    xa = [utile(pB2, [128, C], F32, f"xa{m}") for m in range(8)]
    woR = [utile(pB2, [128, C], F32, f"woR{k}") for k in range(8)]
    for k in range(KT):
        nc.sync.dma_start(woR[k][:], wd["Wo"][ts(k, 128), :])
    rn = psm.tile([128, 8], F32)
    for m in range(8):
        acc = ps_bigA()
        for k in range(KT):
            ao = utile(pw, [128, 128], F32, "wqkb", bufs=4)
            nc.sync.dma_start(ao[:], aoTd[ts(k, 128), ts(m, 128)])
            for n2 in range(2):
                nc.tensor.matmul(acc[:, ts(n2, 512)], ao[:],
                                 woR[k][:, ts(n2, 512)],
                                 start=(k == 0), stop=(k == KT - 1))
        nc.vector.tensor_tensor(xa[m][:], acc[:], bob[:], OP.add)
        xt = utile(pt, [128, C], F32, "xin")
        nc.sync.dma_start(xt[:], x_d[ts(m, 128), :])
        x1stg = utile(pw, [128, C], F32, "x1stg", bufs=2)
        nc.vector.tensor_tensor(x1stg[:], xa[m][:], xt[:], OP.add)
        nc.sync.dma_start(x1d[ts(m, 128), :], x1stg[:])
        ss = utile(psm, [128, 1], F32, "nrm_ss")
        sq = utile(pt, [128, C], F32, "ln_xc")
        nc.scalar.activation(sq[:], xa[m][:], AF.Square, accum_out=ss[:])
        rr = refined_rsqrt_recip(ss, "nrm")
        nc.vector.tensor_copy(rn[:, m:m + 1], rr[:])
        nc.vector.tensor_scalar(xa[m][:], xa[m][:], rn[:, m:m + 1], None, OP.mult)
        if dbg:
            nc.sync.dma_start(dbg["dbg_xattn"][ts(m, 128), :], xa[m][:])

    # ===== de-interleave metric -> maT/mbT; scores; node stats; ranks =====
    pB3 = tc.alloc_tile_pool(name="pB3", bufs=1)
    xae = [utile(pB3, [128, C], F32, f"xae{m}") for m in range(4)]
    xao = [utile(pB3, [128, C], F32, f"xao{m}") for m in range(4)]
    for m in range(4):
        nc.sync.dma_start(xae[m][:64, :], xa[2 * m][0:128:2, :])
        nc.sync.dma_start(xae[m][64:, :], xa[2 * m + 1][0:128:2, :])
        nc.sync.dma_start(xao[m][:64, :], xa[2 * m][1:128:2, :])
        nc.sync.dma_start(xao[m][64:, :], xa[2 * m + 1][1:128:2, :])
    pB4 = tc.alloc_tile_pool(name="pB4", bufs=1)
    maT = [utile(pB4, [128, NE], F32, f"maT{k}") for k in range(8)]
    mbT = [utile(pB4, [128, NE], F32, f"mbT{k}") for k in range(8)]
    transpose_blocks(xae, maT, NE, C)
    transpose_blocks(xao, mbT, NE, C)

    nm_t = psm.tile([128, 4], F32)
    ni_t = psm.tile([128, 4], F32)
    for m in range(4):
        acc = ps_bigA()
        for k in range(KT):
            nc.tensor.matmul(acc[:, :512], maT[k][:, ts(m, 128)], mbT[k][:],
                             start=(k == 0), stop=(k == KT - 1))
        mx8 = utile(psm, [128, 8], F32, "mx8")
        ix8 = utile(psm, [128, 8], U32, "ix8")
        nc.vector.max_with_indices(mx8[:], ix8[:], acc[:, :512])
        nc.vector.tensor_copy(nm_t[:, m:m + 1], mx8[:, 0:1])
        nc.vector.tensor_copy(ni_t[:, m:m + 1], ix8[:, 0:1])

    nm_row = utile(pB4, [1, 512], F32, "nm_row")
    for m in range(4):
        p = ps_av()
        pf = p.rearrange("p a b -> p (a b)")
        nc.tensor.transpose(pf[:1, :128], nm_t[:, m:m + 1], ident[:])
        nc.vector.tensor_copy(nm_row[:, ts(m, 128)], pf[:1, :128])
    NMB = bcast_row(nm_row[:], 512, "nmb", pB4)

    rank_t = psm.tile([128, 4], F32)
    for m in range(4):
        gt = utile(pB4, [128, 512], F32, "rk_gt", bufs=1)
        nc.vector.tensor_scalar(gt[:], NMB[:], nm_t[:, m:m + 1], None, OP.is_gt)
        eq = utile(pB4, [128, 512], F32, "rk_eq", bufs=1)
        nc.vector.tensor_scalar(eq[:], NMB[:], nm_t[:, m:m + 1], None, OP.is_equal)
        flt = utile(pB4, [128, 512], F32, "rk_flt", bufs=1)
        pio = utile(psm, [128, 1], F32, "rk_pio")
        nc.vector.tensor_scalar_add(pio[:], piota[:], float(128 * m))
        nc.vector.tensor_scalar(flt[:], IOTA512B[:], pio[:], None, OP.is_lt)
        nc.vector.tensor_tensor(eq[:], eq[:], flt[:], OP.mult)
        nc.vector.tensor_tensor(gt[:], gt[:], eq[:], OP.add)
        nc.vector.reduce_sum(rank_t[:, m:m + 1], gt[:], axis=mybir.AxisListType.X)
    if dbg:
        for (tt, nme) in [(nm_t, "dbg_nm"), (rank_t, "dbg_rank"),
                          (ni_t, "dbg_nodeidx")]:
            nc.sync.dma_start(dbg[nme].rearrange("(m p) -> p m", p=128), tt[:])
    pB4.release()
    pB3.release()
    pB2.release()

    # ================= dst merge (x1 from DRAM; dstn -> DRAM) =============
    pM = tc.alloc_tile_pool(name="pM", bufs=1)
    x1e = [utile(pM, [128, C + 8], F32, f"x1e{m}") for m in range(4)]
    x1o = [utile(pM, [128, C], F32, f"x1o{m}") for m in range(4)]
    for m in range(4):
        nc.vector.memset(x1e[m][:, C:C + 1], 1.0)
        nc.sync.dma_start(x1e[m][:, :C], x1d[256 * m:256 * m + 256:2, :])
        nc.sync.dma_start(x1o[m][:], x1d[256 * m + 1:256 * m + 256:2, :])
    st = [utile(pM, [128, 512], F32, f"st{m}") for m in range(4)]
    for m in range(4):
        msk = utile(psm, [128, 1], F32, "st_m")
        nc.vector.tensor_scalar(msk[:], rank_t[:, m:m + 1], float(R) - 0.5, None,
                                OP.is_lt)
        nc.vector.tensor_scalar(st[m][:], IOTA512B[:], ni_t[:, m:m + 1], None,
                                OP.is_equal)
        nc.vector.tensor_scalar(st[m][:], st[m][:], msk[:], None, OP.mult)
    for m in range(4):
        acc = ps_bigA()
        cacc = ps_av()
        for k in range(4):
            for n2 in range(2):
                nc.tensor.matmul(acc[:, ts(n2, 512)], st[k][:, ts(m, 128)],
                                 x1e[k][:, n2 * 512:n2 * 512 + 512],
                                 start=(k == 0), stop=(k == 3))
            nc.tensor.matmul(cacc[:, 0, :1], st[k][:, ts(m, 128)],
                             x1e[k][:, C:C + 1], start=(k == 0), stop=(k == 3))
        cnt = utile(psm, [128, 1], F32, "cnt")
        nc.vector.tensor_scalar_add(cnt[:], cacc[:, 0, 0:1], 1.0)
        rec = utile(psm, [128, 1], F32, "cntr")
        nc.vector.reciprocal(rec[:], cnt[:])
        dst_stg = utile(pM, [128, C], F32, "dst_stg", bufs=2)
        nc.vector.tensor_tensor(dst_stg[:], acc[:], x1o[m][:], OP.add)
        nc.vector.tensor_scalar(dst_stg[:], dst_stg[:], rec[:], None, OP.mult)
        nc.sync.dma_start(dstnd[ts(m, 128), :], dst_stg[:])

    # ========== MLP (f32r): W1/W2 streamed once; SBUF out accumulation ======
    def row_src_ap(i):
        if i < 4:
            return x1d[256 * i:256 * i + 256:2, :]
        return dstnd[ts(i - 4, 128), :]

    pM.release()
    pC4 = tc.alloc_tile_pool(name="pC4", bufs=1)
    g2b = brow("g2", pC4)
    be2b = brow("be2", pC4)
    h2 = []
    for i in range(8):
        rsrc = utile(pt, [128, C], F32, "xin")
        nc.sync.dma_start(rsrc[:], row_src_ap(i))
        h = utile(pt, [128, C], F32, "ht", bufs=4)
        layer_norm(rsrc, h, g2b, be2b)
        h2.append(h)
        if dbg:
            nc.sync.dma_start(dbg["dbg_mlpin"][ts(i, 128), :], rsrc[:])
    h2T = [utile(pC4, [128, N], F32R, f"h2T{k}") for k in range(8)]
    transpose_blocks(h2, h2T, N, C)

    bm1T = bcol("bm1", pC4)
    bm2b = brow("bm2", pC4)
    for q in range(4):
        oacc = [ps_bigA(), ps_bigB()]     # out token tiles 2q, 2q+1
        for mtg in range(8):
            w1cs = []
            for k in range(KT):
                w1c = utile(pC4, [128, 512], F32, "w1c", bufs=4)
                nc.sync.dma_start(w1c[:], wd["W1"][ts(k, 128), ts(mtg, 512)])
                w1cr = utile(pC4, [128, 512], F32R, f"w1cr{k}", bufs=2)
                nc.vector.tensor_copy(w1cr[:], w1c[:])
                w1cs.append(w1cr)
            for mi in range(4):
                mt = mtg * 4 + mi
                yp = ps_mm() if mi % 2 == 0 else                     ps_av().rearrange("p a b -> p (a b)")
                for k in range(KT):
                    nc.tensor.matmul(yp[:, :256], w1cs[k][:, ts(mi, 128)],
                                     h2T[k][:, q * 256:q * 256 + 256],
                                     start=(k == 0), stop=(k == KT - 1))
                g1t = utile(pC4, [128, 256], F32R, "g1t", bufs=3)
                nc.scalar.activation(g1t[:], yp[:, :256], AF.Gelu_apprx_tanh,
                                     bias=bm1T[:, mt:mt + 1])
                w2t = utile(pC4, [128, C], F32, "w2s", bufs=4)
                nc.sync.dma_start(w2t[:], wd["W2"][ts(mt, 128), :])
                w2r = utile(pC4, [128, C], F32R, "w2r", bufs=3)
                nc.vector.tensor_copy(w2r[:], w2t[:])
                for tl in range(2):
                    for n2 in range(2):
                        nc.tensor.matmul(oacc[tl][:, ts(n2, 512)],
                                         g1t[:, tl * 128:tl * 128 + 128],
                                         w2r[:, ts(n2, 512)],
                                         start=(mt == 0), stop=(mt == 31))
        for tl in range(2):
            row = 2 * q + tl
            res = utile(pt, [128, C], F32, "xin")
            nc.sync.dma_start(res[:], row_src_ap(row))
            mf = utile(pC4, [128, C], F32, "mof", bufs=2)
            nc.vector.tensor_tensor(mf[:], oacc[tl][:], bm2b[:], OP.add)
            nc.vector.tensor_tensor(mf[:], mf[:], res[:], OP.add)
            mr = utile(pC4, [128, C], F32R, "mor", bufs=2)
            nc.vector.tensor_copy(mr[:], mf[:])
            nc.sync.dma_start(mod[ts(row, 128), :], mr[:])
            if dbg:
                nc.sync.dma_start(dbg["dbg_mlpout"][ts(row, 128), :], mf[:])
    pC4.release()

    # ================= Stage D: pooling + Wp -> combined^T =================
    pD = tc.alloc_tile_pool(name="pD", bufs=1)
    # ApT[p, f] = 0.5 iff source row p pools into output f:
    #   even block: base = rank[p]-16, match iff (2f - base) in {-1, 0}
    #   dst  block: base = d,          match iff (2(f-248) - base) in {-1, 0}
    iota2e = utile(pD, [128, 504], F32, "iota2e")
    nc.vector.tensor_scalar_mul(iota2e[:], IOTA504B[:], 2.0)
    apT = [utile(pD, [128, 504], F32R, f"apT{m}") for m in range(8)]
    for m in range(8):
        base = utile(psm, [128, 1], F32, "ap_r")
        if m < 4:
            nc.vector.tensor_scalar_add(base[:], rank_t[:, m:m + 1], -float(R))
        else:
            nc.vector.tensor_scalar_add(base[:], piota[:],
                                        float(128 * (m - 4) + NE - R))
        d1 = utile(pD, [128, 504], F32, "ap_d1")
        nc.vector.tensor_scalar(d1[:], iota2e[:], base[:], None, OP.subtract)
        a1 = utile(pD, [128, 504], F32, "ap_a1")
        nc.vector.tensor_scalar(a1[:], d1[:], -1.5, None, OP.is_ge)
        b1 = utile(pD, [128, 504], F32, "ap_b1")
        nc.vector.tensor_scalar(b1[:], d1[:], 0.5, None, OP.is_le)
        nc.vector.scalar_tensor_tensor(apT[m][:], a1[:], 0.5, b1[:],
                                       OP.mult, OP.mult)
    pooledT = [utile(pD, [128, NP], F32R, f"pooledT{k}") for k in range(8)]
    for m in range(4):
        acc = ps_bigA()
        for k in range(8):
            mob = utile(pD, [128, C], F32R, "mob", bufs=3)
            nc.sync.dma_start(mob[:], mod[ts(k, 128), :])
            for n2 in range(2):
                nc.tensor.matmul(acc[:PP, ts(n2, 512)],
                                 apT[k][:, m * PP:(m + 1) * PP],
                                 mob[:, ts(n2, 512)], start=(k == 0), stop=(k == 7))
        pst = utile(pD, [128, C], F32, "pstg", bufs=2)
        nc.vector.tensor_copy(pst[:PP, :], acc[:PP, :])
        if dbg:
            nc.sync.dma_start(dbg["dbg_pooled"][ts(m, PP), :], pst[:PP, :])
        for bj in range(8):
            p = ps_av()
            pf = p.rearrange("p a b -> p (a b)")
            nc.tensor.transpose(pf[:128, :PP], pst[:PP, ts(bj, 128)],
                                ident[:PP, :PP])
            nc.vector.tensor_copy(pooledT[bj][:, m * PP:(m + 1) * PP],
                                  pf[:128, :PP])

    pE = tc.alloc_tile_pool(name="pE", bufs=1)
    bp3T = bcol("bp", pD, scale=3.0)
    cmbTr = [utile(pD, [128, NP], F32R, f"cmbTr{m}") for m in range(8)]
    for mg in range(2):
        wcs = []
        for k in range(KT):
            wc = utile(pw, [128, 512], F32, "ws4k")
            nc.sync.dma_start(wc[:], wd["Wp"][ts(k, 128), ts(mg, 512)])
            wcr = utile(pD, [128, 512], F32R, f"wpc{k}", bufs=1)
            nc.vector.tensor_copy(wcr[:], wc[:])
            wcs.append(wcr)
        for mi in range(4):
            m = mg * 4 + mi
            acc = ps_mm()
            for k in range(KT):
                nc.tensor.matmul(acc[:, :NP], wcs[k][:, ts(mi, 128)],
                                 pooledT[k][:], start=(k == 0), stop=(k == KT - 1))
            nc.scalar.activation(cmbTr[m][:], acc[:, :NP], AF.Identity,
                                 bias=bp3T[:, m:m + 1], scale=3.0)

    # ================= Stage E: MQA =================
    bqT = bcol("bq", pE)

    def make_mqT(m):
        acc = ps_mm()
        for k in range(KT):
            wb = utile(pw, [128, 128], F32, "w1b", bufs=4)
            nc.sync.dma_start(wb[:], wd["Wq"][ts(k, 128), ts(m, 128)])
            wr = utile(pw, [128, 128], F32R, "w1r", bufs=4)
            nc.vector.tensor_copy(wr[:], wb[:])
            nc.tensor.matmul(acc[:, :NP], wr[:], cmbTr[k][:],
                             start=(k == 0), stop=(k == KT - 1))
        t = utile(pE, [128, NP], F32R, "mqT", bufs=2)
        nc.scalar.activation(t[:], acc[:, :NP], AF.Identity, bias=bqT[:, m:m + 1])
        return t

    wkv = utile(pE, [128, KT, 2 * DH], F32, "wkv")
    wkvr = utile(pE, [128, KT, 2 * DH], F32R, "wkvr")
    for k in range(KT):
        nc.sync.dma_start(wkv[:, k, :DH], wd["Wk"][ts(k, 128), :])
        nc.sync.dma_start(wkv[:, k, DH:], wd["Wv"][ts(k, 128), :])
    nc.vector.tensor_copy(wkvr[:], wkv[:])
    bkT = utile(pE, [64, 1], F32, "bkT")
    nc.sync.dma_start(bkT[:], wd["bk"][:, None])
    mkT = utile(pE, [128, NP], F32R, "mkT")
    macc = ps_mm()
    for k in range(KT):
        nc.tensor.matmul(macc[:64, :NP], wkvr[:, k, :DH], cmbTr[k][:],
                         start=(k == 0), stop=(k == KT - 1))
    mkf = utile(pE, [64, NP], F32, "mkf")
    nc.scalar.activation(mkf[:], macc[:64, :NP], AF.Identity, bias=bkT[:])
    nc.vector.tensor_copy(mkT[:64, :], mkf[:])
    nc.sync.dma_start(mkT[64:, :], mkT[:64, :])

    bvb = bcast_row(load_row(wd["bv"], DH, "bv_r", pE), DH, "bv_b", pE)
    mv_pad = [utile(pE, [128, DH + 1], F32, f"mvp{m}") for m in range(4)]
    for m in range(4):
        acc = ps_av()
        for k in range(KT):
            nc.tensor.matmul(acc[:PP, 0, :DH], cmbTr[k][:, m * PP:(m + 1) * PP],
                             wkvr[:, k, DH:], start=(k == 0), stop=(k == KT - 1))
        nc.vector.memset(mv_pad[m][:], 0.0)
        nc.vector.memset(mv_pad[m][:, DH:], 1.0)
        nc.vector.tensor_tensor(mv_pad[m][:PP, :DH], acc[:PP, 0, :DH], bvb[:PP, :],
                                OP.add)

    mqT_cur = None
    for h in range(H):
        po = (h % 2) * 64
        if h % 2 == 0:
            mqT_cur = make_mqT(h // 2)
        mqT_h = mqT_cur[po:po + 64, :]
        ep = []
        for mm in range(4):
            sp = ps_mm()
            nc.tensor.matmul(sp[:PP, :NP], mkT[po:po + 64, mm * PP:(mm + 1) * PP],
                             mqT_h[:], start=True, stop=True)
            et = utile(pE, [128, NP], F32, "e2", bufs=4)
            nc.scalar.activation(et[:PP, :], sp[:PP, :NP], AF.Exp,
                                 scale=float(DH ** -0.5))
            ep.append(et)
        av2 = ps_av().rearrange("p a b -> p (a b)")
        for mm in range(4):
            nc.tensor.matmul(av2[:DH + 1, :NP], mv_pad[mm][:PP, :],
                             ep[mm][:PP, :], start=(mm == 0), stop=(mm == 3))
        rrow = utile(pE, [1, NP], F32, "rrow2", bufs=2)
        nc.vector.reciprocal(rrow[:], av2[DH:DH + 1, :NP])
        rb = ps_mm()
        nc.tensor.matmul(rb[:DH, :NP], ones_col[:, :DH], rrow[:],
                         start=True, stop=True)
        rbs = utile(pE, [64, NP], F32, "rbs2", bufs=2)
        nc.vector.tensor_copy(rbs[:], rb[:DH, :NP])
        stg = utile(pE, [64, NP], F32R, "mqstg", bufs=2)
        nc.vector.tensor_tensor(stg[:], av2[:DH, :NP], rbs[:], OP.mult)
        nc.sync.dma_start(mqaTd[h * 64:h * 64 + 64, :], stg[:])
    pE.release()
    pD.release()

    # ================= Stage F: Wmo + FFN =================
    pF = tc.alloc_tile_pool(name="pF", bufs=1)
    mqaT = [utile(pF, [128, NP], F32R, f"mqaT{k}") for k in range(8)]
    for k in range(8):
        nc.sync.dma_start(mqaT[k][:, :NP], mqaTd[ts(k, 128), :])
    bmoT = bcol("bmo", pF)
    omoT = [utile(pF, [128, NP], F32R, f"omoT{m}") for m in range(8)]
    for mg in range(2):
        wcs = []
        for k in range(KT):
            wc = utile(pw, [128, 512], F32, "ws4k")
            nc.sync.dma_start(wc[:], wd["Wmo"][ts(k, 128), ts(mg, 512)])
            wcr = utile(pF, [128, 512], F32R, f"wmc{k}", bufs=1)
            nc.vector.tensor_copy(wcr[:], wc[:])
            wcs.append(wcr)
        for mi in range(4):
            m = mg * 4 + mi
            acc = ps_mm()
            for k in range(KT):
                nc.tensor.matmul(acc[:, :NP], wcs[k][:, ts(mi, 128)],
                                 mqaT[k][:], start=(k == 0), stop=(k == KT - 1))
            nc.scalar.activation(omoT[m][:], acc[:, :NP], AF.Identity,
                                 bias=bmoT[:, m:m + 1])

    bf1T = bcol("bf1", pF)
    bf2b = brow("bf2", pF)
    for half in range(2):
        t0 = half * 252
        oacc = [ps_bigA(), ps_bigB()]
        for kkg in range(8):
            wf1cs = []
            for k in range(KT):
                wf1c = utile(pF, [128, 512], F32, "w1c", bufs=3)
                nc.sync.dma_start(wf1c[:], wd["Wf1"][ts(k, 128), ts(kkg, 512)])
                wf1cr = utile(pF, [128, 512], F32R, f"w1cr{k}", bufs=1)
                nc.vector.tensor_copy(wf1cr[:], wf1c[:])
                wf1cs.append(wf1cr)
            for ki in range(4):
                kk = kkg * 4 + ki
                yp = ps_mm() if ki % 2 == 0 else                     ps_av().rearrange("p a b -> p (a b)")
                for k in range(KT):
                    nc.tensor.matmul(yp[:, :252], wf1cs[k][:, ts(ki, 128)],
                                     omoT[k][:, t0:t0 + 252],
                                     start=(k == 0), stop=(k == KT - 1))
                g2t = utile(pF, [128, 252], F32R, "g2t", bufs=3)
                nc.scalar.activation(g2t[:], yp[:, :252], AF.Silu,
                                     bias=bf1T[:, kk:kk + 1])
                wf2t = utile(pF, [128, C], F32, "w2s", bufs=3)
                nc.sync.dma_start(wf2t[:], wd["Wf2"][ts(kk, 128), :])
                wf2r = utile(pF, [128, C], F32R, "wf2r", bufs=3)
                nc.vector.tensor_copy(wf2r[:], wf2t[:])
                for tl in range(2):
                    for n2 in range(2):
                        nc.tensor.matmul(oacc[tl][:PP, ts(n2, 512)],
                                         g2t[:, tl * 126:tl * 126 + 126],
                                         wf2r[:, ts(n2, 512)],
                                         start=(kk == 0), stop=(kk == 31))
        for tl in range(2):
            row0 = (2 * half + tl) * PP
            of = utile(pF, [128, C], F32, "of", bufs=2)
            nc.vector.tensor_tensor(of[:PP, :], oacc[tl][:PP, :], bf2b[:PP, :],
                                    OP.add)
            nc.sync.dma_start(out_d[row0:row0 + PP, :], of[:PP, :])
    pF.release()
    for pool in (pt, pw, psm, pc, pp):
        pool.release()


_BUILT = None


def kernel(**inputs):
    global _BUILT
    if _BUILT is None:
        _BUILT = build(debug=DEBUG)
    nc = _BUILT
    x = np.ascontiguousarray(inputs["x"], dtype=np.float32)
    base = {k: np.ascontiguousarray(v, dtype=np.float32) for k, v in inputs.items()
            if k != "x"}
    in_maps = []
    for i in range(8):
        m = dict(base)
        m["x"] = x[i]
        in_maps.append(m)
    res = run_bass_kernel_spmd(nc, in_maps, core_ids=list(range(8)))
    out = np.stack([res.results[i]["out"] for i in range(8)], axis=0)
    return out.astype(np.float32)


# revision 46
# speedup vs baseline: 6052.6655x; 1.0070x over previous
"""AdaptiveTokenMerger (ToMe block + merger) TRN2 Bass kernel.

Data-parallel over batch: 8 samples -> 8 NeuronCores, one sample per core.
Per-core pipeline (sample x [1024, 1024]):
  A (f32, ranking-critical): LN1 -> qkv -> MHA (transposed-softmax with the
    denominator folded in as an appended ones-column of v) -> Wo -> x_attn
  B: metric scores -> node_max/argmax -> ranks via pairwise comparisons ->
    dst scatter-add expressed as a one-hot matmul
  C (f32r): MLP over rows [x1_even(512); dst_new(512)], fused W1/W2 per
    token-quarter, output accumulated in PSUM across all 32 W1 column tiles
  D: pooling as a rank-dependent one-hot matmul -> Wp -> combined = 3q
  E (f32r): multi-query attention  F (f32r): FFN -> out [504, 1024]

Precision: everything upstream of the rank/argmax decisions is true fp32
(4 cyc/row on PE); post-merge matmuls use float32r (TF32-ish, 1 cyc/row).

PSUM budget (8 banks): BIGA/BIGB [128,1024] (2+2), MM [128,512] x2 (2),
AV [128,4,128] x2 (2).
"""
import numpy as np

import concourse.bass as bass
import concourse.tile as tile
from concourse import bacc, mybir
from concourse.bass import ts
from concourse.bass_utils import run_bass_kernel_spmd
from concourse.masks import make_identity

F32 = mybir.dt.float32
F32R = mybir.dt.float32r
U32 = mybir.dt.uint32

N, C, H = 1024, 1024, 16
R = 16
DH = C // H          # 64
NE = N // 2          # 512
NP = (N - R) // 2    # 504
PP = 126             # pooled tokens per partition tile
KT = C // 128        # 8
AF = mybir.ActivationFunctionType
OP = mybir.AluOpType

DEBUG = False


def build(debug=False):
    nc = bacc.Bacc("TRN2", target_bir_lowering=False, debug=False, num_devices=8)
    x_d = nc.dram_tensor("x", [N, C], F32, kind="ExternalInput").ap()
    wd = {}
    for name, shape in [
        ("g1", [C]), ("be1", [C]), ("Wqkv", [C, 3 * C]), ("bqkv", [3 * C]),
        ("Wo", [C, C]), ("bo", [C]), ("g2", [C]), ("be2", [C]),
        ("W1", [C, 4 * C]), ("bm1", [4 * C]), ("W2", [4 * C, C]), ("bm2", [C]),
        ("Wp", [C, C]), ("bp", [C]), ("Wq", [C, C]), ("bq", [C]),
        ("Wk", [C, DH]), ("bk", [DH]), ("Wv", [C, DH]), ("bv", [DH]),
        ("Wmo", [C, C]), ("bmo", [C]), ("Wf1", [C, 4 * C]), ("bf1", [4 * C]),
        ("Wf2", [4 * C, C]), ("bf2", [C]),
    ]:
        wd[name] = nc.dram_tensor(name, shape, F32, kind="ExternalInput").ap()
    out_d = nc.dram_tensor("out", [NP, C], F32, kind="ExternalOutput").ap()
    dbg = {}
    if debug:
        for name, shape in [
            ("dbg_xattn", [N, C]), ("dbg_nm", [NE]), ("dbg_rank", [NE]),
            ("dbg_nodeidx", [NE]), ("dbg_mlpin", [N, C]), ("dbg_mlpout", [N, C]),
            ("dbg_pooled", [NP, C]),
        ]:
            dbg[name] = nc.dram_tensor(name, shape, F32, kind="ExternalOutput").ap()
    with tile.TileContext(nc) as tc:
        _build_tile(nc, tc, x_d, wd, out_d, dbg)
    nc.compile()
    return nc


def _build_tile(nc, tc, x_d, wd, out_d, dbg):
    # DRAM spill buffers
    qkTd = nc.dram_tensor("qkTd", [2 * C, N], F32).ap()
    aoTd = nc.dram_tensor("aoTd", [C, N], F32).ap()
    h2d = nc.dram_tensor("h2d", [C, N], F32R).ap()
    x1d = nc.dram_tensor("x1d", [N, C], F32).ap()
    dstnd = nc.dram_tensor("dstnd", [NE, C], F32).ap()
    mod = nc.dram_tensor("mod", [N, C], F32R).ap()
    mqaTd = nc.dram_tensor("mqaTd", [C, NP], F32R).ap()

    pc = tc.alloc_tile_pool(name="const", bufs=1)
    psm = tc.alloc_tile_pool(name="small", bufs=1)
    pw = tc.alloc_tile_pool(name="wstream", bufs=2)
    pt = tc.alloc_tile_pool(name="tmp", bufs=2)
    pp = tc.alloc_tile_pool(name="psum", bufs=1, space="PSUM")

    _ct = {}

    def utile(pool, shape, dtype, tag, bufs=None):
        _ct[tag] = _ct.get(tag, 0) + 1
        kw = {"bufs": bufs} if bufs is not None else {}
        return pool.tile(shape, dtype, tag=tag, name=f"{tag}_{_ct[tag]}", **kw)

    def ps_bigA():
        return utile(pp, [128, 1024], F32, "BIGA")

    def ps_bigB():
        return utile(pp, [128, 1024], F32, "BIGB")

    def ps_mm():
        return utile(pp, [128, 512], F32, "MM", bufs=2)

    def ps_av():
        return utile(pp, [128, 4, 128], F32, "AV", bufs=2)

    # ---------- constants ----------
    ident = pc.tile([128, 128], F32)
    make_identity(nc, ident[:])
    ones_col = pc.tile([1, 128], F32)
    nc.gpsimd.memset(ones_col[:], 1.0)
    piota = pc.tile([128, 1], F32)
    nc.gpsimd.iota(piota[:], [[0, 1]], channel_multiplier=1,
                   allow_small_or_imprecise_dtypes=True)
    iota512_row = pc.tile([1, 512], F32)
    nc.gpsimd.iota(iota512_row[:], [[1, 512]], channel_multiplier=0,
                   allow_small_or_imprecise_dtypes=True)
    iota504_row = pc.tile([1, 504], F32)
    nc.gpsimd.iota(iota504_row[:], [[1, 504]], channel_multiplier=0,
                   allow_small_or_imprecise_dtypes=True)

    def bcast_row(row_ap, n, tag, pool, scale=1.0):
        t = utile(pool, [128, n], F32, tag)
        for c0 in range(0, n, 512):
            cw = min(512, n - c0)
            p = ps_mm()
            nc.tensor.matmul(p[:, :cw], ones_col[:], row_ap[:, c0:c0 + cw],
                             start=True, stop=True)
            if scale == 1.0:
                nc.vector.tensor_copy(t[:, c0:c0 + cw], p[:, :cw])
            else:
                nc.vector.tensor_scalar_mul(t[:, c0:c0 + cw], p[:, :cw], scale)
        return t

    def load_row(dram_ap, n, tag, pool):
        t = utile(pw, [1, n], F32, "rowstg", bufs=1)
        nc.sync.dma_start(t[:], dram_ap[None, :])
        return t

    def brow(name, pool, scale=1.0):
        n = wd[name].shape[0]
        return bcast_row(load_row(wd[name], n, name + "_r", pool), n,
                         name + "_b", pool, scale)

    def bcol(name, pool, scale=1.0):
        n = wd[name].shape[0]
        t = utile(pool, [128, n // 128], F32, name + "_c")
        nc.sync.dma_start(t[:], wd[name].rearrange("(t p) -> p t", p=128))
        if scale != 1.0:
            nc.vector.tensor_scalar_mul(t[:], t[:], scale)
        return t

    IOTA512B = bcast_row(iota512_row[:], 512, "iota512b", pc)
    IOTA504B = bcast_row(iota504_row[:], 504, "iota504b", pc)

    def transpose_blocks(src_tiles, dst, n_rows, n_cols):
        """dst[c, r] = src[r, c]; dst is tile-list or sink(bj, bi, pf, cw, rw)."""
        for bi in range((n_rows + 127) // 128):
            rw = min(128, n_rows - bi * 128)
            for bj in range((n_cols + 127) // 128):
                cw = min(128, n_cols - bj * 128)
                p = ps_av()
                pf = p.rearrange("p a b -> p (a b)")
                nc.tensor.transpose(pf[:cw, :rw],
                                    src_tiles[bi][:rw, bj * 128:bj * 128 + cw],
                                    ident[:rw, :rw])
                if callable(dst):
                    dst(bj, bi, pf, cw, rw)
                else:
                    nc.vector.tensor_copy(dst[bj][:cw, bi * 128:bi * 128 + rw],
                                          pf[:cw, :rw])

    def refined_rsqrt_recip(vv, tag):
        """returns 1/sqrt(vv) with one Newton step on sqrt (ACT sqrt is loose)."""
        s0 = utile(psm, [128, 1], F32, tag + "_s0")
        nc.scalar.sqrt(s0[:], vv[:])
        r0 = utile(psm, [128, 1], F32, tag + "_r0")
        nc.vector.reciprocal(r0[:], s0[:])
        t = utile(psm, [128, 1], F32, tag + "_t")
        nc.vector.tensor_tensor(t[:], vv[:], r0[:], OP.mult)
        nc.vector.tensor_tensor(t[:], t[:], s0[:], OP.add)
        nc.vector.tensor_scalar_mul(t[:], t[:], 0.5)
        rr = utile(psm, [128, 1], F32, tag + "_rr")
        nc.vector.reciprocal(rr[:], t[:])
        return rr

    def layer_norm(src, dst, gb, bb):
        m = utile(psm, [128, 1], F32, "ln_m")
        nc.vector.reduce_sum(m[:], src[:, :C], axis=mybir.AxisListType.X)
        nc.vector.tensor_scalar_mul(m[:], m[:], 1.0 / C)
        xc = utile(pt, [128, C], F32, "ln_xc")
        nc.vector.tensor_scalar(xc[:], src[:, :C], m[:], None, OP.subtract)
        ss = utile(psm, [128, 1], F32, "ln_ss")
        nc.scalar.activation(dst[:, :C], xc[:], AF.Square, accum_out=ss[:])
        v = utile(psm, [128, 1], F32, "ln_v")
        nc.vector.tensor_scalar(v[:], ss[:], 1.0 / C, 1e-5, OP.mult, OP.add)
        rstd = refined_rsqrt_recip(v, "ln")
        nc.vector.tensor_scalar(dst[:, :C], xc[:], rstd[:], None, OP.mult)
        nc.vector.tensor_tensor(dst[:, :C], dst[:, :C], gb[:], OP.mult)
        nc.vector.tensor_tensor(dst[:, :C], dst[:, :C], bb[:], OP.add)

    # ================= Stage A: LN1 -> hT =================
    pbA = tc.alloc_tile_pool(name="biasA", bufs=1)
    pHT = tc.alloc_tile_pool(name="pHT", bufs=1)
    pVP = tc.alloc_tile_pool(name="pVP", bufs=1)
    pAttn = tc.alloc_tile_pool(name="pAttn", bufs=1)

    g1b = brow("g1", pbA)
    be1b = brow("be1", pbA)
    hT = [utile(pHT, [128, N], F32, f"hT{k}") for k in range(8)]
    ht = []
    for i in range(8):
        xt = utile(pt, [128, C], F32, "xin")
        nc.sync.dma_start(xt[:], x_d[ts(i, 128), :])
        h = utile(pt, [128, C], F32, "ht", bufs=4)
        layer_norm(xt, h, g1b, be1b)
        ht.append(h)
    transpose_blocks(ht, hT, N, C)

    # ===== qk^T -> qkTd (DRAM) ; v_pad (SBUF) =====
    bqkT = bcol("bqkv", pbA)
    for mp in range(8):
        accq = ps_bigA()
        acck = ps_bigB()
        for k in range(KT):
            wq = utile(pw, [128, 128], F32, "wqkb", bufs=4)
            nc.sync.dma_start(wq[:], wd["Wqkv"][ts(k, 128), ts(mp, 128)])
            wk = utile(pw, [128, 128], F32, "wqkb", bufs=4)
            nc.sync.dma_start(wk[:],
                              wd["Wqkv"][ts(k, 128), C + mp * 128:C + (mp + 1) * 128])
            for n2 in range(2):
                nc.tensor.matmul(accq[:, ts(n2, 512)], wq[:], hT[k][:, ts(n2, 512)],
                                 start=(k == 0), stop=(k == KT - 1))
                nc.tensor.matmul(acck[:, ts(n2, 512)], wk[:], hT[k][:, ts(n2, 512)],
                                 start=(k == 0), stop=(k == KT - 1))
        stgq = utile(pAttn, [128, N], F32, "qkstg", bufs=2)
        nc.scalar.activation(stgq[:], accq[:], AF.Identity, bias=bqkT[:, mp:mp + 1])
        nc.sync.dma_start(qkTd[ts(mp, 128), :], stgq[:])
        stgk = utile(pAttn, [128, N], F32, "qkstg", bufs=2)
        nc.scalar.activation(stgk[:], acck[:], AF.Identity,
                             bias=bqkT[:, 8 + mp:9 + mp])
        nc.sync.dma_start(qkTd[C + mp * 128:C + (mp + 1) * 128, :], stgk[:])

    bvqkvb = bcast_row(load_row(wd["bqkv"][2 * C:], C, "bvq_r", pbA), C,
                       "bvq_b", pbA)
    v_pad = [utile(pVP, [128, H, DH + 1], F32, f"vp{j}") for j in range(8)]
    for j in range(8):
        nc.vector.memset(v_pad[j][:, :, DH:DH + 1], 1.0)
        acc = ps_bigA()
        for k in range(KT):
            wv = utile(pVP, [128, C], F32, "wv", bufs=3)
            nc.sync.dma_start(wv[:], wd["Wqkv"][ts(k, 128), 2 * C:])
            for n2 in range(2):
                nc.tensor.matmul(acc[:, ts(n2, 512)], hT[k][:, ts(j, 128)],
                                 wv[:, ts(n2, 512)],
                                 start=(k == 0), stop=(k == KT - 1))
        for h in range(H):
            nc.vector.tensor_tensor(v_pad[j][:, h, :DH], acc[:, ts(h, DH)],
                                    bvqkvb[:, ts(h, DH)], OP.add)

    # ===== attention: stream kT/qT per head; out -> aoTd (already c-major) ==
    # out[dh|sum, i] = v_pad[j].T @ expT[j, i], accumulated over j-tiles.
    for h in range(H):
        kth = utile(pAttn, [64, N], F32, "kth", bufs=2)
        nc.sync.dma_start(kth[:], qkTd[C + h * 64:C + h * 64 + 64, :])
        qth = utile(pAttn, [64, N], F32, "qth", bufs=2)
        nc.sync.dma_start(qth[:], qkTd[h * 64:h * 64 + 64, :])
        av = [ps_av().rearrange("p a b -> p (a b)") for _ in range(2)]
        for j in range(8):
            for n2 in range(2):
                sp = ps_mm()
                nc.tensor.matmul(sp[:], kth[:, ts(j, 128)], qth[:, ts(n2, 512)],
                                 start=True, stop=True)
                et = utile(pAttn, [128, 512], F32, "exp", bufs=3)
                nc.scalar.activation(et[:], sp[:], AF.Exp, scale=float(DH ** -0.5))
                nc.tensor.matmul(av[n2][:DH + 1, :512], v_pad[j][:, h, :], et[:],
                                 start=(j == 0), stop=(j == 7))
        for n2 in range(2):
            rrow = utile(pAttn, [1, 512], F32, "rrow", bufs=2)
            nc.vector.reciprocal(rrow[:], av[n2][DH:DH + 1, :512])
            rb = ps_mm()
            nc.tensor.matmul(rb[:DH, :512], ones_col[:, :DH], rrow[:],
                             start=True, stop=True)
            rbs = utile(pAttn, [64, 512], F32, "rbs", bufs=2)
            nc.vector.tensor_copy(rbs[:], rb[:DH, :512])
            stg = utile(pAttn, [64, 512], F32, "aot_stg", bufs=2)
            nc.vector.tensor_tensor(stg[:], av[n2][:DH, :512], rbs[:],
                                    OP.mult)
            nc.sync.dma_start(aoTd[h * 64:h * 64 + 64, ts(n2, 512)], stg[:])
    pAttn.release()
    pVP.release()
    pHT.release()
    pbA.release()

    # ================= Wo -> x_attn, x1 (-> DRAM), metric =================
    pB2 = tc.alloc_tile_pool(name="pB2", bufs=1)
    bob = brow("bo", pB2)
    xa = [utile(pB2, [128, C], F32, f"xa{m}") for m in range(8)]
    woR = [utile(pB2, [128, C], F32, f"woR{k}") for k in range(8)]
    for k in range(KT):
        nc.sync.dma_start(woR[k][:], wd["Wo"][ts(k, 128), :])
    rn = psm.tile([128, 8], F32)
    for m in range(8):
        acc = ps_bigA()
        for k in range(KT):
            ao = utile(pw, [128, 128], F32, "wqkb", bufs=4)
            nc.sync.dma_start(ao[:], aoTd[ts(k, 128), ts(m, 128)])
            for n2 in range(2):
                nc.tensor.matmul(acc[:, ts(n2, 512)], ao[:],
                                 woR[k][:, ts(n2, 512)],
                                 start=(k == 0), stop=(k == KT - 1))
        nc.vector.tensor_tensor(xa[m][:], acc[:], bob[:], OP.add)
        xt = utile(pt, [128, C], F32, "xin")
        nc.sync.dma_start(xt[:], x_d[ts(m, 128), :])
        x1stg = utile(pw, [128, C], F32, "x1stg", bufs=2)
        nc.vector.tensor_tensor(x1stg[:], xa[m][:], xt[:], OP.add)
        nc.sync.dma_start(x1d[ts(m, 128), :], x1stg[:])
        ss = utile(psm, [128, 1], F32, "nrm_ss")
        sq = utile(pt, [128, C], F32, "ln_xc")
        nc.scalar.activation(sq[:], xa[m][:], AF.Square, accum_out=ss[:])
        rr = refined_rsqrt_recip(ss, "nrm")
        nc.vector.tensor_copy(rn[:, m:m + 1], rr[:])
        nc.vector.tensor_scalar(xa[m][:], xa[m][:], rn[:, m:m + 1], None, OP.mult)
        if dbg:
            nc.sync.dma_start(dbg["dbg_xattn"][ts(m, 128), :], xa[m][:])

    # ===== de-interleave metric -> maT/mbT; scores; node stats; ranks =====
    pB3 = tc.alloc_tile_pool(name="pB3", bufs=1)
    xae = [utile(pB3, [128, C], F32, f"xae{m}") for m in range(4)]
    xao = [utile(pB3, [128, C], F32, f"xao{m}") for m in range(4)]
    for m in range(4):
        nc.sync.dma_start(xae[m][:64, :], xa[2 * m][0:128:2, :])
        nc.sync.dma_start(xae[m][64:, :], xa[2 * m + 1][0:128:2, :])
        nc.sync.dma_start(xao[m][:64, :], xa[2 * m][1:128:2, :])
        nc.sync.dma_start(xao[m][64:, :], xa[2 * m + 1][1:128:2, :])
    pB4 = tc.alloc_tile_pool(name="pB4", bufs=1)
    maT = [utile(pB4, [128, NE], F32, f"maT{k}") for k in range(8)]
    mbT = [utile(pB4, [128, NE], F32, f"mbT{k}") for k in range(8)]
    transpose_blocks(xae, maT, NE, C)
    transpose_blocks(xao, mbT, NE, C)

    nm_t = psm.tile([128, 4], F32)
    ni_t = psm.tile([128, 4], F32)
    for m in range(4):
        acc = ps_bigA()
        for k in range(KT):
            nc.tensor.matmul(acc[:, :512], maT[k][:, ts(m, 128)], mbT[k][:],
                             start=(k == 0), stop=(k == KT - 1))
        mx8 = utile(psm, [128, 8], F32, "mx8")
        ix8 = utile(psm, [128, 8], U32, "ix8")
        nc.vector.max_with_indices(mx8[:], ix8[:], acc[:, :512])
        nc.vector.tensor_copy(nm_t[:, m:m + 1], mx8[:, 0:1])
        nc.vector.tensor_copy(ni_t[:, m:m + 1], ix8[:, 0:1])

    nm_row = utile(pB4, [1, 512], F32, "nm_row")
    for m in range(4):
        p = ps_av()
        pf = p.rearrange("p a b -> p (a b)")
        nc.tensor.transpose(pf[:1, :128], nm_t[:, m:m + 1], ident[:])
        nc.vector.tensor_copy(nm_row[:, ts(m, 128)], pf[:1, :128])
    NMB = bcast_row(nm_row[:], 512, "nmb", pB4)

    rank_t = psm.tile([128, 4], F32)
    for m in range(4):
        gt = utile(pB4, [128, 512], F32, "rk_gt", bufs=1)
        nc.vector.tensor_scalar(gt[:], NMB[:], nm_t[:, m:m + 1], None, OP.is_gt)
        eq = utile(pB4, [128, 512], F32, "rk_eq", bufs=1)
        nc.vector.tensor_scalar(eq[:], NMB[:], nm_t[:, m:m + 1], None, OP.is_equal)
        flt = utile(pB4, [128, 512], F32, "rk_flt", bufs=1)
        pio = utile(psm, [128, 1], F32, "rk_pio")
        nc.vector.tensor_scalar_add(pio[:], piota[:], float(128 * m))
        nc.vector.tensor_scalar(flt[:], IOTA512B[:], pio[:], None, OP.is_lt)
        nc.vector.tensor_tensor(eq[:], eq[:], flt[:], OP.mult)
        nc.vector.tensor_tensor(gt[:], gt[:], eq[:], OP.add)
        nc.vector.reduce_sum(rank_t[:, m:m + 1], gt[:], axis=mybir.AxisListType.X)
    if dbg:
        for (tt, nme) in [(nm_t, "dbg_nm"), (rank_t, "dbg_rank"),
                          (ni_t, "dbg_nodeidx")]:
            nc.sync.dma_start(dbg[nme].rearrange("(m p) -> p m", p=128), tt[:])
    pB4.release()
    pB3.release()
    pB2.release()

    # ================= dst merge (x1 from DRAM; dstn -> DRAM) =============
    pM = tc.alloc_tile_pool(name="pM", bufs=1)
    x1e = [utile(pM, [128, C + 8], F32, f"x1e{m}") for m in range(4)]
    x1o = [utile(pM, [128, C], F32, f"x1o{m}") for m in range(4)]
    for m in range(4):
        nc.vector.memset(x1e[m][:, C:C + 1], 1.0)
        nc.sync.dma_start(x1e[m][:, :C], x1d[256 * m:256 * m + 256:2, :])
        nc.sync.dma_start(x1o[m][:], x1d[256 * m + 1:256 * m + 256:2, :])
    st = [utile(pM, [128, 512], F32, f"st{m}") for m in range(4)]
    for m in range(4):
        msk = utile(psm, [128, 1], F32, "st_m")
        nc.vector.tensor_scalar(msk[:], rank_t[:, m:m + 1], float(R) - 0.5, None,
                                OP.is_lt)
        nc.vector.tensor_scalar(st[m][:], IOTA512B[:], ni_t[:, m:m + 1], None,
                                OP.is_equal)
        nc.vector.tensor_scalar(st[m][:], st[m][:], msk[:], None, OP.mult)
    for m in range(4):
        acc = ps_bigA()
        cacc = ps_av()
        for k in range(4):
            for n2 in range(2):
                nc.tensor.matmul(acc[:, ts(n2, 512)], st[k][:, ts(m, 128)],
                                 x1e[k][:, n2 * 512:n2 * 512 + 512],
                                 start=(k == 0), stop=(k == 3))
            nc.tensor.matmul(cacc[:, 0, :1], st[k][:, ts(m, 128)],
                             x1e[k][:, C:C + 1], start=(k == 0), stop=(k == 3))
        cnt = utile(psm, [128, 1], F32, "cnt")
        nc.vector.tensor_scalar_add(cnt[:], cacc[:, 0, 0:1], 1.0)
        rec = utile(psm, [128, 1], F32, "cntr")
        nc.vector.reciprocal(rec[:], cnt[:])
        dst_stg = utile(pM, [128, C], F32, "dst_stg", bufs=2)
        nc.vector.tensor_tensor(dst_stg[:], acc[:], x1o[m][:], OP.add)
        nc.vector.tensor_scalar(dst_stg[:], dst_stg[:], rec[:], None, OP.mult)
        nc.sync.dma_start(dstnd[ts(m, 128), :], dst_stg[:])

    # ========== MLP (f32r): W1/W2 streamed once; SBUF out accumulation ======
    def row_src_ap(i):
        if i < 4:
            return x1d[256 * i:256 * i + 256:2, :]
        return dstnd[ts(i - 4, 128), :]

    pM.release()
    pC4 = tc.alloc_tile_pool(name="pC4", bufs=1)
    g2b = brow("g2", pC4)
    be2b = brow("be2", pC4)
    h2 = []
    for i in range(8):
        rsrc = utile(pt, [128, C], F32, "xin")
        nc.sync.dma_start(rsrc[:], row_src_ap(i))
        h = utile(pt, [128, C], F32, "ht", bufs=4)
        layer_norm(rsrc, h, g2b, be2b)
        h2.append(h)
        if dbg:
            nc.sync.dma_start(dbg["dbg_mlpin"][ts(i, 128), :], rsrc[:])
    h2T = [utile(pC4, [128, N], F32R, f"h2T{k}") for k in range(8)]
    transpose_blocks(h2, h2T, N, C)

    bm1T = bcol("bm1", pC4)
    bm2b = brow("bm2", pC4)
    for q in range(4):
        oacc = [ps_bigA(), ps_bigB()]     # out token tiles 2q, 2q+1
        for mtg in range(8):
            w1cs = []
            for k in range(KT):
                w1c = utile(pC4, [128, 512], F32, "w1c", bufs=6)
                nc.sync.dma_start(w1c[:], wd["W1"][ts(k, 128), ts(mtg, 512)])
                w1cr = utile(pC4, [128, 512], F32R, f"w1cr{k}", bufs=2)
                nc.vector.tensor_copy(w1cr[:], w1c[:])
                w1cs.append(w1cr)
            for mi in range(4):
                mt = mtg * 4 + mi
                yp = ps_mm() if mi % 2 == 0 else                     ps_av().rearrange("p a b -> p (a b)")
                for k in range(KT):
                    nc.tensor.matmul(yp[:, :256], w1cs[k][:, ts(mi, 128)],
                                     h2T[k][:, q * 256:q * 256 + 256],
                                     start=(k == 0), stop=(k == KT - 1))
                g1t = utile(pC4, [128, 256], F32R, "g1t", bufs=3)
                nc.scalar.activation(g1t[:], yp[:, :256], AF.Gelu_apprx_tanh,
                                     bias=bm1T[:, mt:mt + 1])
                w2t = utile(pC4, [128, C], F32, "w2s", bufs=6)
                nc.sync.dma_start(w2t[:], wd["W2"][ts(mt, 128), :])
                w2r = utile(pC4, [128, C], F32R, "w2r", bufs=3)
                nc.vector.tensor_copy(w2r[:], w2t[:])
                for tl in range(2):
                    for n2 in range(2):
                        nc.tensor.matmul(oacc[tl][:, ts(n2, 512)],
                                         g1t[:, tl * 128:tl * 128 + 128],
                                         w2r[:, ts(n2, 512)],
                                         start=(mt == 0), stop=(mt == 31))
        for tl in range(2):
            row = 2 * q + tl
            res = utile(pt, [128, C], F32, "xin")
            nc.sync.dma_start(res[:], row_src_ap(row))
            mf = utile(pC4, [128, C], F32, "mof", bufs=2)
            nc.vector.tensor_tensor(mf[:], oacc[tl][:], bm2b[:], OP.add)
            nc.vector.tensor_tensor(mf[:], mf[:], res[:], OP.add)
            mr = utile(pC4, [128, C], F32R, "mor", bufs=2)
            nc.vector.tensor_copy(mr[:], mf[:])
            nc.sync.dma_start(mod[ts(row, 128), :], mr[:])
            if dbg:
                nc.sync.dma_start(dbg["dbg_mlpout"][ts(row, 128), :], mf[:])
    pC4.release()

    # ================= Stage D: pooling + Wp -> combined^T =================
    pD = tc.alloc_tile_pool(name="pD", bufs=1)
    # ApT[p, f] = 0.5 iff source row p pools into output f:
    #   even block: base = rank[p]-16, match iff (2f - base) in {-1, 0}
    #   dst  block: base = d,          match iff (2(f-248) - base) in {-1, 0}
    iota2e = utile(pD, [128, 504], F32, "iota2e")
    nc.vector.tensor_scalar_mul(iota2e[:], IOTA504B[:], 2.0)
    apT = [utile(pD, [128, 504], F32R, f"apT{m}") for m in range(8)]
    for m in range(8):
        base = utile(psm, [128, 1], F32, "ap_r")
        if m < 4:
            nc.vector.tensor_scalar_add(base[:], rank_t[:, m:m + 1], -float(R))
        else:
            nc.vector.tensor_scalar_add(base[:], piota[:],
                                        float(128 * (m - 4) + NE - R))
        d1 = utile(pD, [128, 504], F32, "ap_d1")
        nc.vector.tensor_scalar(d1[:], iota2e[:], base[:], None, OP.subtract)
        a1 = utile(pD, [128, 504], F32, "ap_a1")
        nc.vector.tensor_scalar(a1[:], d1[:], -1.5, None, OP.is_ge)
        b1 = utile(pD, [128, 504], F32, "ap_b1")
        nc.vector.tensor_scalar(b1[:], d1[:], 0.5, None, OP.is_le)
        nc.vector.scalar_tensor_tensor(apT[m][:], a1[:], 0.5, b1[:],
                                       OP.mult, OP.mult)
    pooledT = [utile(pD, [128, NP], F32R, f"pooledT{k}") for k in range(8)]
    for m in range(4):
        acc = ps_bigA()
        for k in range(8):
            mob = utile(pD, [128, C], F32R, "mob", bufs=3)
            nc.sync.dma_start(mob[:], mod[ts(k, 128), :])
            for n2 in range(2):
                nc.tensor.matmul(acc[:PP, ts(n2, 512)],
                                 apT[k][:, m * PP:(m + 1) * PP],
                                 mob[:, ts(n2, 512)], start=(k == 0), stop=(k == 7))
        pst = utile(pD, [128, C], F32, "pstg", bufs=2)
        nc.vector.tensor_copy(pst[:PP, :], acc[:PP, :])
        if dbg:
            nc.sync.dma_start(dbg["dbg_pooled"][ts(m, PP), :], pst[:PP, :])
        for bj in range(8):
            p = ps_av()
            pf = p.rearrange("p a b -> p (a b)")
            nc.tensor.transpose(pf[:128, :PP], pst[:PP, ts(bj, 128)],
                                ident[:PP, :PP])
            nc.vector.tensor_copy(pooledT[bj][:, m * PP:(m + 1) * PP],
                                  pf[:128, :PP])

    pE = tc.alloc_tile_pool(name="pE", bufs=1)
    bp3T = bcol("bp", pD, scale=3.0)
    cmbTr = [utile(pD, [128, NP], F32R, f"cmbTr{m}") for m in range(8)]
    for mg in range(2):
        wcs = []
        for k in range(KT):
            wc = utile(pw, [128, 512], F32, "ws4k")
            nc.sync.dma_start(wc[:], wd["Wp"][ts(k, 128), ts(mg, 512)])
            wcr = utile(pD, [128, 512], F32R, f"wpc{k}", bufs=1)
            nc.vector.tensor_copy(wcr[:], wc[:])
            wcs.append(wcr)
        for mi in range(4):
            m = mg * 4 + mi
            acc = ps_mm()
            for k in range(KT):
                nc.tensor.matmul(acc[:, :NP], wcs[k][:, ts(mi, 128)],
                                 pooledT[k][:], start=(k == 0), stop=(k == KT - 1))
            nc.scalar.activation(cmbTr[m][:], acc[:, :NP], AF.Identity,
                                 bias=bp3T[:, m:m + 1], scale=3.0)

    # ================= Stage E: MQA =================
    bqT = bcol("bq", pE)

    def make_mqT(m):
        acc = ps_mm()
        for k in range(KT):
            wb = utile(pw, [128, 128], F32, "w1b", bufs=4)
            nc.sync.dma_start(wb[:], wd["Wq"][ts(k, 128), ts(m, 128)])
            wr = utile(pw, [128, 128], F32R, "w1r", bufs=4)
            nc.vector.tensor_copy(wr[:], wb[:])
            nc.tensor.matmul(acc[:, :NP], wr[:], cmbTr[k][:],
                             start=(k == 0), stop=(k == KT - 1))
        t = utile(pE, [128, NP], F32R, "mqT", bufs=2)
        nc.scalar.activation(t[:], acc[:, :NP], AF.Identity, bias=bqT[:, m:m + 1])
        return t

    wkv = utile(pE, [128, KT, 2 * DH], F32, "wkv")
    wkvr = utile(pE, [128, KT, 2 * DH], F32R, "wkvr")
    for k in range(KT):
        nc.sync.dma_start(wkv[:, k, :DH], wd["Wk"][ts(k, 128), :])
        nc.sync.dma_start(wkv[:, k, DH:], wd["Wv"][ts(k, 128), :])
    nc.vector.tensor_copy(wkvr[:], wkv[:])
    bkT = utile(pE, [64, 1], F32, "bkT")
    nc.sync.dma_start(bkT[:], wd["bk"][:, None])
    mkT = utile(pE, [128, NP], F32R, "mkT")
    macc = ps_mm()
    for k in range(KT):
        nc.tensor.matmul(macc[:64, :NP], wkvr[:, k, :DH], cmbTr[k][:],
                         start=(k == 0), stop=(k == KT - 1))
    mkf = utile(pE, [64, NP], F32, "mkf")
    nc.scalar.activation(mkf[:], macc[:64, :NP], AF.Identity, bias=bkT[:])
    nc.vector.tensor_copy(mkT[:64, :], mkf[:])
    nc.sync.dma_start(mkT[64:, :], mkT[:64, :])

    bvb = bcast_row(load_row(wd["bv"], DH, "bv_r", pE), DH, "bv_b", pE)
    mv_pad = [utile(pE, [128, DH + 1], F32, f"mvp{m}") for m in range(4)]
    for m in range(4):
        acc = ps_av()
        for k in range(KT):
            nc.tensor.matmul(acc[:PP, 0, :DH], cmbTr[k][:, m * PP:(m + 1) * PP],
                             wkvr[:, k, DH:], start=(k == 0), stop=(k == KT - 1))
        nc.vector.memset(mv_pad[m][:], 0.0)
        nc.vector.memset(mv_pad[m][:, DH:], 1.0)
        nc.vector.tensor_tensor(mv_pad[m][:PP, :DH], acc[:PP, 0, :DH], bvb[:PP, :],
                                OP.add)

    mqT_cur = None
    for h in range(H):
        po = (h % 2) * 64
        if h % 2 == 0:
            mqT_cur = make_mqT(h // 2)
        mqT_h = mqT_cur[po:po + 64, :]
        ep = []
        for mm in range(4):
            sp = ps_mm()
            nc.tensor.matmul(sp[:PP, :NP], mkT[po:po + 64, mm * PP:(mm + 1) * PP],
                             mqT_h[:], start=True, stop=True)
            et = utile(pE, [128, NP], F32, "e2", bufs=4)
            nc.scalar.activation(et[:PP, :], sp[:PP, :NP], AF.Exp,
                                 scale=float(DH ** -0.5))
            ep.append(et)
        av2 = ps_av().rearrange("p a b -> p (a b)")
        for mm in range(4):
            nc.tensor.matmul(av2[:DH + 1, :NP], mv_pad[mm][:PP, :],
                             ep[mm][:PP, :], start=(mm == 0), stop=(mm == 3))
        rrow = utile(pE, [1, NP], F32, "rrow2", bufs=2)
        nc.vector.reciprocal(rrow[:], av2[DH:DH + 1, :NP])
        rb = ps_mm()
        nc.tensor.matmul(rb[:DH, :NP], ones_col[:, :DH], rrow[:],
                         start=True, stop=True)
        rbs = utile(pE, [64, NP], F32, "rbs2", bufs=2)
        nc.vector.tensor_copy(rbs[:], rb[:DH, :NP])
        stg = utile(pE, [64, NP], F32R, "mqstg", bufs=2)
        nc.vector.tensor_tensor(stg[:], av2[:DH, :NP], rbs[:], OP.mult)
        nc.sync.dma_start(mqaTd[h * 64:h * 64 + 64, :], stg[:])
    pE.release()
    pD.release()

    # ================= Stage F: Wmo + FFN =================
    pF = tc.alloc_tile_pool(name="pF", bufs=1)
    mqaT = [utile(pF, [128, NP], F32R, f"mqaT{k}") for k in range(8)]
    for k in range(8):
        nc.sync.dma_start(mqaT[k][:, :NP], mqaTd[ts(k, 128), :])
    bmoT = bcol("bmo", pF)
    omoT = [utile(pF, [128, NP], F32R, f"omoT{m}") for m in range(8)]
    for mg in range(2):
        wcs = []
        for k in range(KT):
            wc = utile(pw, [128, 512], F32, "ws4k")
            nc.sync.dma_start(wc[:], wd["Wmo"][ts(k, 128), ts(mg, 512)])
            wcr = utile(pF, [128, 512], F32R, f"wmc{k}", bufs=1)
            nc.vector.tensor_copy(wcr[:], wc[:])
            wcs.append(wcr)
        for mi in range(4):
            m = mg * 4 + mi
            acc = ps_mm()
            for k in range(KT):
                nc.tensor.matmul(acc[:, :NP], wcs[k][:, ts(mi, 128)],
                                 mqaT[k][:], start=(k == 0), stop=(k == KT - 1))
            nc.scalar.activation(omoT[m][:], acc[:, :NP], AF.Identity,
                                 bias=bmoT[:, m:m + 1])

    bf1T = bcol("bf1", pF)
    bf2b = brow("bf2", pF)
    for half in range(2):
        t0 = half * 252
        oacc = [ps_bigA(), ps_bigB()]
        for kkg in range(8):
            wf1cs = []
            for k in range(KT):
                wf1c = utile(pF, [128, 512], F32, "w1c", bufs=3)
                nc.sync.dma_start(wf1c[:], wd["Wf1"][ts(k, 128), ts(kkg, 512)])
                wf1cr = utile(pF, [128, 512], F32R, f"w1cr{k}", bufs=1)
                nc.vector.tensor_copy(wf1cr[:], wf1c[:])
                wf1cs.append(wf1cr)
            for ki in range(4):
                kk = kkg * 4 + ki
                yp = ps_mm() if ki % 2 == 0 else                     ps_av().rearrange("p a b -> p (a b)")
                for k in range(KT):
                    nc.tensor.matmul(yp[:, :252], wf1cs[k][:, ts(ki, 128)],
                                     omoT[k][:, t0:t0 + 252],
                                     start=(k == 0), stop=(k == KT - 1))
                g2t = utile(pF, [128, 252], F32R, "g2t", bufs=3)
                nc.scalar.activation(g2t[:], yp[:, :252], AF.Silu,
                                     bias=bf1T[:, kk:kk + 1])
                wf2t = utile(pF, [128, C], F32, "w2s", bufs=3)
                nc.sync.dma_start(wf2t[:], wd["Wf2"][ts(kk, 128), :])
                wf2r = utile(pF, [128, C], F32R, "wf2r", bufs=3)
                nc.vector.tensor_copy(wf2r[:], wf2t[:])
                for tl in range(2):
                    for n2 in range(2):
                        nc.tensor.matmul(oacc[tl][:PP, ts(n2, 512)],
                                         g2t[:, tl * 126:tl * 126 + 126],
                                         wf2r[:, ts(n2, 512)],
                                         start=(kk == 0), stop=(kk == 31))
        for tl in range(2):
            row0 = (2 * half + tl) * PP
            of = utile(pF, [128, C], F32, "of", bufs=2)
            nc.vector.tensor_tensor(of[:PP, :], oacc[tl][:PP, :], bf2b[:PP, :],
                                    OP.add)
            nc.sync.dma_start(out_d[row0:row0 + PP, :], of[:PP, :])
    pF.release()
    for pool in (pt, pw, psm, pc, pp):
        pool.release()


_BUILT = None


def kernel(**inputs):
    global _BUILT
    if _BUILT is None:
        _BUILT = build(debug=DEBUG)
    nc = _BUILT
    x = np.ascontiguousarray(inputs["x"], dtype=np.float32)
    base = {k: np.ascontiguousarray(v, dtype=np.float32) for k, v in inputs.items()
            if k != "x"}
    in_maps = []
    for i in range(8):
        m = dict(base)
        m["x"] = x[i]
        in_maps.append(m)
    res = run_bass_kernel_spmd(nc, in_maps, core_ids=list(range(8)))
    out = np.stack([res.results[i]["out"] for i in range(8)], axis=0)
    return out.astype(np.float32)


# revision 47
# speedup vs baseline: 6093.5205x; 1.0067x over previous
"""AdaptiveTokenMerger (ToMe block + merger) TRN2 Bass kernel.

Data-parallel over batch: 8 samples -> 8 NeuronCores, one sample per core.
Per-core pipeline (sample x [1024, 1024]):
  A (f32, ranking-critical): LN1 -> qkv -> MHA (transposed-softmax with the
    denominator folded in as an appended ones-column of v) -> Wo -> x_attn
  B: metric scores -> node_max/argmax -> ranks via pairwise comparisons ->
    dst scatter-add expressed as a one-hot matmul
  C (f32r): MLP over rows [x1_even(512); dst_new(512)], fused W1/W2 per
    token-quarter, output accumulated in PSUM across all 32 W1 column tiles
  D: pooling as a rank-dependent one-hot matmul -> Wp -> combined = 3q
  E (f32r): multi-query attention  F (f32r): FFN -> out [504, 1024]

Precision: everything upstream of the rank/argmax decisions is true fp32
(4 cyc/row on PE); post-merge matmuls use float32r (TF32-ish, 1 cyc/row).

PSUM budget (8 banks): BIGA/BIGB [128,1024] (2+2), MM [128,512] x2 (2),
AV [128,4,128] x2 (2).
"""
import numpy as np

import concourse.bass as bass
import concourse.tile as tile
from concourse import bacc, mybir
from concourse.bass import ts
from concourse.bass_utils import run_bass_kernel_spmd
from concourse.masks import make_identity

F32 = mybir.dt.float32
F32R = mybir.dt.float32r
U32 = mybir.dt.uint32

N, C, H = 1024, 1024, 16
R = 16
DH = C // H          # 64
NE = N // 2          # 512
NP = (N - R) // 2    # 504
PP = 126             # pooled tokens per partition tile
KT = C // 128        # 8
AF = mybir.ActivationFunctionType
OP = mybir.AluOpType

DEBUG = False


def build(debug=False):
    nc = bacc.Bacc("TRN2", target_bir_lowering=False, debug=False, num_devices=8)
    x_d = nc.dram_tensor("x", [N, C], F32, kind="ExternalInput").ap()
    wd = {}
    for name, shape in [
        ("g1", [C]), ("be1", [C]), ("Wqkv", [C, 3 * C]), ("bqkv", [3 * C]),
        ("Wo", [C, C]), ("bo", [C]), ("g2", [C]), ("be2", [C]),
        ("W1", [C, 4 * C]), ("bm1", [4 * C]), ("W2", [4 * C, C]), ("bm2", [C]),
        ("Wp", [C, C]), ("bp", [C]), ("Wq", [C, C]), ("bq", [C]),
        ("Wk", [C, DH]), ("bk", [DH]), ("Wv", [C, DH]), ("bv", [DH]),
        ("Wmo", [C, C]), ("bmo", [C]), ("Wf1", [C, 4 * C]), ("bf1", [4 * C]),
        ("Wf2", [4 * C, C]), ("bf2", [C]),
    ]:
        wd[name] = nc.dram_tensor(name, shape, F32, kind="ExternalInput").ap()
    out_d = nc.dram_tensor("out", [NP, C], F32, kind="ExternalOutput").ap()
    dbg = {}
    if debug:
        for name, shape in [
            ("dbg_xattn", [N, C]), ("dbg_nm", [NE]), ("dbg_rank", [NE]),
            ("dbg_nodeidx", [NE]), ("dbg_mlpin", [N, C]), ("dbg_mlpout", [N, C]),
            ("dbg_pooled", [NP, C]),
        ]:
            dbg[name] = nc.dram_tensor(name, shape, F32, kind="ExternalOutput").ap()
    with tile.TileContext(nc) as tc:
        _build_tile(nc, tc, x_d, wd, out_d, dbg)
    nc.compile()
    return nc


def _build_tile(nc, tc, x_d, wd, out_d, dbg):
    # DRAM spill buffers
    qkTd = nc.dram_tensor("qkTd", [2 * C, N], F32).ap()
    aoTd = nc.dram_tensor("aoTd", [C, N], F32).ap()
    h2d = nc.dram_tensor("h2d", [C, N], F32R).ap()
    x1d = nc.dram_tensor("x1d", [N, C], F32).ap()
    dstnd = nc.dram_tensor("dstnd", [NE, C], F32).ap()
    mod = nc.dram_tensor("mod", [N, C], F32R).ap()
    mqaTd = nc.dram_tensor("mqaTd", [C, NP], F32R).ap()

    pc = tc.alloc_tile_pool(name="const", bufs=1)
    psm = tc.alloc_tile_pool(name="small", bufs=1)
    pw = tc.alloc_tile_pool(name="wstream", bufs=2)
    pt = tc.alloc_tile_pool(name="tmp", bufs=2)
    pp = tc.alloc_tile_pool(name="psum", bufs=1, space="PSUM")

    _ct = {}

    def utile(pool, shape, dtype, tag, bufs=None):
        _ct[tag] = _ct.get(tag, 0) + 1
        kw = {"bufs": bufs} if bufs is not None else {}
        return pool.tile(shape, dtype, tag=tag, name=f"{tag}_{_ct[tag]}", **kw)

    def ps_bigA():
        return utile(pp, [128, 1024], F32, "BIGA")

    def ps_bigB():
        return utile(pp, [128, 1024], F32, "BIGB")

    def ps_mm():
        return utile(pp, [128, 512], F32, "MM", bufs=2)

    def ps_av():
        return utile(pp, [128, 4, 128], F32, "AV", bufs=2)

    # ---------- constants ----------
    ident = pc.tile([128, 128], F32)
    make_identity(nc, ident[:])
    ones_col = pc.tile([1, 128], F32)
    nc.gpsimd.memset(ones_col[:], 1.0)
    piota = pc.tile([128, 1], F32)
    nc.gpsimd.iota(piota[:], [[0, 1]], channel_multiplier=1,
                   allow_small_or_imprecise_dtypes=True)
    iota512_row = pc.tile([1, 512], F32)
    nc.gpsimd.iota(iota512_row[:], [[1, 512]], channel_multiplier=0,
                   allow_small_or_imprecise_dtypes=True)
    iota504_row = pc.tile([1, 504], F32)
    nc.gpsimd.iota(iota504_row[:], [[1, 504]], channel_multiplier=0,
                   allow_small_or_imprecise_dtypes=True)

    def bcast_row(row_ap, n, tag, pool, scale=1.0):
        t = utile(pool, [128, n], F32, tag)
        for c0 in range(0, n, 512):
            cw = min(512, n - c0)
            p = ps_mm()
            nc.tensor.matmul(p[:, :cw], ones_col[:], row_ap[:, c0:c0 + cw],
                             start=True, stop=True)
            if scale == 1.0:
                nc.vector.tensor_copy(t[:, c0:c0 + cw], p[:, :cw])
            else:
                nc.vector.tensor_scalar_mul(t[:, c0:c0 + cw], p[:, :cw], scale)
        return t

    def load_row(dram_ap, n, tag, pool):
        t = utile(pw, [1, n], F32, "rowstg", bufs=1)
        nc.sync.dma_start(t[:], dram_ap[None, :])
        return t

    def brow(name, pool, scale=1.0):
        n = wd[name].shape[0]
        return bcast_row(load_row(wd[name], n, name + "_r", pool), n,
                         name + "_b", pool, scale)

    def bcol(name, pool, scale=1.0):
        n = wd[name].shape[0]
        t = utile(pool, [128, n // 128], F32, name + "_c")
        nc.sync.dma_start(t[:], wd[name].rearrange("(t p) -> p t", p=128))
        if scale != 1.0:
            nc.vector.tensor_scalar_mul(t[:], t[:], scale)
        return t

    IOTA512B = bcast_row(iota512_row[:], 512, "iota512b", pc)
    IOTA504B = bcast_row(iota504_row[:], 504, "iota504b", pc)

    def transpose_blocks(src_tiles, dst, n_rows, n_cols):
        """dst[c, r] = src[r, c]; dst is tile-list or sink(bj, bi, pf, cw, rw)."""
        for bi in range((n_rows + 127) // 128):
            rw = min(128, n_rows - bi * 128)
            for bj in range((n_cols + 127) // 128):
                cw = min(128, n_cols - bj * 128)
                p = ps_av()
                pf = p.rearrange("p a b -> p (a b)")
                nc.tensor.transpose(pf[:cw, :rw],
                                    src_tiles[bi][:rw, bj * 128:bj * 128 + cw],
                                    ident[:rw, :rw])
                if callable(dst):
                    dst(bj, bi, pf, cw, rw)
                else:
                    nc.vector.tensor_copy(dst[bj][:cw, bi * 128:bi * 128 + rw],
                                          pf[:cw, :rw])

    def refined_rsqrt_recip(vv, tag):
        """returns 1/sqrt(vv) with one Newton step on sqrt (ACT sqrt is loose)."""
        s0 = utile(psm, [128, 1], F32, tag + "_s0")
        nc.scalar.sqrt(s0[:], vv[:])
        r0 = utile(psm, [128, 1], F32, tag + "_r0")
        nc.vector.reciprocal(r0[:], s0[:])
        t = utile(psm, [128, 1], F32, tag + "_t")
        nc.vector.tensor_tensor(t[:], vv[:], r0[:], OP.mult)
        nc.vector.tensor_tensor(t[:], t[:], s0[:], OP.add)
        nc.vector.tensor_scalar_mul(t[:], t[:], 0.5)
        rr = utile(psm, [128, 1], F32, tag + "_rr")
        nc.vector.reciprocal(rr[:], t[:])
        return rr

    def layer_norm(src, dst, gb, bb):
        m = utile(psm, [128, 1], F32, "ln_m")
        nc.vector.reduce_sum(m[:], src[:, :C], axis=mybir.AxisListType.X)
        nc.vector.tensor_scalar_mul(m[:], m[:], 1.0 / C)
        xc = utile(pt, [128, C], F32, "ln_xc")
        nc.vector.tensor_scalar(xc[:], src[:, :C], m[:], None, OP.subtract)
        ss = utile(psm, [128, 1], F32, "ln_ss")
        nc.scalar.activation(dst[:, :C], xc[:], AF.Square, accum_out=ss[:])
        v = utile(psm, [128, 1], F32, "ln_v")
        nc.vector.tensor_scalar(v[:], ss[:], 1.0 / C, 1e-5, OP.mult, OP.add)
        rstd = refined_rsqrt_recip(v, "ln")
        nc.vector.tensor_scalar(dst[:, :C], xc[:], rstd[:], None, OP.mult)
        nc.vector.tensor_tensor(dst[:, :C], dst[:, :C], gb[:], OP.mult)
        nc.vector.tensor_tensor(dst[:, :C], dst[:, :C], bb[:], OP.add)

    # ================= Stage A: LN1 -> hT =================
    pbA = tc.alloc_tile_pool(name="biasA", bufs=1)
    pHT = tc.alloc_tile_pool(name="pHT", bufs=1)
    pVP = tc.alloc_tile_pool(name="pVP", bufs=1)
    pAttn = tc.alloc_tile_pool(name="pAttn", bufs=1)

    g1b = brow("g1", pbA)
    be1b = brow("be1", pbA)
    hT = [utile(pHT, [128, N], F32, f"hT{k}") for k in range(8)]
    ht = []
    for i in range(8):
        xt = utile(pt, [128, C], F32, "xin")
        nc.sync.dma_start(xt[:], x_d[ts(i, 128), :])
        h = utile(pt, [128, C], F32, "ht", bufs=4)
        layer_norm(xt, h, g1b, be1b)
        ht.append(h)
    transpose_blocks(ht, hT, N, C)

    # ===== qk^T -> qkTd (DRAM) ; v_pad (SBUF) =====
    bqkT = bcol("bqkv", pbA)
    for mp in range(8):
        accq = ps_bigA()
        acck = ps_bigB()
        for k in range(KT):
            wq = utile(pw, [128, 128], F32, "wqkb", bufs=6)
            nc.sync.dma_start(wq[:], wd["Wqkv"][ts(k, 128), ts(mp, 128)])
            wk = utile(pw, [128, 128], F32, "wqkb", bufs=6)
            nc.sync.dma_start(wk[:],
                              wd["Wqkv"][ts(k, 128), C + mp * 128:C + (mp + 1) * 128])
            for n2 in range(2):
                nc.tensor.matmul(accq[:, ts(n2, 512)], wq[:], hT[k][:, ts(n2, 512)],
                                 start=(k == 0), stop=(k == KT - 1))
                nc.tensor.matmul(acck[:, ts(n2, 512)], wk[:], hT[k][:, ts(n2, 512)],
                                 start=(k == 0), stop=(k == KT - 1))
        stgq = utile(pAttn, [128, N], F32, "qkstg", bufs=2)
        nc.scalar.activation(stgq[:], accq[:], AF.Identity, bias=bqkT[:, mp:mp + 1])
        nc.sync.dma_start(qkTd[ts(mp, 128), :], stgq[:])
        stgk = utile(pAttn, [128, N], F32, "qkstg", bufs=2)
        nc.scalar.activation(stgk[:], acck[:], AF.Identity,
                             bias=bqkT[:, 8 + mp:9 + mp])
        nc.sync.dma_start(qkTd[C + mp * 128:C + (mp + 1) * 128, :], stgk[:])

    bvqkvb = bcast_row(load_row(wd["bqkv"][2 * C:], C, "bvq_r", pbA), C,
                       "bvq_b", pbA)
    v_pad = [utile(pVP, [128, H, DH + 1], F32, f"vp{j}") for j in range(8)]
    for j in range(8):
        nc.vector.memset(v_pad[j][:, :, DH:DH + 1], 1.0)
        acc = ps_bigA()
        for k in range(KT):
            wv = utile(pVP, [128, C], F32, "wv", bufs=3)
            nc.sync.dma_start(wv[:], wd["Wqkv"][ts(k, 128), 2 * C:])
            for n2 in range(2):
                nc.tensor.matmul(acc[:, ts(n2, 512)], hT[k][:, ts(j, 128)],
                                 wv[:, ts(n2, 512)],
                                 start=(k == 0), stop=(k == KT - 1))
        for h in range(H):
            nc.vector.tensor_tensor(v_pad[j][:, h, :DH], acc[:, ts(h, DH)],
                                    bvqkvb[:, ts(h, DH)], OP.add)

    # ===== attention: stream kT/qT per head; out -> aoTd (already c-major) ==
    # out[dh|sum, i] = v_pad[j].T @ expT[j, i], accumulated over j-tiles.
    for h in range(H):
        kth = utile(pAttn, [64, N], F32, "kth", bufs=2)
        nc.sync.dma_start(kth[:], qkTd[C + h * 64:C + h * 64 + 64, :])
        qth = utile(pAttn, [64, N], F32, "qth", bufs=2)
        nc.sync.dma_start(qth[:], qkTd[h * 64:h * 64 + 64, :])
        av = [ps_av().rearrange("p a b -> p (a b)") for _ in range(2)]
        for j in range(8):
            for n2 in range(2):
                sp = ps_mm()
                nc.tensor.matmul(sp[:], kth[:, ts(j, 128)], qth[:, ts(n2, 512)],
                                 start=True, stop=True)
                et = utile(pAttn, [128, 512], F32, "exp", bufs=3)
                nc.scalar.activation(et[:], sp[:], AF.Exp, scale=float(DH ** -0.5))
                nc.tensor.matmul(av[n2][:DH + 1, :512], v_pad[j][:, h, :], et[:],
                                 start=(j == 0), stop=(j == 7))
        for n2 in range(2):
            rrow = utile(pAttn, [1, 512], F32, "rrow", bufs=2)
            nc.vector.reciprocal(rrow[:], av[n2][DH:DH + 1, :512])
            rb = ps_mm()
            nc.tensor.matmul(rb[:DH, :512], ones_col[:, :DH], rrow[:],
                             start=True, stop=True)
            rbs = utile(pAttn, [64, 512], F32, "rbs", bufs=2)
            nc.vector.tensor_copy(rbs[:], rb[:DH, :512])
            stg = utile(pAttn, [64, 512], F32, "aot_stg", bufs=2)
            nc.vector.tensor_tensor(stg[:], av[n2][:DH, :512], rbs[:],
                                    OP.mult)
            nc.sync.dma_start(aoTd[h * 64:h * 64 + 64, ts(n2, 512)], stg[:])
    pAttn.release()
    pVP.release()
    pHT.release()
    pbA.release()

    # ================= Wo -> x_attn, x1 (-> DRAM), metric =================
    pB2 = tc.alloc_tile_pool(name="pB2", bufs=1)
    bob = brow("bo", pB2)
    xa = [utile(pB2, [128, C], F32, f"xa{m}") for m in range(8)]
    woR = [utile(pB2, [128, C], F32, f"woR{k}") for k in range(8)]
    for k in range(KT):
        nc.sync.dma_start(woR[k][:], wd["Wo"][ts(k, 128), :])
    rn = psm.tile([128, 8], F32)
    for m in range(8):
        acc = ps_bigA()
        for k in range(KT):
            ao = utile(pw, [128, 128], F32, "wqkb", bufs=6)
            nc.sync.dma_start(ao[:], aoTd[ts(k, 128), ts(m, 128)])
            for n2 in range(2):
                nc.tensor.matmul(acc[:, ts(n2, 512)], ao[:],
                                 woR[k][:, ts(n2, 512)],
                                 start=(k == 0), stop=(k == KT - 1))
        nc.vector.tensor_tensor(xa[m][:], acc[:], bob[:], OP.add)
        xt = utile(pt, [128, C], F32, "xin")
        nc.sync.dma_start(xt[:], x_d[ts(m, 128), :])
        x1stg = utile(pw, [128, C], F32, "x1stg", bufs=2)
        nc.vector.tensor_tensor(x1stg[:], xa[m][:], xt[:], OP.add)
        nc.sync.dma_start(x1d[ts(m, 128), :], x1stg[:])
        ss = utile(psm, [128, 1], F32, "nrm_ss")
        sq = utile(pt, [128, C], F32, "ln_xc")
        nc.scalar.activation(sq[:], xa[m][:], AF.Square, accum_out=ss[:])
        rr = refined_rsqrt_recip(ss, "nrm")
        nc.vector.tensor_copy(rn[:, m:m + 1], rr[:])
        nc.vector.tensor_scalar(xa[m][:], xa[m][:], rn[:, m:m + 1], None, OP.mult)
        if dbg:
            nc.sync.dma_start(dbg["dbg_xattn"][ts(m, 128), :], xa[m][:])

    # ===== de-interleave metric -> maT/mbT; scores; node stats; ranks =====
    pB3 = tc.alloc_tile_pool(name="pB3", bufs=1)
    xae = [utile(pB3, [128, C], F32, f"xae{m}") for m in range(4)]
    xao = [utile(pB3, [128, C], F32, f"xao{m}") for m in range(4)]
    for m in range(4):
        nc.sync.dma_start(xae[m][:64, :], xa[2 * m][0:128:2, :])
        nc.sync.dma_start(xae[m][64:, :], xa[2 * m + 1][0:128:2, :])
        nc.sync.dma_start(xao[m][:64, :], xa[2 * m][1:128:2, :])
        nc.sync.dma_start(xao[m][64:, :], xa[2 * m + 1][1:128:2, :])
    pB4 = tc.alloc_tile_pool(name="pB4", bufs=1)
    maT = [utile(pB4, [128, NE], F32, f"maT{k}") for k in range(8)]
    mbT = [utile(pB4, [128, NE], F32, f"mbT{k}") for k in range(8)]
    transpose_blocks(xae, maT, NE, C)
    transpose_blocks(xao, mbT, NE, C)

    nm_t = psm.tile([128, 4], F32)
    ni_t = psm.tile([128, 4], F32)
    for m in range(4):
        acc = ps_bigA()
        for k in range(KT):
            nc.tensor.matmul(acc[:, :512], maT[k][:, ts(m, 128)], mbT[k][:],
                             start=(k == 0), stop=(k == KT - 1))
        mx8 = utile(psm, [128, 8], F32, "mx8")
        ix8 = utile(psm, [128, 8], U32, "ix8")
        nc.vector.max_with_indices(mx8[:], ix8[:], acc[:, :512])
        nc.vector.tensor_copy(nm_t[:, m:m + 1], mx8[:, 0:1])
        nc.vector.tensor_copy(ni_t[:, m:m + 1], ix8[:, 0:1])

    nm_row = utile(pB4, [1, 512], F32, "nm_row")
    for m in range(4):
        p = ps_av()
        pf = p.rearrange("p a b -> p (a b)")
        nc.tensor.transpose(pf[:1, :128], nm_t[:, m:m + 1], ident[:])
        nc.vector.tensor_copy(nm_row[:, ts(m, 128)], pf[:1, :128])
    NMB = bcast_row(nm_row[:], 512, "nmb", pB4)

    rank_t = psm.tile([128, 4], F32)
    for m in range(4):
        gt = utile(pB4, [128, 512], F32, "rk_gt", bufs=1)
        nc.vector.tensor_scalar(gt[:], NMB[:], nm_t[:, m:m + 1], None, OP.is_gt)
        eq = utile(pB4, [128, 512], F32, "rk_eq", bufs=1)
        nc.vector.tensor_scalar(eq[:], NMB[:], nm_t[:, m:m + 1], None, OP.is_equal)
        flt = utile(pB4, [128, 512], F32, "rk_flt", bufs=1)
        pio = utile(psm, [128, 1], F32, "rk_pio")
        nc.vector.tensor_scalar_add(pio[:], piota[:], float(128 * m))
        nc.vector.tensor_scalar(flt[:], IOTA512B[:], pio[:], None, OP.is_lt)
        nc.vector.tensor_tensor(eq[:], eq[:], flt[:], OP.mult)
        nc.vector.tensor_tensor(gt[:], gt[:], eq[:], OP.add)
        nc.vector.reduce_sum(rank_t[:, m:m + 1], gt[:], axis=mybir.AxisListType.X)
    if dbg:
        for (tt, nme) in [(nm_t, "dbg_nm"), (rank_t, "dbg_rank"),
                          (ni_t, "dbg_nodeidx")]:
            nc.sync.dma_start(dbg[nme].rearrange("(m p) -> p m", p=128), tt[:])
    pB4.release()
    pB3.release()
    pB2.release()

    # ================= dst merge (x1 from DRAM; dstn -> DRAM) =============
    pM = tc.alloc_tile_pool(name="pM", bufs=1)
    x1e = [utile(pM, [128, C + 8], F32, f"x1e{m}") for m in range(4)]
    x1o = [utile(pM, [128, C], F32, f"x1o{m}") for m in range(4)]
    for m in range(4):
        nc.vector.memset(x1e[m][:, C:C + 1], 1.0)
        nc.sync.dma_start(x1e[m][:, :C], x1d[256 * m:256 * m + 256:2, :])
        nc.sync.dma_start(x1o[m][:], x1d[256 * m + 1:256 * m + 256:2, :])
    st = [utile(pM, [128, 512], F32, f"st{m}") for m in range(4)]
    for m in range(4):
        msk = utile(psm, [128, 1], F32, "st_m")
        nc.vector.tensor_scalar(msk[:], rank_t[:, m:m + 1], float(R) - 0.5, None,
                                OP.is_lt)
        nc.vector.tensor_scalar(st[m][:], IOTA512B[:], ni_t[:, m:m + 1], None,
                                OP.is_equal)
        nc.vector.tensor_scalar(st[m][:], st[m][:], msk[:], None, OP.mult)
    for m in range(4):
        acc = ps_bigA()
        cacc = ps_av()
        for k in range(4):
            for n2 in range(2):
                nc.tensor.matmul(acc[:, ts(n2, 512)], st[k][:, ts(m, 128)],
                                 x1e[k][:, n2 * 512:n2 * 512 + 512],
                                 start=(k == 0), stop=(k == 3))
            nc.tensor.matmul(cacc[:, 0, :1], st[k][:, ts(m, 128)],
                             x1e[k][:, C:C + 1], start=(k == 0), stop=(k == 3))
        cnt = utile(psm, [128, 1], F32, "cnt")
        nc.vector.tensor_scalar_add(cnt[:], cacc[:, 0, 0:1], 1.0)
        rec = utile(psm, [128, 1], F32, "cntr")
        nc.vector.reciprocal(rec[:], cnt[:])
        dst_stg = utile(pM, [128, C], F32, "dst_stg", bufs=2)
        nc.vector.tensor_tensor(dst_stg[:], acc[:], x1o[m][:], OP.add)
        nc.vector.tensor_scalar(dst_stg[:], dst_stg[:], rec[:], None, OP.mult)
        nc.sync.dma_start(dstnd[ts(m, 128), :], dst_stg[:])

    # ========== MLP (f32r): W1/W2 streamed once; SBUF out accumulation ======
    def row_src_ap(i):
        if i < 4:
            return x1d[256 * i:256 * i + 256:2, :]
        return dstnd[ts(i - 4, 128), :]

    pM.release()
    pC4 = tc.alloc_tile_pool(name="pC4", bufs=1)
    g2b = brow("g2", pC4)
    be2b = brow("be2", pC4)
    h2 = []
    for i in range(8):
        rsrc = utile(pt, [128, C], F32, "xin")
        nc.sync.dma_start(rsrc[:], row_src_ap(i))
        h = utile(pt, [128, C], F32, "ht", bufs=4)
        layer_norm(rsrc, h, g2b, be2b)
        h2.append(h)
        if dbg:
            nc.sync.dma_start(dbg["dbg_mlpin"][ts(i, 128), :], rsrc[:])
    h2T = [utile(pC4, [128, N], F32R, f"h2T{k}") for k in range(8)]
    transpose_blocks(h2, h2T, N, C)

    bm1T = bcol("bm1", pC4)
    bm2b = brow("bm2", pC4)
    for q in range(4):
        oacc = [ps_bigA(), ps_bigB()]     # out token tiles 2q, 2q+1
        for mtg in range(8):
            w1cs = []
            for k in range(KT):
                w1c = utile(pC4, [128, 512], F32, "w1c", bufs=6)
                nc.sync.dma_start(w1c[:], wd["W1"][ts(k, 128), ts(mtg, 512)])
                w1cr = utile(pC4, [128, 512], F32R, f"w1cr{k}", bufs=2)
                nc.vector.tensor_copy(w1cr[:], w1c[:])
                w1cs.append(w1cr)
            for mi in range(4):
                mt = mtg * 4 + mi
                yp = ps_mm() if mi % 2 == 0 else                     ps_av().rearrange("p a b -> p (a b)")
                for k in range(KT):
                    nc.tensor.matmul(yp[:, :256], w1cs[k][:, ts(mi, 128)],
                                     h2T[k][:, q * 256:q * 256 + 256],
                                     start=(k == 0), stop=(k == KT - 1))
                g1t = utile(pC4, [128, 256], F32R, "g1t", bufs=3)
                nc.scalar.activation(g1t[:], yp[:, :256], AF.Gelu_apprx_tanh,
                                     bias=bm1T[:, mt:mt + 1])
                w2t = utile(pC4, [128, C], F32, "w2s", bufs=6)
                nc.sync.dma_start(w2t[:], wd["W2"][ts(mt, 128), :])
                w2r = utile(pC4, [128, C], F32R, "w2r", bufs=3)
                nc.vector.tensor_copy(w2r[:], w2t[:])
                for tl in range(2):
                    for n2 in range(2):
                        nc.tensor.matmul(oacc[tl][:, ts(n2, 512)],
                                         g1t[:, tl * 128:tl * 128 + 128],
                                         w2r[:, ts(n2, 512)],
                                         start=(mt == 0), stop=(mt == 31))
        for tl in range(2):
            row = 2 * q + tl
            res = utile(pt, [128, C], F32, "xin")
            nc.sync.dma_start(res[:], row_src_ap(row))
            mf = utile(pC4, [128, C], F32, "mof", bufs=2)
            nc.vector.tensor_tensor(mf[:], oacc[tl][:], bm2b[:], OP.add)
            nc.vector.tensor_tensor(mf[:], mf[:], res[:], OP.add)
            mr = utile(pC4, [128, C], F32R, "mor", bufs=2)
            nc.vector.tensor_copy(mr[:], mf[:])
            nc.sync.dma_start(mod[ts(row, 128), :], mr[:])
            if dbg:
                nc.sync.dma_start(dbg["dbg_mlpout"][ts(row, 128), :], mf[:])
    pC4.release()

    # ================= Stage D: pooling + Wp -> combined^T =================
    pD = tc.alloc_tile_pool(name="pD", bufs=1)
    # ApT[p, f] = 0.5 iff source row p pools into output f:
    #   even block: base = rank[p]-16, match iff (2f - base) in {-1, 0}
    #   dst  block: base = d,          match iff (2(f-248) - base) in {-1, 0}
    iota2e = utile(pD, [128, 504], F32, "iota2e")
    nc.vector.tensor_scalar_mul(iota2e[:], IOTA504B[:], 2.0)
    apT = [utile(pD, [128, 504], F32R, f"apT{m}") for m in range(8)]
    for m in range(8):
        base = utile(psm, [128, 1], F32, "ap_r")
        if m < 4:
            nc.vector.tensor_scalar_add(base[:], rank_t[:, m:m + 1], -float(R))
        else:
            nc.vector.tensor_scalar_add(base[:], piota[:],
                                        float(128 * (m - 4) + NE - R))
        d1 = utile(pD, [128, 504], F32, "ap_d1")
        nc.vector.tensor_scalar(d1[:], iota2e[:], base[:], None, OP.subtract)
        a1 = utile(pD, [128, 504], F32, "ap_a1")
        nc.vector.tensor_scalar(a1[:], d1[:], -1.5, None, OP.is_ge)
        b1 = utile(pD, [128, 504], F32, "ap_b1")
        nc.vector.tensor_scalar(b1[:], d1[:], 0.5, None, OP.is_le)
        nc.vector.scalar_tensor_tensor(apT[m][:], a1[:], 0.5, b1[:],
                                       OP.mult, OP.mult)
    pooledT = [utile(pD, [128, NP], F32R, f"pooledT{k}") for k in range(8)]
    for m in range(4):
        acc = ps_bigA()
        for k in range(8):
            mob = utile(pD, [128, C], F32R, "mob", bufs=4)
            nc.sync.dma_start(mob[:], mod[ts(k, 128), :])
            for n2 in range(2):
                nc.tensor.matmul(acc[:PP, ts(n2, 512)],
                                 apT[k][:, m * PP:(m + 1) * PP],
                                 mob[:, ts(n2, 512)], start=(k == 0), stop=(k == 7))
        pst = utile(pD, [128, C], F32, "pstg", bufs=2)
        nc.vector.tensor_copy(pst[:PP, :], acc[:PP, :])
        if dbg:
            nc.sync.dma_start(dbg["dbg_pooled"][ts(m, PP), :], pst[:PP, :])
        for bj in range(8):
            p = ps_av()
            pf = p.rearrange("p a b -> p (a b)")
            nc.tensor.transpose(pf[:128, :PP], pst[:PP, ts(bj, 128)],
                                ident[:PP, :PP])
            nc.vector.tensor_copy(pooledT[bj][:, m * PP:(m + 1) * PP],
                                  pf[:128, :PP])

    pE = tc.alloc_tile_pool(name="pE", bufs=1)
    bp3T = bcol("bp", pD, scale=3.0)
    cmbTr = [utile(pD, [128, NP], F32R, f"cmbTr{m}") for m in range(8)]
    for mg in range(2):
        wcs = []
        for k in range(KT):
            wc = utile(pw, [128, 512], F32, "ws4k")
            nc.sync.dma_start(wc[:], wd["Wp"][ts(k, 128), ts(mg, 512)])
            wcr = utile(pD, [128, 512], F32R, f"wpc{k}", bufs=1)
            nc.vector.tensor_copy(wcr[:], wc[:])
            wcs.append(wcr)
        for mi in range(4):
            m = mg * 4 + mi
            acc = ps_mm()
            for k in range(KT):
                nc.tensor.matmul(acc[:, :NP], wcs[k][:, ts(mi, 128)],
                                 pooledT[k][:], start=(k == 0), stop=(k == KT - 1))
            nc.scalar.activation(cmbTr[m][:], acc[:, :NP], AF.Identity,
                                 bias=bp3T[:, m:m + 1], scale=3.0)

    # ================= Stage E: MQA =================
    bqT = bcol("bq", pE)

    def make_mqT(m):
        acc = ps_mm()
        for k in range(KT):
            wb = utile(pw, [128, 128], F32, "w1b", bufs=4)
            nc.sync.dma_start(wb[:], wd["Wq"][ts(k, 128), ts(m, 128)])
            wr = utile(pw, [128, 128], F32R, "w1r", bufs=4)
            nc.vector.tensor_copy(wr[:], wb[:])
            nc.tensor.matmul(acc[:, :NP], wr[:], cmbTr[k][:],
                             start=(k == 0), stop=(k == KT - 1))
        t = utile(pE, [128, NP], F32R, "mqT", bufs=2)
        nc.scalar.activation(t[:], acc[:, :NP], AF.Identity, bias=bqT[:, m:m + 1])
        return t

    wkv = utile(pE, [128, KT, 2 * DH], F32, "wkv")
    wkvr = utile(pE, [128, KT, 2 * DH], F32R, "wkvr")
    for k in range(KT):
        nc.sync.dma_start(wkv[:, k, :DH], wd["Wk"][ts(k, 128), :])
        nc.sync.dma_start(wkv[:, k, DH:], wd["Wv"][ts(k, 128), :])
    nc.vector.tensor_copy(wkvr[:], wkv[:])
    bkT = utile(pE, [64, 1], F32, "bkT")
    nc.sync.dma_start(bkT[:], wd["bk"][:, None])
    mkT = utile(pE, [128, NP], F32R, "mkT")
    macc = ps_mm()
    for k in range(KT):
        nc.tensor.matmul(macc[:64, :NP], wkvr[:, k, :DH], cmbTr[k][:],
                         start=(k == 0), stop=(k == KT - 1))
    mkf = utile(pE, [64, NP], F32, "mkf")
    nc.scalar.activation(mkf[:], macc[:64, :NP], AF.Identity, bias=bkT[:])
    nc.vector.tensor_copy(mkT[:64, :], mkf[:])
    nc.sync.dma_start(mkT[64:, :], mkT[:64, :])

    bvb = bcast_row(load_row(wd["bv"], DH, "bv_r", pE), DH, "bv_b", pE)
    mv_pad = [utile(pE, [128, DH + 1], F32, f"mvp{m}") for m in range(4)]
    for m in range(4):
        acc = ps_av()
        for k in range(KT):
            nc.tensor.matmul(acc[:PP, 0, :DH], cmbTr[k][:, m * PP:(m + 1) * PP],
                             wkvr[:, k, DH:], start=(k == 0), stop=(k == KT - 1))
        nc.vector.memset(mv_pad[m][:], 0.0)
        nc.vector.memset(mv_pad[m][:, DH:], 1.0)
        nc.vector.tensor_tensor(mv_pad[m][:PP, :DH], acc[:PP, 0, :DH], bvb[:PP, :],
                                OP.add)

    mqT_cur = None
    for h in range(H):
        po = (h % 2) * 64
        if h % 2 == 0:
            mqT_cur = make_mqT(h // 2)
        mqT_h = mqT_cur[po:po + 64, :]
        ep = []
        for mm in range(4):
            sp = ps_mm()
            nc.tensor.matmul(sp[:PP, :NP], mkT[po:po + 64, mm * PP:(mm + 1) * PP],
                             mqT_h[:], start=True, stop=True)
            et = utile(pE, [128, NP], F32, "e2", bufs=4)
            nc.scalar.activation(et[:PP, :], sp[:PP, :NP], AF.Exp,
                                 scale=float(DH ** -0.5))
            ep.append(et)
        av2 = ps_av().rearrange("p a b -> p (a b)")
        for mm in range(4):
            nc.tensor.matmul(av2[:DH + 1, :NP], mv_pad[mm][:PP, :],
                             ep[mm][:PP, :], start=(mm == 0), stop=(mm == 3))
        rrow = utile(pE, [1, NP], F32, "rrow2", bufs=2)
        nc.vector.reciprocal(rrow[:], av2[DH:DH + 1, :NP])
        rb = ps_mm()
        nc.tensor.matmul(rb[:DH, :NP], ones_col[:, :DH], rrow[:],
                         start=True, stop=True)
        rbs = utile(pE, [64, NP], F32, "rbs2", bufs=2)
        nc.vector.tensor_copy(rbs[:], rb[:DH, :NP])
        stg = utile(pE, [64, NP], F32R, "mqstg", bufs=2)
        nc.vector.tensor_tensor(stg[:], av2[:DH, :NP], rbs[:], OP.mult)
        nc.sync.dma_start(mqaTd[h * 64:h * 64 + 64, :], stg[:])
    pE.release()
    pD.release()

    # ================= Stage F: Wmo + FFN =================
    pF = tc.alloc_tile_pool(name="pF", bufs=1)
    mqaT = [utile(pF, [128, NP], F32R, f"mqaT{k}") for k in range(8)]
    for k in range(8):
        nc.sync.dma_start(mqaT[k][:, :NP], mqaTd[ts(k, 128), :])
    bmoT = bcol("bmo", pF)
    omoT = [utile(pF, [128, NP], F32R, f"omoT{m}") for m in range(8)]
    for mg in range(2):
        wcs = []
        for k in range(KT):
            wc = utile(pw, [128, 512], F32, "ws4k")
            nc.sync.dma_start(wc[:], wd["Wmo"][ts(k, 128), ts(mg, 512)])
            wcr = utile(pF, [128, 512], F32R, f"wmc{k}", bufs=1)
            nc.vector.tensor_copy(wcr[:], wc[:])
            wcs.append(wcr)
        for mi in range(4):
            m = mg * 4 + mi
            acc = ps_mm()
            for k in range(KT):
                nc.tensor.matmul(acc[:, :NP], wcs[k][:, ts(mi, 128)],
                                 mqaT[k][:], start=(k == 0), stop=(k == KT - 1))
            nc.scalar.activation(omoT[m][:], acc[:, :NP], AF.Identity,
                                 bias=bmoT[:, m:m + 1])

    bf1T = bcol("bf1", pF)
    bf2b = brow("bf2", pF)
    for half in range(2):
        t0 = half * 252
        oacc = [ps_bigA(), ps_bigB()]
        for kkg in range(8):
            wf1cs = []
            for k in range(KT):
                wf1c = utile(pF, [128, 512], F32, "w1c", bufs=6)
                nc.sync.dma_start(wf1c[:], wd["Wf1"][ts(k, 128), ts(kkg, 512)])
                wf1cr = utile(pF, [128, 512], F32R, f"w1cr{k}", bufs=1)
                nc.vector.tensor_copy(wf1cr[:], wf1c[:])
                wf1cs.append(wf1cr)
            for ki in range(4):
                kk = kkg * 4 + ki
                yp = ps_mm() if ki % 2 == 0 else                     ps_av().rearrange("p a b -> p (a b)")
                for k in range(KT):
                    nc.tensor.matmul(yp[:, :252], wf1cs[k][:, ts(ki, 128)],
                                     omoT[k][:, t0:t0 + 252],
                                     start=(k == 0), stop=(k == KT - 1))
                g2t = utile(pF, [128, 252], F32R, "g2t", bufs=3)
                nc.scalar.activation(g2t[:], yp[:, :252], AF.Silu,
                                     bias=bf1T[:, kk:kk + 1])
                wf2t = utile(pF, [128, C], F32, "w2s", bufs=6)
                nc.sync.dma_start(wf2t[:], wd["Wf2"][ts(kk, 128), :])
                wf2r = utile(pF, [128, C], F32R, "wf2r", bufs=3)
                nc.vector.tensor_copy(wf2r[:], wf2t[:])
                for tl in range(2):
                    for n2 in range(2):
                        nc.tensor.matmul(oacc[tl][:PP, ts(n2, 512)],
                                         g2t[:, tl * 126:tl * 126 + 126],
                                         wf2r[:, ts(n2, 512)],
                                         start=(kk == 0), stop=(kk == 31))
        for tl in range(2):
            row0 = (2 * half + tl) * PP
            of = utile(pF, [128, C], F32, "of", bufs=2)
            nc.vector.tensor_tensor(of[:PP, :], oacc[tl][:PP, :], bf2b[:PP, :],
                                    OP.add)
            nc.sync.dma_start(out_d[row0:row0 + PP, :], of[:PP, :])
    pF.release()
    for pool in (pt, pw, psm, pc, pp):
        pool.release()


_BUILT = None


def kernel(**inputs):
    global _BUILT
    if _BUILT is None:
        _BUILT = build(debug=DEBUG)
    nc = _BUILT
    x = np.ascontiguousarray(inputs["x"], dtype=np.float32)
    base = {k: np.ascontiguousarray(v, dtype=np.float32) for k, v in inputs.items()
            if k != "x"}
    in_maps = []
    for i in range(8):
        m = dict(base)
        m["x"] = x[i]
        in_maps.append(m)
    res = run_bass_kernel_spmd(nc, in_maps, core_ids=list(range(8)))
    out = np.stack([res.results[i]["out"] for i in range(8)], axis=0)
    return out.astype(np.float32)
